# revision 1
# baseline (speedup 1.0000x reference)
"""Bass/Trainium2 kernel for nn_BiLSTM_Tok_83837761618147.

Strategy (8 NeuronCores, SPMD, full inputs in / full output out):
  - Token dim sharded 8 ways (16384 tokens/core, with halos).
  - BiLSTM parallelized via chunked recurrence with burn-in: each core runs
    128 lanes x (128+64) steps forward and 128 lanes x (129+64) steps
    backward (state forgets exponentially; 64 warmup steps reach fp32
    accuracy; the true h0/c0-seeded lanes cover the sequence ends exactly).
  - Gate pre-activations computed by PE matmuls directly into PSUM
    (bias via a K=4 indicator matmul); w_hh @ h accumulated on top.
  - Attention (tanh/logits/exp) + ragged segment softmax-sum done on
    device via an e-weighted one-hot (token x segment-window) matmul.
  - Host combines per-core partial [segment, 257] sums, normalizes, and
    applies the tiny tag projection.
"""

import numpy as np
import ml_dtypes

BF16 = ml_dtypes.bfloat16

T = 131072
D = 256
H = 128
HID = 256
TAGS = 10
S = 1024
NCORE = 8
PC = T // NCORE          # 16384 tokens per core
B = 64                   # burn-in steps
LF = 128                 # forward lane length (tokens per lane)
LB = 129                 # backward lane length
NL = 128                 # lanes per direction
NSF = B + LF             # 192 forward steps
NSB = B + LB             # 193 backward steps
SH = 16640               # x shard rows [tc0-64, tc0-64+SH)
SWIN = 256               # segment window width per core
NQ = PC                  # main attention window positions
NTILE = NQ // 128        # 128 main token tiles
HBW = LB * NL - LB + LB + B  # hbT width: 16512
HBT_W = 16512
ATT_W = NQ + 128         # att buffer width (main + extra tile)

_BUILT = {}
LAST_RESULT = None


def _build():
    if "nc" in _BUILT:
        return _BUILT["nc"]
    import contextlib
    from concourse import bacc, mybir
    from concourse.tile import TileContext

    F32 = mybir.dt.float32
    BF = mybir.dt.bfloat16
    AF = mybir.ActivationFunctionType
    ALU = mybir.AluOpType

    nc = bacc.Bacc()

    def din(name, shape, dt):
        return nc.declare_dram_parameter(name, list(shape), dt, isOutput=False)

    x_in = din("xT", [256, SH], BF)
    wih_f_in = din("wih_f", [256, 512], BF)
    wih_b_in = din("wih_b", [256, 512], BF)
    whh_f_in = din("whh_f", [128, 512], BF)
    whh_b_in = din("whh_b", [128, 512], BF)
    b4_f_in = din("b4_f", [128, 128], BF)
    b4_b_in = din("b4_b", [128, 128], BF)
    i4_in = din("i4", [128, 1024], BF)
    h0f_in = din("h0f", [128, 128], BF)
    c0f_in = din("c0f", [128, 128], BF)
    h0b_in = din("h0b", [128, 128], BF)
    c0b_in = din("c0b", [128, 128], BF)
    wom_in = din("wom", [256, 256], BF)
    uo_in = din("uo", [256, 1], BF)
    iota_in = din("iota", [128, 256], F32)
    identb_in = din("identb", [128, 128], BF)
    identf_in = din("identf", [128, 128], F32)
    seg_in = din("seg", [128, 129], F32)
    ctx_out = nc.declare_dram_parameter("ctx", [256, 257], F32, isOutput=True)
    att_dram = nc.dram_tensor("att_stage", [1, ATT_W], F32)

    with TileContext(nc) as tc, contextlib.ExitStack() as ctx:
        pp = ctx.enter_context(tc.tile_pool(name="persist", bufs=1))

        xT0 = pp.tile([128, SH], BF, tag="xT0", name="xT0")
        xT1 = pp.tile([128, SH], BF, tag="xT1", name="xT1")
        hfT = pp.tile([128, NQ], BF, tag="hfT", name="hfT")
        hbT = pp.tile([128, HBT_W], BF, tag="hbT", name="hbT")
        hf_head = pp.tile([128, 64], BF, tag="hfh", name="hfh")
        hb_head = pp.tile([128, 64], BF, tag="hbh", name="hbh")
        scr = [[pp.tile([128, 128], BF, tag=f"scr{d}{i}", name=f"scr{d}{i}") for i in range(2)]
               for d in range(2)]
        wih = [[pp.tile([128, 512], BF, tag=f"wih{d}{k}", name=f"wih{d}{k}") for k in range(2)]
               for d in range(2)]
        whh = [pp.tile([128, 512], BF, tag=f"whh{d}", name=f"whh{d}") for d in range(2)]
        b4 = [pp.tile([128, 128], BF, tag=f"b4{d}", name=f"b4{d}") for d in range(2)]
        i4 = pp.tile([128, 1024], BF, tag="i4", name="i4")
        h0 = [pp.tile([128, 128], BF, tag=f"h0{d}", name=f"h0{d}") for d in range(2)]
        c0 = [pp.tile([128, 128], BF, tag=f"c0{d}", name=f"c0{d}") for d in range(2)]
        wom = [pp.tile([128, 256], BF, tag=f"wom{k}", name=f"wom{k}") for k in range(2)]
        uo = [pp.tile([128, 1], BF, tag=f"uo{k}", name=f"uo{k}") for k in range(2)]
        iota_t = pp.tile([128, 256], F32, tag="iota", name="iota")
        identb = pp.tile([128, 128], BF, tag="identb", name="identb")
        identf = pp.tile([128, 128], F32, tag="identf", name="identf")
        seg_t = pp.tile([128, 129], F32, tag="seg", name="seg")
        CFB = pp.tile([128, 256], BF, tag="CFB", name="CFB")
        e_cm = pp.tile([128, 129], F32, tag="ecm", name="ecm")
        hfx = pp.tile([128, 128], BF, tag="hfx", name="hfx")
        hbx = pp.tile([128, 128], BF, tag="hbx", name="hbx")
        ctx_sb = [pp.tile([128, 257], F32, tag=f"ctxsb{k}", name=f"ctxsb{k}") for k in range(2)]

        # ---- input DMAs ----
        nc.sync.dma_start(xT0[:], x_in[0:128, :])
        nc.sync.dma_start(xT1[:], x_in[128:256, :])
        for d, t_ in ((0, wih_f_in), (1, wih_b_in)):
            nc.sync.dma_start(wih[d][0][:], t_[0:128, :])
            nc.sync.dma_start(wih[d][1][:], t_[128:256, :])
        nc.sync.dma_start(whh[0][:], whh_f_in[:])
        nc.sync.dma_start(whh[1][:], whh_b_in[:])
        nc.sync.dma_start(b4[0][:], b4_f_in[:])
        nc.sync.dma_start(b4[1][:], b4_b_in[:])
        nc.sync.dma_start(i4[:], i4_in[:])
        nc.sync.dma_start(h0[0][:], h0f_in[:])
        nc.sync.dma_start(c0[0][:], c0f_in[:])
        nc.sync.dma_start(h0[1][:], h0b_in[:])
        nc.sync.dma_start(c0[1][:], c0b_in[:])
        nc.sync.dma_start(wom[0][:], wom_in[0:128, :])
        nc.sync.dma_start(wom[1][:], wom_in[128:256, :])
        nc.sync.dma_start(uo[0][:], uo_in[0:128, :])
        nc.sync.dma_start(uo[1][:], uo_in[128:256, :])
        nc.sync.dma_start(iota_t[:], iota_in[:])
        nc.sync.dma_start(identb[:], identb_in[:])
        nc.sync.dma_start(identf[:], identf_in[:])
        nc.sync.dma_start(seg_t[:], seg_in[:])

        # init cell state from seeds: CFB = [c0f | c0b]
        nc.vector.tensor_copy(CFB[:, 0:128], c0[0][:])
        nc.vector.tensor_copy(CFB[:, 128:256], c0[1][:])

        xT = [xT0, xT1]

        def fwd_pre_rhs(kh, s0):
            # cols {128m + 64 + s0 + ds}, ds in {0,1}, m in [0,128)
            base = 64 + s0
            v = xT[kh][:, base:base + 16384]
            v = v.rearrange("p (m b) -> p b m", b=128)
            return v[:, 0:2, :]

        def bwd_pre_rhs(kh, s):
            # backward lane k' reads x col 193 + 129*k' - s, k' in [0,128)
            a = 193 - s
            return xT[kh][:, a:a + 129 * 127 + 1:129]

        def h_src(d, s):
            # h state produced at step s-1 (read at step s)
            if s == 0:
                return h0[d][:]
            sp = s - 1
            if sp < B:
                return scr[d][sp % 2][:]
            if d == 0:
                return hfT[:, sp - 64:sp - 64 + 127 * 128 + 1:128]
            a = 192 - sp
            return hbT[:, a:a + 129 * 127 + 1:129]

        def h_dst(d, s):
            if s < B:
                return scr[d][s % 2][:]
            if d == 0:
                if s == 192:
                    return scr[0][0][:]
                return hfT[:, s - 64:s - 64 + 127 * 128 + 1:128]
            a = 192 - s
            return hbT[:, a:a + 129 * 127 + 1:129]

        with tc.tile_pool(name="psG", bufs=4, space="PSUM") as psg, \
             tc.tile_pool(name="sig", bufs=3) as sigp, \
             tc.tile_pool(name="tg", bufs=3) as tgp, \
             tc.tile_pool(name="tcn", bufs=3) as tcp, \
             tc.tile_pool(name="tmp1", bufs=3) as t1p, \
             tc.tile_pool(name="tmp2", bufs=3) as t2p:

            G = {}

            def emit_pre(s_):
                # pre-gate + bias matmuls for step s_ (both dirs)
                if s_ >= NSB:
                    return
                g = psg.tile([128, 1024], F32, tag="G", name="G")
                G[s_] = g
                for d in range(2):
                    do = d * 512
                    nc.tensor.matmul(g[:, do:do + 512], b4[d][:], i4[:, 0:512],
                                     start=True, stop=False)
                    for kh in range(2):
                        for j in range(4):
                            if d == 0:
                                if s_ >= NSF:
                                    continue
                                base = 64 + s_
                                rhs = xT[kh][:, base:base + 127 * 128 + 1:128]
                            else:
                                rhs = bwd_pre_rhs(kh, s_)
                            nc.tensor.matmul(
                                g[:, do + 128 * j:do + 128 * j + 128],
                                wih[d][kh][:, 128 * j:128 * j + 128],
                                rhs, start=False, stop=False)

            for s_ in range(3):
                emit_pre(s_)

            for s in range(NSB):
                g = G.pop(s)
                emit_pre(s + 3)
                # w_hh matmuls (accumulate into this step's gate region)
                for d in range(2):
                    if d == 0 and s >= NSF:
                        continue
                    hs = h_src(d, s)
                    for j in range(4):
                        nc.tensor.matmul(
                            g[:, 512 * d + 128 * j:512 * d + 128 * j + 128],
                            whh[d][:, 128 * j:128 * j + 128], hs,
                            start=False, stop=True)
                # gates
                sig = sigp.tile([128, 768], BF, tag="sig", name="sig")
                src_sig = g[:].rearrange("p (a q) -> p a q", q=512)[:, :, 0:384]
                dst_sig = sig[:].rearrange("p (a q) -> p a q", q=384)
                nc.scalar.activation(dst_sig, src_sig, AF.Sigmoid)
                tg = tgp.tile([128, 256], BF, tag="tg", name="tg")
                src_tg = g[:].rearrange("p (a q) -> p a q", q=512)[:, :, 384:512]
                nc.scalar.activation(tg[:].rearrange("p (a q) -> p a q", q=128),
                                     src_tg, AF.Tanh)
                # c update
                sigr = sig[:].rearrange("p (a q) -> p a q", q=384)
                t1 = t1p.tile([128, 256], BF, tag="t1", name="t1")
                t2 = t2p.tile([128, 256], BF, tag="t2", name="t2")
                cr = CFB[:].rearrange("p (a q) -> p a q", q=128)
                nc.vector.tensor_tensor(t1[:].rearrange("p (a q) -> p a q", q=128),
                                        sigr[:, :, 128:256], cr, ALU.mult)
                nc.vector.tensor_tensor(t2[:].rearrange("p (a q) -> p a q", q=128),
                                        sigr[:, :, 0:128],
                                        tg[:].rearrange("p (a q) -> p a q", q=128),
                                        ALU.mult)
                nc.vector.tensor_tensor(CFB[:], t1[:], t2[:], ALU.add)
                tcn = tcp.tile([128, 256], BF, tag="tcn", name="tcn")
                nc.scalar.activation(tcn[:], CFB[:], AF.Tanh)
                # h = sigma_o * tanh(c), written straight to its storage slot
                for d in range(2):
                    if d == 0 and s >= NSF:
                        continue
                    nc.vector.tensor_tensor(h_dst(d, s),
                                            sig[:, 384 * d + 256:384 * d + 384],
                                            tcn[:, 128 * d:128 * d + 128],
                                            ALU.mult)
                if s < B:
                    nc.vector.tensor_copy(hf_head[:, s:s + 1],
                                          scr[0][s % 2][:, 0:1])
                    nc.vector.tensor_copy(hb_head[:, 63 - s:64 - s],
                                          scr[1][s % 2][:, 126:127])

        # ---------------- attention phase ----------------
        # assemble extra window tiles
        nc.vector.tensor_copy(hfx[:, 0:64], hf_head[:])
        nc.vector.tensor_copy(hfx[:, 64:128], hfT[:, 16256:16320])
        nc.vector.tensor_copy(hbx[:, 0:64], hbT[:, 63:127])
        nc.vector.tensor_copy(hbx[:, 64:128], hb_head[:])

        with tc.tile_pool(name="psU", bufs=2, space="PSUM") as psu, \
             tc.tile_pool(name="uT", bufs=2) as utp, \
             tc.tile_pool(name="psA", bufs=2, space="PSUM") as psa:
            for gidx in range(33):
                if gidx < 32:
                    n = 512
                    hfr = hfT[:, 512 * gidx:512 * gidx + 512]
                    hbr = hbT[:, 512 * gidx + 127:512 * gidx + 127 + 512]
                    aout = att_dram[0:1, 512 * gidx:512 * gidx + 512]
                else:
                    n = 128
                    hfr = hfx[:]
                    hbr = hbx[:]
                    aout = att_dram[0:1, NQ:NQ + 128]
                pa = psa.tile([1, 512], F32, tag="psA", name="psA")
                for c2 in range(2):
                    pu = psu.tile([128, 512], F32, tag="psU", name="psU")
                    nc.tensor.matmul(pu[:, 0:n], wom[0][:, 128 * c2:128 * c2 + 128],
                                     hfr, start=True, stop=False)
                    nc.tensor.matmul(pu[:, 0:n], wom[1][:, 128 * c2:128 * c2 + 128],
                                     hbr, start=False, stop=True)
                    ut = utp.tile([128, 512], BF, tag="uT", name="uT")
                    nc.scalar.activation(ut[:, 0:n], pu[:, 0:n], AF.Tanh)
                    nc.tensor.matmul(pa[0:1, 0:n], uo[c2][:], ut[:, 0:n],
                                     start=(c2 == 0), stop=(c2 == 1))
                asb = utp.tile([1, 512], F32, tag="asb", name="asb")
                nc.vector.tensor_copy(asb[0:1, 0:n], pa[0:1, 0:n])
                nc.sync.dma_start(aout, asb[0:1, 0:n])

        # att -> column-major e
        with tc.tile_pool(name="psT", bufs=2, space="PSUM") as pst, \
             tc.tile_pool(name="anm", bufs=1) as anmp:
            att_nm = anmp.tile([128, 128], F32, tag="anm", name="anm")
            nc.sync.dma_start(
                att_nm[:],
                att_dram[0:1, 0:NQ].rearrange("a (n p) -> (a n) p", p=128))
            ps_a = pst.tile([128, 128], F32, tag="psT", name="psT")
            nc.tensor.transpose(ps_a[:], att_nm[:], identf[:])
            nc.scalar.activation(e_cm[:, 0:128], ps_a[:], AF.Exp)
            att_x = anmp.tile([128, 1], F32, tag="attx", name="attx")
            nc.sync.dma_start(
                att_x[:],
                att_dram[0:1, NQ:NQ + 128].rearrange("a (n p) -> (a n) p", p=1))
            nc.scalar.activation(e_cm[:, 128:129], att_x[:], AF.Exp)

        # ragged context accumulation
        with tc.tile_pool(name="psT2", bufs=2, space="PSUM") as pst2, \
             tc.tile_pool(name="yp", bufs=2) as yp, \
             tc.tile_pool(name="iw", bufs=2) as iwp, \
             tc.tile_pool(name="psC", bufs=1, space="PSUM") as psc:
            ctxp = [psc.tile([128, 257], F32, tag=f"ctxp{k}", name=f"ctxp{k}") for k in range(2)]
            for nti in range(NTILE + 1):
                if nti < NTILE:
                    hfr = hfT[:, 128 * nti:128 * nti + 128]
                    hbr = hbT[:, 128 * nti + 127:128 * nti + 255]
                else:
                    hfr = hfx[:]
                    hbr = hbx[:]
                ps_t = pst2.tile([128, 256], BF, tag="psT2", name="psT2")
                nc.tensor.transpose(ps_t[:, 0:128], hfr, identb[:])
                nc.tensor.transpose(ps_t[:, 128:256], hbr, identb[:])
                y = yp.tile([128, 257], BF, tag="y", name="y")
                nc.vector.tensor_copy(y[:, 0:256], ps_t[:])
                nc.vector.memset(y[:, 256:257], 1.0)
                iw = iwp.tile([128, 256], BF, tag="iw", name="iw")
                nc.vector.tensor_scalar(iw[:], iota_t[:],
                                        seg_t[:, nti:nti + 1],
                                        e_cm[:, nti:nti + 1],
                                        ALU.is_equal, ALU.mult)
                for k in range(2):
                    nc.tensor.matmul(ctxp[k][:], iw[:, 128 * k:128 * k + 128],
                                     y[:], start=(nti == 0), stop=(nti == NTILE))
            for k in range(2):
                nc.vector.tensor_copy(ctx_sb[k][:], ctxp[k][:])
        for k in range(2):
            nc.sync.dma_start(ctx_out[128 * k:128 * k + 128, :], ctx_sb[k][:])

    nc.finalize()
    _BUILT["nc"] = nc
    return nc


def _host_prep(inputs):
    x = np.asarray(inputs["sentence"], np.float32)
    doc_mask = np.asarray(inputs["doc_mask"]).astype(np.int64)
    h0g = np.asarray(inputs["h0"], np.float32)
    c0g = np.asarray(inputs["c0"], np.float32)

    perm = np.r_[0:128, 128:256, 384:512, 256:384]  # i,f,o,g order

    def wprep(w):  # [4H, X] -> lhsT [X, 4H] with gate perm, bf16
        return np.ascontiguousarray(w.astype(np.float32).T[:, perm]).astype(BF16)

    wih = {d: wprep(np.asarray(inputs[f"w_ih_{s}"], np.float32))
           for d, s in ((0, "f"), (1, "b"))}
    whh = {d: wprep(np.asarray(inputs[f"w_hh_{s}"], np.float32))
           for d, s in ((0, "f"), (1, "b"))}
    bias = {d: (np.asarray(inputs[f"b_ih_{s}"], np.float32)
                + np.asarray(inputs[f"b_hh_{s}"], np.float32))[perm]
            for d, s in ((0, "f"), (1, "b"))}
    b4 = {}
    for d in range(2):
        m = np.zeros((128, 128), np.float32)
        for k in range(4):
            m[k, :] = bias[d][128 * k:128 * k + 128]
        b4[d] = m.astype(BF16)
    i4 = np.zeros((128, 1024), np.float32)
    for r in range(2):
        for k in range(4):
            i4[k, 512 * r + 128 * k: 512 * r + 128 * k + 128] = 1.0
    i4 = i4.astype(BF16)

    wom = np.asarray(inputs["w_omega"], np.float32).astype(BF16)
    uo = np.asarray(inputs["u_omega"], np.float32).astype(BF16)
    iota = np.tile(np.arange(256, dtype=np.float32), (128, 1))
    identb = np.eye(128, dtype=np.float32).astype(BF16)
    identf = np.eye(128, dtype=np.float32)

    seg_global = np.searchsorted(doc_mask, np.arange(T), side="right")

    in_maps = []
    s_los = []
    xpad = np.zeros((T + 512, D), np.float32)
    xpad[64:64 + T] = x  # global row r ↔ token r - 64
    for c in range(NCORE):
        tc0 = c * PC
        xs = xpad[tc0:tc0 + SH]  # token tc0-64+i at row i
        xT = np.ascontiguousarray(xs.T).astype(BF16)

        # seeds
        h0f = np.zeros((128, 128), np.float32)
        c0f = np.zeros((128, 128), np.float32)
        h0b = np.zeros((128, 128), np.float32)
        c0b = np.zeros((128, 128), np.float32)
        if c == 0:
            h0f[:, 0] = h0g[0]
            c0f[:, 0] = c0g[0]
        if c == NCORE - 1:
            h0b[:, 126] = h0g[1]
            c0b[:, 126] = c0g[1]

        # segment ids, col-major [128, 129]
        segm = np.full((128, 129), -1.0, np.float32)
        toks_main = tc0 + 64 + np.arange(NQ)
        valid = toks_main < T
        if c == NCORE - 1:
            valid &= (np.arange(NQ) < 16256)  # tail handled by W_tail
        toks_extra = np.full(128, -1, np.int64)
        if c == 0:
            toks_extra[0:64] = np.arange(64)          # W_head: tokens [0,64)
        if c == NCORE - 1:
            toks_extra[64:128] = T - 64 + np.arange(64)  # W_tail
        all_toks = np.concatenate([toks_main[valid],
                                   toks_extra[toks_extra >= 0]])
        s_lo = int(seg_global[all_toks].min()) if all_toks.size else 0
        s_hi = int(seg_global[all_toks].max()) if all_toks.size else 0
        assert s_hi - s_lo < SWIN, f"segment window too wide: {s_hi - s_lo}"
        s_los.append(s_lo)
        sm = np.where(valid, seg_global[np.minimum(toks_main, T - 1)] - s_lo,
                      -1.0).astype(np.float32)
        segm[:, 0:128] = sm.reshape(128, 128).T  # segm[p, n] = seg(q=128n+p)
        se = np.full(128, -1.0, np.float32)
        mask_x = toks_extra >= 0
        se[mask_x] = seg_global[toks_extra[mask_x]] - s_lo
        segm[:, 128] = se

        in_maps.append({
            "xT": xT,
            "wih_f": wih[0], "wih_b": wih[1],
            "whh_f": whh[0], "whh_b": whh[1],
            "b4_f": b4[0], "b4_b": b4[1], "i4": i4,
            "h0f": h0f.astype(BF16), "c0f": c0f.astype(BF16),
            "h0b": h0b.astype(BF16), "c0b": c0b.astype(BF16),
            "wom": wom, "uo": uo, "iota": iota,
            "identb": identb, "identf": identf,
            "seg": segm,
        })
    return in_maps, s_los


def kernel(**inputs):
    global LAST_RESULT
    from concourse.bass_utils import run_bass_kernel_spmd

    nc = _build()
    in_maps, s_los = _host_prep(inputs)
    res = run_bass_kernel_spmd(nc, in_maps, core_ids=list(range(NCORE)))
    LAST_RESULT = res

    G = np.zeros((S + SWIN, 257), np.float64)
    for c in range(NCORE):
        ctx = np.asarray(res.results[c]["ctx"], np.float32)
        G[s_los[c]:s_los[c] + SWIN] += ctx
    G = G[:S]
    z = G[:, 256]
    ctx = G[:, :256] / np.where(z == 0, 1.0, z)[:, None]
    w_tag = np.asarray(inputs["w_tag"], np.float32)
    b_tag = np.asarray(inputs["b_tag"], np.float32)
    out = ctx.astype(np.float32) @ w_tag.T + b_tag
    return out.astype(np.float32)



# revision 5
# speedup vs baseline: 1.0796x; 1.0796x over previous
"""Bass/Trainium2 kernel for nn_BiLSTM_Tok_83837761618147.

Strategy (8 NeuronCores, SPMD, full inputs in / full output out):
  - Token dim sharded 8 ways (16384 tokens/core, with halos).
  - BiLSTM parallelized via chunked recurrence with burn-in: each core runs
    128 lanes x (128+64) steps forward and 128 lanes x (129+64) steps
    backward (state forgets exponentially; 64 warmup steps reach fp32
    accuracy; the true h0/c0-seeded lanes cover the sequence ends exactly).
  - Gate pre-activations computed by PE matmuls directly into PSUM
    (bias via a K=4 indicator matmul); w_hh @ h accumulated on top.
  - Attention (tanh/logits/exp) + ragged segment softmax-sum done on
    device via an e-weighted one-hot (token x segment-window) matmul.
  - Host combines per-core partial [segment, 257] sums, normalizes, and
    applies the tiny tag projection.
"""

import numpy as np
import ml_dtypes

BF16 = ml_dtypes.bfloat16

T = 131072
D = 256
H = 128
HID = 256
TAGS = 10
S = 1024
NCORE = 8
PC = T // NCORE          # 16384 tokens per core
B = 64                   # burn-in steps
LF = 128                 # forward lane length (tokens per lane)
LB = 129                 # backward lane length
NL = 128                 # lanes per direction
NSF = B + LF             # 192 forward steps
NSB = B + LB             # 193 backward steps
SH = 16640               # x shard rows [tc0-64, tc0-64+SH)
SWIN = 256               # segment window width per core
NQ = PC                  # main attention window positions
NTILE = NQ // 128        # 128 main token tiles
HBW = LB * NL - LB + LB + B  # hbT width: 16512
HBT_W = 16512
ATT_W = NQ + 128         # att buffer width (main + extra tile)

_BUILT = {}
LAST_RESULT = None


def _build():
    if "nc" in _BUILT:
        return _BUILT["nc"]
    import contextlib
    from concourse import bacc, mybir
    from concourse.tile import TileContext

    F32 = mybir.dt.float32
    BF = mybir.dt.bfloat16
    AF = mybir.ActivationFunctionType
    ALU = mybir.AluOpType

    nc = bacc.Bacc()

    def din(name, shape, dt):
        return nc.declare_dram_parameter(name, list(shape), dt, isOutput=False)

    x_in = din("xT", [256, SH], BF)
    wih_f_in = din("wih_f", [256, 512], BF)
    wih_b_in = din("wih_b", [256, 512], BF)
    whh_f_in = din("whh_f", [128, 512], BF)
    whh_b_in = din("whh_b", [128, 512], BF)
    b4_f_in = din("b4_f", [128, 128], BF)
    b4_b_in = din("b4_b", [128, 128], BF)
    i4_in = din("i4", [128, 1024], BF)
    h0f_in = din("h0f", [128, 128], BF)
    c0f_in = din("c0f", [128, 128], BF)
    h0b_in = din("h0b", [128, 128], BF)
    c0b_in = din("c0b", [128, 128], BF)
    wom_in = din("wom", [256, 256], BF)
    uo_in = din("uo", [256, 1], BF)
    iota_in = din("iota", [128, 256], F32)
    identb_in = din("identb", [128, 128], BF)
    identf_in = din("identf", [128, 128], F32)
    seg_in = din("seg", [128, 129], F32)
    ctx_out = nc.declare_dram_parameter("ctx", [256, 257], F32, isOutput=True)
    att_dram = nc.dram_tensor("att_stage", [1, ATT_W], F32)

    with TileContext(nc) as tc, contextlib.ExitStack() as ctx:
        pp = ctx.enter_context(tc.tile_pool(name="persist", bufs=1))

        xT0 = pp.tile([128, SH], BF, tag="xT0", name="xT0")
        xT1 = pp.tile([128, SH], BF, tag="xT1", name="xT1")
        hfT = pp.tile([128, NQ], BF, tag="hfT", name="hfT")
        hbT = pp.tile([128, HBT_W], BF, tag="hbT", name="hbT")
        hf_head = pp.tile([128, 64], BF, tag="hfh", name="hfh")
        hb_head = pp.tile([128, 64], BF, tag="hbh", name="hbh")
        wih = [[pp.tile([128, 512], BF, tag=f"wih{d}{k}", name=f"wih{d}{k}") for k in range(2)]
               for d in range(2)]
        whh = [pp.tile([128, 512], BF, tag=f"whh{d}", name=f"whh{d}") for d in range(2)]
        b4 = [pp.tile([128, 128], BF, tag=f"b4{d}", name=f"b4{d}") for d in range(2)]
        i4 = pp.tile([128, 1024], BF, tag="i4", name="i4")
        h0 = [pp.tile([128, 128], BF, tag=f"h0{d}", name=f"h0{d}") for d in range(2)]
        c0 = [pp.tile([128, 128], BF, tag=f"c0{d}", name=f"c0{d}") for d in range(2)]
        wom = [pp.tile([128, 256], BF, tag=f"wom{k}", name=f"wom{k}") for k in range(2)]
        uo = [pp.tile([128, 1], BF, tag=f"uo{k}", name=f"uo{k}") for k in range(2)]
        iota_t = pp.tile([128, 256], F32, tag="iota", name="iota")
        identb = pp.tile([128, 128], BF, tag="identb", name="identb")
        identf = pp.tile([128, 128], F32, tag="identf", name="identf")
        seg_t = pp.tile([128, 129], F32, tag="seg", name="seg")
        CFB = pp.tile([128, 256], BF, tag="CFB", name="CFB")
        e_cm = pp.tile([128, 129], F32, tag="ecm", name="ecm")
        hfx = pp.tile([128, 128], BF, tag="hfx", name="hfx")
        hbx = pp.tile([128, 128], BF, tag="hbx", name="hbx")
        ctx_sb = [pp.tile([128, 257], F32, tag=f"ctxsb{k}", name=f"ctxsb{k}") for k in range(2)]

        # ---- input DMAs ----
        nc.sync.dma_start(xT0[:], x_in[0:128, :])
        nc.sync.dma_start(xT1[:], x_in[128:256, :])
        for d, t_ in ((0, wih_f_in), (1, wih_b_in)):
            nc.sync.dma_start(wih[d][0][:], t_[0:128, :])
            nc.sync.dma_start(wih[d][1][:], t_[128:256, :])
        nc.sync.dma_start(whh[0][:], whh_f_in[:])
        nc.sync.dma_start(whh[1][:], whh_b_in[:])
        nc.sync.dma_start(b4[0][:], b4_f_in[:])
        nc.sync.dma_start(b4[1][:], b4_b_in[:])
        nc.sync.dma_start(i4[:], i4_in[:])
        nc.sync.dma_start(h0[0][:], h0f_in[:])
        nc.sync.dma_start(c0[0][:], c0f_in[:])
        nc.sync.dma_start(h0[1][:], h0b_in[:])
        nc.sync.dma_start(c0[1][:], c0b_in[:])
        nc.sync.dma_start(wom[0][:], wom_in[0:128, :])
        nc.sync.dma_start(wom[1][:], wom_in[128:256, :])
        nc.sync.dma_start(uo[0][:], uo_in[0:128, :])
        nc.sync.dma_start(uo[1][:], uo_in[128:256, :])
        nc.sync.dma_start(iota_t[:], iota_in[:])
        nc.sync.dma_start(identb[:], identb_in[:])
        nc.sync.dma_start(identf[:], identf_in[:])
        nc.sync.dma_start(seg_t[:], seg_in[:])

        # init cell state from seeds: CFB = [c0f | c0b]
        nc.vector.tensor_copy(CFB[:, 0:128], c0[0][:])
        nc.vector.tensor_copy(CFB[:, 128:256], c0[1][:])

        xT = [xT0, xT1]

        def fwd_pre_rhs(kh, s0):
            # cols {128m + 64 + s0 + ds}, ds in {0,1}, m in [0,128)
            base = 64 + s0
            v = xT[kh][:, base:base + 16384]
            v = v.rearrange("p (m b) -> p b m", b=128)
            return v[:, 0:2, :]

        def bwd_pre_rhs(kh, s):
            # backward lane k' reads x col 193 + 129*k' - s, k' in [0,128)
            a = 193 - s
            return xT[kh][:, a:a + 129 * 127 + 1:129]

        with tc.tile_pool(name="psG", bufs=4, space="PSUM") as psg, \
             tc.tile_pool(name="sig", bufs=3) as sigp, \
             tc.tile_pool(name="tg", bufs=3) as tgp, \
             tc.tile_pool(name="tcn", bufs=3) as tcp, \
             tc.tile_pool(name="tmp1", bufs=3) as t1p, \
             tc.tile_pool(name="tmp2", bufs=3) as t2p, \
             tc.tile_pool(name="hsc", bufs=4) as hscp:

            G = {}

            def emit_pre(s_):
                # pre-gate + bias matmuls for step s_ (both dirs)
                if s_ >= NSB:
                    return
                g = psg.tile([128, 1024], F32, tag="G", name="G")
                G[s_] = g
                for d in range(2):
                    do = d * 512
                    nc.tensor.matmul(g[:, do:do + 512], b4[d][:], i4[:, 0:512],
                                     start=True, stop=False)
                    for kh in range(2):
                        for j in range(4):
                            if d == 0:
                                if s_ >= NSF:
                                    continue
                                base = 64 + s_
                                rhs = xT[kh][:, base:base + 127 * 128 + 1:128]
                            else:
                                rhs = bwd_pre_rhs(kh, s_)
                            nc.tensor.matmul(
                                g[:, do + 128 * j:do + 128 * j + 128],
                                wih[d][kh][:, 128 * j:128 * j + 128],
                                rhs, start=False, stop=False)

            for s_ in range(3):
                emit_pre(s_)

            hs_prev = None
            for s in range(NSB):
                g = G.pop(s)
                emit_pre(s + 3)
                # w_hh matmuls (accumulate into this step's gate region)
                for d in range(2):
                    if d == 0 and s >= NSF:
                        continue
                    hs = h0[d][:] if s == 0 else hs_prev[:, 128 * d:128 * d + 128]
                    for j in range(4):
                        nc.tensor.matmul(
                            g[:, 512 * d + 128 * j:512 * d + 128 * j + 128],
                            whh[d][:, 128 * j:128 * j + 128], hs,
                            start=False, stop=True)
                # gates
                sig = sigp.tile([128, 768], BF, tag="sig", name="sig")
                src_sig = g[:].rearrange("p (a q) -> p a q", q=512)[:, :, 0:384]
                dst_sig = sig[:].rearrange("p (a q) -> p a q", q=384)
                nc.scalar.activation(dst_sig, src_sig, AF.Sigmoid)
                tg = tgp.tile([128, 256], BF, tag="tg", name="tg")
                src_tg = g[:].rearrange("p (a q) -> p a q", q=512)[:, :, 384:512]
                nc.scalar.activation(tg[:].rearrange("p (a q) -> p a q", q=128),
                                     src_tg, AF.Tanh)
                # c update
                sigr = sig[:].rearrange("p (a q) -> p a q", q=384)
                t1 = t1p.tile([128, 256], BF, tag="t1", name="t1")
                t2 = t2p.tile([128, 256], BF, tag="t2", name="t2")
                cr = CFB[:].rearrange("p (a q) -> p a q", q=128)
                nc.vector.tensor_tensor(t1[:].rearrange("p (a q) -> p a q", q=128),
                                        sigr[:, :, 128:256], cr, ALU.mult)
                nc.vector.tensor_tensor(t2[:].rearrange("p (a q) -> p a q", q=128),
                                        sigr[:, :, 0:128],
                                        tg[:].rearrange("p (a q) -> p a q", q=128),
                                        ALU.mult)
                nc.vector.tensor_tensor(CFB[:], t1[:], t2[:], ALU.add)
                tcn = tcp.tile([128, 256], BF, tag="tcn", name="tcn")
                nc.scalar.activation(tcn[:], CFB[:], AF.Tanh)
                # h = sigma_o * tanh(c) -> contiguous scratch (fast DVE write)
                hs_cur = hscp.tile([128, 256], BF, tag="hsc", name="hsc")
                for d in range(2):
                    if d == 0 and s >= NSF:
                        continue
                    nc.vector.tensor_tensor(hs_cur[:, 128 * d:128 * d + 128],
                                            sig[:, 384 * d + 256:384 * d + 384],
                                            tcn[:, 128 * d:128 * d + 128],
                                            ALU.mult)
                # off-critical-path strided copies into token-major h stores
                if s >= B:
                    if s < NSF:
                        nc.scalar.copy(
                            hfT[:, s - 64:s - 64 + 127 * 128 + 1:128],
                            hs_cur[:, 0:128])
                    a = 192 - s
                    nc.gpsimd.tensor_copy(
                        hbT[:, a:a + 129 * 127 + 1:129],
                        hs_cur[:, 128:256])
                if s < B:
                    nc.vector.tensor_copy(hf_head[:, s:s + 1],
                                          hs_cur[:, 0:1])
                    nc.vector.tensor_copy(hb_head[:, 63 - s:64 - s],
                                          hs_cur[:, 254:255])
                hs_prev = hs_cur

        # ---------------- attention phase ----------------
        # assemble extra window tiles
        nc.vector.tensor_copy(hfx[:, 0:64], hf_head[:])
        nc.vector.tensor_copy(hfx[:, 64:128], hfT[:, 16256:16320])
        nc.vector.tensor_copy(hbx[:, 0:64], hbT[:, 63:127])
        nc.vector.tensor_copy(hbx[:, 64:128], hb_head[:])

        with tc.tile_pool(name="psU", bufs=2, space="PSUM") as psu, \
             tc.tile_pool(name="uT", bufs=2) as utp, \
             tc.tile_pool(name="psA", bufs=2, space="PSUM") as psa:
            for gidx in range(33):
                if gidx < 32:
                    n = 512
                    hfr = hfT[:, 512 * gidx:512 * gidx + 512]
                    hbr = hbT[:, 512 * gidx + 127:512 * gidx + 127 + 512]
                    aout = att_dram[0:1, 512 * gidx:512 * gidx + 512]
                else:
                    n = 128
                    hfr = hfx[:]
                    hbr = hbx[:]
                    aout = att_dram[0:1, NQ:NQ + 128]
                pa = psa.tile([1, 512], F32, tag="psA", name="psA")
                for c2 in range(2):
                    pu = psu.tile([128, 512], F32, tag="psU", name="psU")
                    nc.tensor.matmul(pu[:, 0:n], wom[0][:, 128 * c2:128 * c2 + 128],
                                     hfr, start=True, stop=False)
                    nc.tensor.matmul(pu[:, 0:n], wom[1][:, 128 * c2:128 * c2 + 128],
                                     hbr, start=False, stop=True)
                    ut = utp.tile([128, 512], BF, tag="uT", name="uT")
                    nc.scalar.activation(ut[:, 0:n], pu[:, 0:n], AF.Tanh)
                    nc.tensor.matmul(pa[0:1, 0:n], uo[c2][:], ut[:, 0:n],
                                     start=(c2 == 0), stop=(c2 == 1))
                asb = utp.tile([1, 512], F32, tag="asb", name="asb")
                nc.vector.tensor_copy(asb[0:1, 0:n], pa[0:1, 0:n])
                nc.sync.dma_start(aout, asb[0:1, 0:n])

        # att -> column-major e
        with tc.tile_pool(name="psT", bufs=2, space="PSUM") as pst, \
             tc.tile_pool(name="anm", bufs=1) as anmp:
            att_nm = anmp.tile([128, 128], F32, tag="anm", name="anm")
            nc.sync.dma_start(
                att_nm[:],
                att_dram[0:1, 0:NQ].rearrange("a (n p) -> (a n) p", p=128))
            ps_a = pst.tile([128, 128], F32, tag="psT", name="psT")
            nc.tensor.transpose(ps_a[:], att_nm[:], identf[:])
            nc.scalar.activation(e_cm[:, 0:128], ps_a[:], AF.Exp)
            att_x = anmp.tile([128, 1], F32, tag="attx", name="attx")
            nc.sync.dma_start(
                att_x[:],
                att_dram[0:1, NQ:NQ + 128].rearrange("a (n p) -> (a n) p", p=1))
            nc.scalar.activation(e_cm[:, 128:129], att_x[:], AF.Exp)

        # ragged context accumulation
        with tc.tile_pool(name="psT2", bufs=2, space="PSUM") as pst2, \
             tc.tile_pool(name="yp", bufs=2) as yp, \
             tc.tile_pool(name="iw", bufs=2) as iwp, \
             tc.tile_pool(name="psC", bufs=1, space="PSUM") as psc:
            ctxp = [psc.tile([128, 257], F32, tag=f"ctxp{k}", name=f"ctxp{k}") for k in range(2)]
            for nti in range(NTILE + 1):
                if nti < NTILE:
                    hfr = hfT[:, 128 * nti:128 * nti + 128]
                    hbr = hbT[:, 128 * nti + 127:128 * nti + 255]
                else:
                    hfr = hfx[:]
                    hbr = hbx[:]
                ps_t = pst2.tile([128, 256], BF, tag="psT2", name="psT2")
                nc.tensor.transpose(ps_t[:, 0:128], hfr, identb[:])
                nc.tensor.transpose(ps_t[:, 128:256], hbr, identb[:])
                y = yp.tile([128, 257], BF, tag="y", name="y")
                nc.vector.tensor_copy(y[:, 0:256], ps_t[:])
                nc.vector.memset(y[:, 256:257], 1.0)
                iw = iwp.tile([128, 256], BF, tag="iw", name="iw")
                nc.vector.tensor_scalar(iw[:], iota_t[:],
                                        seg_t[:, nti:nti + 1],
                                        e_cm[:, nti:nti + 1],
                                        ALU.is_equal, ALU.mult)
                for k in range(2):
                    nc.tensor.matmul(ctxp[k][:], iw[:, 128 * k:128 * k + 128],
                                     y[:], start=(nti == 0), stop=(nti == NTILE))
            for k in range(2):
                nc.vector.tensor_copy(ctx_sb[k][:], ctxp[k][:])
        for k in range(2):
            nc.sync.dma_start(ctx_out[128 * k:128 * k + 128, :], ctx_sb[k][:])

    nc.finalize()
    _BUILT["nc"] = nc
    return nc


def _host_prep(inputs):
    x = np.asarray(inputs["sentence"], np.float32)
    doc_mask = np.asarray(inputs["doc_mask"]).astype(np.int64)
    h0g = np.asarray(inputs["h0"], np.float32)
    c0g = np.asarray(inputs["c0"], np.float32)

    perm = np.r_[0:128, 128:256, 384:512, 256:384]  # i,f,o,g order

    def wprep(w):  # [4H, X] -> lhsT [X, 4H] with gate perm, bf16
        return np.ascontiguousarray(w.astype(np.float32).T[:, perm]).astype(BF16)

    wih = {d: wprep(np.asarray(inputs[f"w_ih_{s}"], np.float32))
           for d, s in ((0, "f"), (1, "b"))}
    whh = {d: wprep(np.asarray(inputs[f"w_hh_{s}"], np.float32))
           for d, s in ((0, "f"), (1, "b"))}
    bias = {d: (np.asarray(inputs[f"b_ih_{s}"], np.float32)
                + np.asarray(inputs[f"b_hh_{s}"], np.float32))[perm]
            for d, s in ((0, "f"), (1, "b"))}
    b4 = {}
    for d in range(2):
        m = np.zeros((128, 128), np.float32)
        for k in range(4):
            m[k, :] = bias[d][128 * k:128 * k + 128]
        b4[d] = m.astype(BF16)
    i4 = np.zeros((128, 1024), np.float32)
    for r in range(2):
        for k in range(4):
            i4[k, 512 * r + 128 * k: 512 * r + 128 * k + 128] = 1.0
    i4 = i4.astype(BF16)

    wom = np.asarray(inputs["w_omega"], np.float32).astype(BF16)
    uo = np.asarray(inputs["u_omega"], np.float32).astype(BF16)
    iota = np.tile(np.arange(256, dtype=np.float32), (128, 1))
    identb = np.eye(128, dtype=np.float32).astype(BF16)
    identf = np.eye(128, dtype=np.float32)

    seg_global = np.searchsorted(doc_mask, np.arange(T), side="right")

    in_maps = []
    s_los = []
    xpad = np.zeros((T + 512, D), np.float32)
    xpad[64:64 + T] = x  # global row r ↔ token r - 64
    for c in range(NCORE):
        tc0 = c * PC
        xs = xpad[tc0:tc0 + SH]  # token tc0-64+i at row i
        xT = np.ascontiguousarray(xs.T).astype(BF16)

        # seeds
        h0f = np.zeros((128, 128), np.float32)
        c0f = np.zeros((128, 128), np.float32)
        h0b = np.zeros((128, 128), np.float32)
        c0b = np.zeros((128, 128), np.float32)
        if c == 0:
            h0f[:, 0] = h0g[0]
            c0f[:, 0] = c0g[0]
        if c == NCORE - 1:
            h0b[:, 126] = h0g[1]
            c0b[:, 126] = c0g[1]

        # segment ids, col-major [128, 129]
        segm = np.full((128, 129), -1.0, np.float32)
        toks_main = tc0 + 64 + np.arange(NQ)
        valid = toks_main < T
        if c == NCORE - 1:
            valid &= (np.arange(NQ) < 16256)  # tail handled by W_tail
        toks_extra = np.full(128, -1, np.int64)
        if c == 0:
            toks_extra[0:64] = np.arange(64)          # W_head: tokens [0,64)
        if c == NCORE - 1:
            toks_extra[64:128] = T - 64 + np.arange(64)  # W_tail
        all_toks = np.concatenate([toks_main[valid],
                                   toks_extra[toks_extra >= 0]])
        s_lo = int(seg_global[all_toks].min()) if all_toks.size else 0
        s_hi = int(seg_global[all_toks].max()) if all_toks.size else 0
        assert s_hi - s_lo < SWIN, f"segment window too wide: {s_hi - s_lo}"
        s_los.append(s_lo)
        sm = np.where(valid, seg_global[np.minimum(toks_main, T - 1)] - s_lo,
                      -1.0).astype(np.float32)
        segm[:, 0:128] = sm.reshape(128, 128).T  # segm[p, n] = seg(q=128n+p)
        se = np.full(128, -1.0, np.float32)
        mask_x = toks_extra >= 0
        se[mask_x] = seg_global[toks_extra[mask_x]] - s_lo
        segm[:, 128] = se

        in_maps.append({
            "xT": xT,
            "wih_f": wih[0], "wih_b": wih[1],
            "whh_f": whh[0], "whh_b": whh[1],
            "b4_f": b4[0], "b4_b": b4[1], "i4": i4,
            "h0f": h0f.astype(BF16), "c0f": c0f.astype(BF16),
            "h0b": h0b.astype(BF16), "c0b": c0b.astype(BF16),
            "wom": wom, "uo": uo, "iota": iota,
            "identb": identb, "identf": identf,
            "seg": segm,
        })
    return in_maps, s_los


def kernel(**inputs):
    global LAST_RESULT
    from concourse.bass_utils import run_bass_kernel_spmd

    nc = _build()
    in_maps, s_los = _host_prep(inputs)
    res = run_bass_kernel_spmd(nc, in_maps, core_ids=list(range(NCORE)))
    LAST_RESULT = res

    G = np.zeros((S + SWIN, 257), np.float64)
    for c in range(NCORE):
        ctx = np.asarray(res.results[c]["ctx"], np.float32)
        G[s_los[c]:s_los[c] + SWIN] += ctx
    G = G[:S]
    z = G[:, 256]
    ctx = G[:, :256] / np.where(z == 0, 1.0, z)[:, None]
    w_tag = np.asarray(inputs["w_tag"], np.float32)
    b_tag = np.asarray(inputs["b_tag"], np.float32)
    out = ctx.astype(np.float32) @ w_tag.T + b_tag
    return out.astype(np.float32)



# revision 16
# speedup vs baseline: 1.3583x; 1.2581x over previous
"""Bass/Trainium2 kernel for nn_BiLSTM_Tok_83837761618147.

Strategy (8 NeuronCores, SPMD, full inputs in / full output out):
  - Token dim sharded 8 ways (16384 tokens/core, with halos).
  - BiLSTM parallelized via chunked recurrence with burn-in: each core runs
    128 lanes x (128+64) steps forward and 128 lanes x (129+64) steps
    backward (state forgets exponentially; 64 warmup steps reach fp32
    accuracy; the true h0/c0-seeded lanes cover the sequence ends exactly).
  - Gate pre-activations computed by PE matmuls directly into PSUM
    (bias via a K=4 indicator matmul); w_hh @ h accumulated on top.
  - Attention (tanh/logits/exp) + ragged segment softmax-sum done on
    device via an e-weighted one-hot (token x segment-window) matmul.
  - Host combines per-core partial [segment, 257] sums, normalizes, and
    applies the tiny tag projection.
"""

import numpy as np
import ml_dtypes

BF16 = ml_dtypes.bfloat16

T = 131072
D = 256
H = 128
HID = 256
TAGS = 10
S = 1024
NCORE = 8
PC = T // NCORE          # 16384 tokens per core
B = 64                   # burn-in steps
LF = 128                 # forward lane length (tokens per lane)
LB = 129                 # backward lane length
NL = 128                 # lanes per direction
NSF = B + LF             # 192 forward steps
NSB = B + LB             # 193 backward steps
SH = 16704               # x shard rows [tc0-64, tc0-64+SH)
SWIN = 256               # segment window width per core
NQ = PC                  # main attention window positions
NTILE = NQ // 128        # 128 main token tiles
HBW = LB * NL - LB + LB + B  # hbT width: 16512
HBT_W = 16512
ATT_W = NQ + 128         # att buffer width (main + extra tile)
RW = 16                  # pre-gate ring depth (steps)

_BUILT = {}
LAST_RESULT = None


def _build():
    if "nc" in _BUILT:
        return _BUILT["nc"]
    import contextlib
    from concourse import bacc, mybir
    from concourse.tile import TileContext

    F32 = mybir.dt.float32
    BF = mybir.dt.bfloat16
    AF = mybir.ActivationFunctionType
    ALU = mybir.AluOpType

    nc = bacc.Bacc()

    def din(name, shape, dt):
        return nc.declare_dram_parameter(name, list(shape), dt, isOutput=False)

    x_in = din("xT", [256, SH], BF)
    wih_f_in = din("wih_f", [256, 512], BF)
    wih_b_in = din("wih_b", [256, 512], BF)
    whh_f_in = din("whh_f", [128, 512], BF)
    whh_b_in = din("whh_b", [128, 512], BF)
    bc_in = din("bc", [128, 8], F32)
    h0f_in = din("h0f", [128, 128], BF)
    c0f_in = din("c0f", [128, 128], BF)
    h0b_in = din("h0b", [128, 128], BF)
    c0b_in = din("c0b", [128, 128], BF)
    wom_in = din("wom", [256, 256], BF)
    uo_in = din("uo", [256, 1], BF)
    iota_in = din("iota", [128, 256], F32)
    identb_in = din("identb", [128, 128], BF)
    identf_in = din("identf", [128, 128], F32)
    seg_in = din("seg", [128, 129], F32)
    ctx_out = nc.declare_dram_parameter("ctx", [256, 257], F32, isOutput=True)
    att_dram = nc.dram_tensor("att_stage", [1, ATT_W], F32)

    with TileContext(nc) as tc, contextlib.ExitStack() as ctx:
        pp = ctx.enter_context(tc.tile_pool(name="persist", bufs=1))

        xT0 = pp.tile([128, SH], BF, tag="xT0", name="xT0")
        xT1 = pp.tile([128, SH], BF, tag="xT1", name="xT1")
        hfT = pp.tile([128, NQ], BF, tag="hfT", name="hfT")
        hbT = pp.tile([128, HBT_W], BF, tag="hbT", name="hbT")
        hf_head = pp.tile([128, 64], BF, tag="hfh", name="hfh")
        hb_head = pp.tile([128, 64], BF, tag="hbh", name="hbh")
        wih = [[pp.tile([128, 512], BF, tag=f"wih{d}{k}", name=f"wih{d}{k}") for k in range(2)]
               for d in range(2)]
        whh = [pp.tile([128, 512], BF, tag=f"whh{d}", name=f"whh{d}") for d in range(2)]
        bc = pp.tile([128, 8], F32, tag="bc", name="bc")
        gring = pp.tile([128, 8 * 128 * RW], BF, tag="gring", name="gring")
        h0 = [pp.tile([128, 128], BF, tag=f"h0{d}", name=f"h0{d}") for d in range(2)]
        c0 = [pp.tile([128, 128], BF, tag=f"c0{d}", name=f"c0{d}") for d in range(2)]
        wom = [pp.tile([128, 256], BF, tag=f"wom{k}", name=f"wom{k}") for k in range(2)]
        uo = [pp.tile([128, 1], BF, tag=f"uo{k}", name=f"uo{k}") for k in range(2)]
        iota_t = pp.tile([128, 256], F32, tag="iota", name="iota")
        identb = pp.tile([128, 128], BF, tag="identb", name="identb")
        identf = pp.tile([128, 128], F32, tag="identf", name="identf")
        seg_t = pp.tile([128, 129], F32, tag="seg", name="seg")
        CFB = pp.tile([128, 256], BF, tag="CFB", name="CFB")
        e_cm = pp.tile([128, 129], F32, tag="ecm", name="ecm")
        hfx = pp.tile([128, 128], BF, tag="hfx", name="hfx")
        hbx = pp.tile([128, 128], BF, tag="hbx", name="hbx")
        ctx_sb = [pp.tile([128, 257], F32, tag=f"ctxsb{k}", name=f"ctxsb{k}") for k in range(2)]

        # ---- input DMAs ----
        nc.sync.dma_start(xT0[:], x_in[0:128, :])
        nc.sync.dma_start(xT1[:], x_in[128:256, :])
        for d, t_ in ((0, wih_f_in), (1, wih_b_in)):
            nc.sync.dma_start(wih[d][0][:], t_[0:128, :])
            nc.sync.dma_start(wih[d][1][:], t_[128:256, :])
        nc.sync.dma_start(whh[0][:], whh_f_in[:])
        nc.sync.dma_start(whh[1][:], whh_b_in[:])
        nc.sync.dma_start(bc[:], bc_in[:])
        nc.sync.dma_start(h0[0][:], h0f_in[:])
        nc.sync.dma_start(c0[0][:], c0f_in[:])
        nc.sync.dma_start(h0[1][:], h0b_in[:])
        nc.sync.dma_start(c0[1][:], c0b_in[:])
        nc.sync.dma_start(wom[0][:], wom_in[0:128, :])
        nc.sync.dma_start(wom[1][:], wom_in[128:256, :])
        nc.sync.dma_start(uo[0][:], uo_in[0:128, :])
        nc.sync.dma_start(uo[1][:], uo_in[128:256, :])
        nc.sync.dma_start(iota_t[:], iota_in[:])
        nc.sync.dma_start(identb[:], identb_in[:])
        nc.sync.dma_start(identf[:], identf_in[:])
        nc.sync.dma_start(seg_t[:], seg_in[:])

        # init cell state from seeds: CFB = [c0f | c0b]
        nc.vector.tensor_copy(CFB[:, 0:128], c0[0][:])
        nc.vector.tensor_copy(CFB[:, 128:256], c0[1][:])

        xT = [xT0, xT1]

        def grv():
            return gring[:].rearrange("p (c l w) -> p c l w", c=8, w=RW)

        with tc.tile_pool(name="psG", bufs=2, space="PSUM") as psg, \
             tc.tile_pool(name="psB", bufs=4, space="PSUM") as psb, \
             tc.tile_pool(name="sig", bufs=3) as sigp, \
             tc.tile_pool(name="tg", bufs=3) as tgp, \
             tc.tile_pool(name="tcn", bufs=3) as tcp, \
             tc.tile_pool(name="tmp1", bufs=3) as t1p, \
             tc.tile_pool(name="tmp2", bufs=3) as t2p, \
             tc.tile_pool(name="hsc", bufs=4) as hscp:

            # ---- pre-gate batch units: G_pre = x @ w_ih.T + b, evacuated to
            # the bf16 ring `gring` 16 steps ahead of consumption ----
            def emit_unit(sb, h2, c):
                d, j = divmod(c, 4)
                ps = [psb.tile([128, 512], F32, tag="psb", name="psb")
                      for _ in range(2)]
                for kh in range(2):
                    for b_ in range(2):
                        s0 = sb * 16 + h2 * 8 + b_ * 4
                        if d == 0:
                            rhs = xT[kh][:, 64 + s0:64 + s0 + 128 * 128].rearrange(
                                "p (l q) -> p l q", q=128)[:, :, 0:4]
                        else:
                            base = 190 - s0
                            rhs = xT[kh][:, base:base + 129 * 128].rearrange(
                                "p (l q) -> p l q", q=129)[:, :, 0:4]
                        nc.tensor.matmul(ps[b_][:],
                                         wih[d][kh][:, 128 * j:128 * j + 128],
                                         rhs, start=(kh == 0), stop=(kh == 1))
                for b_ in range(2):
                    w0 = h2 * 8 + b_ * 4
                    dst = grv()[:, c:c + 1, :, w0:w0 + 4]
                    src = ps[b_][:].rearrange("p (a l q) -> p a l q", a=1, q=4)
                    nc.vector.tensor_scalar(dst, src, bc[:, c:c + 1], None,
                                            ALU.add)

            def emit_unit12(c):
                # step 192, bwd chunks only
                d, j = divmod(c, 4)
                ps = psb.tile([128, 512], F32, tag="psb", name="psb")
                for kh in range(2):
                    rhs = xT[kh][:, 1:1 + 129 * 127 + 1:129]
                    nc.tensor.matmul(ps[:, 0:128],
                                     wih[d][kh][:, 128 * j:128 * j + 128],
                                     rhs, start=(kh == 0), stop=(kh == 1))
                dst = grv()[:, c:c + 1, :, 3:4]
                src = ps[:, 0:128].rearrange("p (a l q) -> p a l q", a=1, q=1)
                nc.vector.tensor_scalar(dst, src, bc[:, c:c + 1], None, ALU.add)

            units = []
            for sb in range(12):
                for h2 in range(2):
                    for c in range(8):
                        units.append((emit_unit, sb, h2, c))
            for c in range(4, 8):
                units.append((emit_unit12, c))

            def inject(s):
                # load G_pre for step s into a fresh PSUM gate tile
                g = psg.tile([128, 1024], F32, tag="G", name="G")
                wf = s % RW
                blk = (wf // 4) * 4
                wb = blk + 3 - (s % 4)
                nc.tensor.matmul(g[:, 0:512], identb[:],
                                 grv()[:, 0:4, :, wf:wf + 1],
                                 start=True, stop=False)
                nc.tensor.matmul(g[:, 512:1024], identb[:],
                                 grv()[:, 4:8, :, wb:wb + 1],
                                 start=True, stop=False)
                return g

            for u in units[:8]:
                u[0](*u[1:])
            g_cur = inject(0)

            hs_prev = None
            for s in range(NSB):
                g = g_cur
                # w_hh matmuls, f-gate first so sigmoid_f can start early
                for j in (1, 0, 3, 2):
                    for d in range(2):
                        if d == 0 and s >= NSF:
                            continue
                        hs = h0[d][:] if s == 0 else hs_prev[:, 128 * d:128 * d + 128]
                        nc.tensor.matmul(
                            g[:, 512 * d + 128 * j:512 * d + 128 * j + 128],
                            whh[d][:, 128 * j:128 * j + 128], hs,
                            start=False, stop=True)
                if 8 + s < len(units):
                    u = units[8 + s]
                    u[0](*u[1:])
                if s + 1 < NSB:
                    g_cur = inject(s + 1)
                # gates: split activations in chain order f, i, g, o
                sig = sigp.tile([128, 768], BF, tag="sig", name="sig")
                gv = g[:].rearrange("p (a q) -> p a q", q=512)
                sv = sig[:].rearrange("p (a q) -> p a q", q=384)
                nc.scalar.activation(sv[:, :, 128:256], gv[:, :, 128:256],
                                     AF.Sigmoid)
                nc.scalar.activation(sv[:, :, 0:128], gv[:, :, 0:128],
                                     AF.Sigmoid)
                tg = tgp.tile([128, 256], BF, tag="tg", name="tg")
                nc.scalar.activation(tg[:].rearrange("p (a q) -> p a q", q=128),
                                     gv[:, :, 384:512], AF.Tanh)
                nc.scalar.activation(sv[:, :, 256:384], gv[:, :, 256:384],
                                     AF.Sigmoid)
                # c update
                sigr = sig[:].rearrange("p (a q) -> p a q", q=384)
                t1 = t1p.tile([128, 256], BF, tag="t1", name="t1")
                t2 = t2p.tile([128, 256], BF, tag="t2", name="t2")
                cr = CFB[:].rearrange("p (a q) -> p a q", q=128)
                nc.vector.tensor_tensor(t1[:].rearrange("p (a q) -> p a q", q=128),
                                        sigr[:, :, 128:256], cr, ALU.mult)
                nc.vector.tensor_tensor(t2[:].rearrange("p (a q) -> p a q", q=128),
                                        sigr[:, :, 0:128],
                                        tg[:].rearrange("p (a q) -> p a q", q=128),
                                        ALU.mult)
                nc.vector.tensor_tensor(CFB[:], t1[:], t2[:], ALU.add)
                tcn = tcp.tile([128, 256], BF, tag="tcn", name="tcn")
                nc.scalar.activation(tcn[:], CFB[:], AF.Tanh)
                # h = sigma_o * tanh(c) -> contiguous scratch (fast DVE write)
                hs_cur = hscp.tile([128, 256], BF, tag="hsc", name="hsc")
                for d in range(2):
                    if d == 0 and s >= NSF:
                        continue
                    nc.vector.tensor_tensor(hs_cur[:, 128 * d:128 * d + 128],
                                            sig[:, 384 * d + 256:384 * d + 384],
                                            tcn[:, 128 * d:128 * d + 128],
                                            ALU.mult)
                # off-critical-path strided copies into token-major h stores
                if s >= B:
                    if s < NSF:
                        nc.gpsimd.tensor_copy(
                            hfT[:, s - 64:s - 64 + 127 * 128 + 1:128],
                            hs_cur[:, 0:128])
                    a = 192 - s
                    nc.gpsimd.tensor_copy(
                        hbT[:, a:a + 129 * 127 + 1:129],
                        hs_cur[:, 128:256])
                if s < B:
                    nc.gpsimd.tensor_copy(hf_head[:, s:s + 1],
                                          hs_cur[:, 0:1])
                    nc.gpsimd.tensor_copy(hb_head[:, 63 - s:64 - s],
                                          hs_cur[:, 254:255])
                hs_prev = hs_cur

        # ---------------- attention phase ----------------
        # assemble extra window tiles
        nc.vector.tensor_copy(hfx[:, 0:64], hf_head[:])
        nc.vector.tensor_copy(hfx[:, 64:128], hfT[:, 16256:16320])
        nc.vector.tensor_copy(hbx[:, 0:64], hbT[:, 63:127])
        nc.vector.tensor_copy(hbx[:, 64:128], hb_head[:])

        with tc.tile_pool(name="psU", bufs=2, space="PSUM") as psu, \
             tc.tile_pool(name="uT", bufs=2) as utp, \
             tc.tile_pool(name="psA", bufs=2, space="PSUM") as psa:
            for gidx in range(33):
                if gidx < 32:
                    n = 512
                    hfr = hfT[:, 512 * gidx:512 * gidx + 512]
                    hbr = hbT[:, 512 * gidx + 127:512 * gidx + 127 + 512]
                    aout = att_dram[0:1, 512 * gidx:512 * gidx + 512]
                else:
                    n = 128
                    hfr = hfx[:]
                    hbr = hbx[:]
                    aout = att_dram[0:1, NQ:NQ + 128]
                pa = psa.tile([1, 512], F32, tag="psA", name="psA")
                for c2 in range(2):
                    pu = psu.tile([128, 512], F32, tag="psU", name="psU")
                    nc.tensor.matmul(pu[:, 0:n], wom[0][:, 128 * c2:128 * c2 + 128],
                                     hfr, start=True, stop=False)
                    nc.tensor.matmul(pu[:, 0:n], wom[1][:, 128 * c2:128 * c2 + 128],
                                     hbr, start=False, stop=True)
                    ut = utp.tile([128, 512], BF, tag="uT", name="uT")
                    nc.scalar.activation(ut[:, 0:n], pu[:, 0:n], AF.Tanh)
                    nc.tensor.matmul(pa[0:1, 0:n], uo[c2][:], ut[:, 0:n],
                                     start=(c2 == 0), stop=(c2 == 1))
                asb = utp.tile([1, 512], F32, tag="asb", name="asb")
                nc.vector.tensor_copy(asb[0:1, 0:n], pa[0:1, 0:n])
                nc.sync.dma_start(aout, asb[0:1, 0:n])

        # att -> column-major e
        with tc.tile_pool(name="psT", bufs=2, space="PSUM") as pst, \
             tc.tile_pool(name="anm", bufs=1) as anmp:
            att_nm = anmp.tile([128, 128], F32, tag="anm", name="anm")
            nc.sync.dma_start(
                att_nm[:],
                att_dram[0:1, 0:NQ].rearrange("a (n p) -> (a n) p", p=128))
            ps_a = pst.tile([128, 128], F32, tag="psT", name="psT")
            nc.tensor.transpose(ps_a[:], att_nm[:], identf[:])
            nc.scalar.activation(e_cm[:, 0:128], ps_a[:], AF.Exp)
            att_x = anmp.tile([128, 1], F32, tag="attx", name="attx")
            nc.sync.dma_start(
                att_x[:],
                att_dram[0:1, NQ:NQ + 128].rearrange("a (n p) -> (a n) p", p=1))
            nc.scalar.activation(e_cm[:, 128:129], att_x[:], AF.Exp)

        # ragged context accumulation
        with tc.tile_pool(name="psT2", bufs=2, space="PSUM") as pst2, \
             tc.tile_pool(name="yp", bufs=2) as yp, \
             tc.tile_pool(name="iw", bufs=2) as iwp, \
             tc.tile_pool(name="psC", bufs=1, space="PSUM") as psc:
            ctxp = [psc.tile([128, 257], F32, tag=f"ctxp{k}", name=f"ctxp{k}") for k in range(2)]
            for nti in range(NTILE + 1):
                if nti < NTILE:
                    hfr = hfT[:, 128 * nti:128 * nti + 128]
                    hbr = hbT[:, 128 * nti + 127:128 * nti + 255]
                else:
                    hfr = hfx[:]
                    hbr = hbx[:]
                ps_t = pst2.tile([128, 256], BF, tag="psT2", name="psT2")
                nc.tensor.transpose(ps_t[:, 0:128], hfr, identb[:])
                nc.tensor.transpose(ps_t[:, 128:256], hbr, identb[:])
                y = yp.tile([128, 257], BF, tag="y", name="y")
                nc.vector.tensor_copy(y[:, 0:256], ps_t[:])
                nc.vector.memset(y[:, 256:257], 1.0)
                iw = iwp.tile([128, 256], BF, tag="iw", name="iw")
                nc.vector.tensor_scalar(iw[:], iota_t[:],
                                        seg_t[:, nti:nti + 1],
                                        e_cm[:, nti:nti + 1],
                                        ALU.is_equal, ALU.mult)
                for k in range(2):
                    nc.tensor.matmul(ctxp[k][:], iw[:, 128 * k:128 * k + 128],
                                     y[:], start=(nti == 0), stop=(nti == NTILE))
            for k in range(2):
                nc.vector.tensor_copy(ctx_sb[k][:], ctxp[k][:])
        for k in range(2):
            nc.sync.dma_start(ctx_out[128 * k:128 * k + 128, :], ctx_sb[k][:])

    nc.finalize()
    _BUILT["nc"] = nc
    return nc


def _host_prep(inputs):
    x = np.asarray(inputs["sentence"], np.float32)
    doc_mask = np.asarray(inputs["doc_mask"]).astype(np.int64)
    h0g = np.asarray(inputs["h0"], np.float32)
    c0g = np.asarray(inputs["c0"], np.float32)

    perm = np.r_[0:128, 128:256, 384:512, 256:384]  # i,f,o,g order

    def wprep(w):  # [4H, X] -> lhsT [X, 4H] with gate perm, bf16
        return np.ascontiguousarray(w.astype(np.float32).T[:, perm]).astype(BF16)

    wih = {d: wprep(np.asarray(inputs[f"w_ih_{s}"], np.float32))
           for d, s in ((0, "f"), (1, "b"))}
    whh = {d: wprep(np.asarray(inputs[f"w_hh_{s}"], np.float32))
           for d, s in ((0, "f"), (1, "b"))}
    bias = {d: (np.asarray(inputs[f"b_ih_{s}"], np.float32)
                + np.asarray(inputs[f"b_hh_{s}"], np.float32))[perm]
            for d, s in ((0, "f"), (1, "b"))}
    bc = np.zeros((128, 8), np.float32)
    for d in range(2):
        for j in range(4):
            bc[:, d * 4 + j] = bias[d][128 * j:128 * j + 128]

    wom = np.asarray(inputs["w_omega"], np.float32).astype(BF16)
    uo = np.asarray(inputs["u_omega"], np.float32).astype(BF16)
    iota = np.tile(np.arange(256, dtype=np.float32), (128, 1))
    identb = np.eye(128, dtype=np.float32).astype(BF16)
    identf = np.eye(128, dtype=np.float32)

    seg_global = np.searchsorted(doc_mask, np.arange(T), side="right")

    in_maps = []
    s_los = []
    xpad = np.zeros((T + 512, D), np.float32)
    xpad[64:64 + T] = x  # global row r ↔ token r - 64
    for c in range(NCORE):
        tc0 = c * PC
        xs = xpad[tc0:tc0 + SH]  # token tc0-64+i at row i
        xT = np.ascontiguousarray(xs.T).astype(BF16)

        # seeds
        h0f = np.zeros((128, 128), np.float32)
        c0f = np.zeros((128, 128), np.float32)
        h0b = np.zeros((128, 128), np.float32)
        c0b = np.zeros((128, 128), np.float32)
        if c == 0:
            h0f[:, 0] = h0g[0]
            c0f[:, 0] = c0g[0]
        if c == NCORE - 1:
            h0b[:, 126] = h0g[1]
            c0b[:, 126] = c0g[1]

        # segment ids, col-major [128, 129]
        segm = np.full((128, 129), -1.0, np.float32)
        toks_main = tc0 + 64 + np.arange(NQ)
        valid = toks_main < T
        if c == NCORE - 1:
            valid &= (np.arange(NQ) < 16256)  # tail handled by W_tail
        toks_extra = np.full(128, -1, np.int64)
        if c == 0:
            toks_extra[0:64] = np.arange(64)          # W_head: tokens [0,64)
        if c == NCORE - 1:
            toks_extra[64:128] = T - 64 + np.arange(64)  # W_tail
        all_toks = np.concatenate([toks_main[valid],
                                   toks_extra[toks_extra >= 0]])
        s_lo = int(seg_global[all_toks].min()) if all_toks.size else 0
        s_hi = int(seg_global[all_toks].max()) if all_toks.size else 0
        assert s_hi - s_lo < SWIN, f"segment window too wide: {s_hi - s_lo}"
        s_los.append(s_lo)
        sm = np.where(valid, seg_global[np.minimum(toks_main, T - 1)] - s_lo,
                      -1.0).astype(np.float32)
        segm[:, 0:128] = sm.reshape(128, 128).T  # segm[p, n] = seg(q=128n+p)
        se = np.full(128, -1.0, np.float32)
        mask_x = toks_extra >= 0
        se[mask_x] = seg_global[toks_extra[mask_x]] - s_lo
        segm[:, 128] = se

        in_maps.append({
            "xT": xT,
            "wih_f": wih[0], "wih_b": wih[1],
            "whh_f": whh[0], "whh_b": whh[1],
            "bc": bc,
            "h0f": h0f.astype(BF16), "c0f": c0f.astype(BF16),
            "h0b": h0b.astype(BF16), "c0b": c0b.astype(BF16),
            "wom": wom, "uo": uo, "iota": iota,
            "identb": identb, "identf": identf,
            "seg": segm,
        })
    return in_maps, s_los


def kernel(**inputs):
    global LAST_RESULT
    from concourse.bass_utils import run_bass_kernel_spmd

    nc = _build()
    in_maps, s_los = _host_prep(inputs)
    res = run_bass_kernel_spmd(nc, in_maps, core_ids=list(range(NCORE)))
    LAST_RESULT = res

    G = np.zeros((S + SWIN, 257), np.float64)
    for c in range(NCORE):
        ctx = np.asarray(res.results[c]["ctx"], np.float32)
        G[s_los[c]:s_los[c] + SWIN] += ctx
    G = G[:S]
    z = G[:, 256]
    ctx = G[:, :256] / np.where(z == 0, 1.0, z)[:, None]
    w_tag = np.asarray(inputs["w_tag"], np.float32)
    b_tag = np.asarray(inputs["b_tag"], np.float32)
    out = ctx.astype(np.float32) @ w_tag.T + b_tag
    return out.astype(np.float32)



# revision 23
# speedup vs baseline: 1.5662x; 1.1531x over previous
"""Bass/Trainium2 kernel for nn_BiLSTM_Tok_83837761618147.

Strategy (8 NeuronCores, SPMD, full inputs in / full output out):
  - Token dim sharded 8 ways (16384 tokens/core, with halos).
  - BiLSTM parallelized via chunked recurrence with burn-in: each core runs
    128 lanes x (128+64) steps forward and 128 lanes x (129+64) steps
    backward (state forgets exponentially; 64 warmup steps reach fp32
    accuracy; the true h0/c0-seeded lanes cover the sequence ends exactly).
  - Gate pre-activations computed by PE matmuls directly into PSUM
    (bias via a K=4 indicator matmul); w_hh @ h accumulated on top.
  - Attention (tanh/logits/exp) + ragged segment softmax-sum done on
    device via an e-weighted one-hot (token x segment-window) matmul.
  - Host combines per-core partial [segment, 257] sums, normalizes, and
    applies the tiny tag projection.
"""

import numpy as np
import ml_dtypes

BF16 = ml_dtypes.bfloat16

T = 131072
D = 256
H = 128
HID = 256
TAGS = 10
S = 1024
NCORE = 8
PC = T // NCORE          # 16384 tokens per core
B = 64                   # burn-in steps
LF = 128                 # forward lane length (tokens per lane)
LB = 129                 # backward lane length
NL = 128                 # lanes per direction
NSF = B + LF             # 192 forward steps
NSB = B + LB             # 193 backward steps
SH = 16704               # x shard rows [tc0-64, tc0-64+SH)
SWIN = 256               # segment window width per core
NQ = PC                  # main attention window positions
NTILE = NQ // 128        # 128 main token tiles
HBW = LB * NL - LB + LB + B  # hbT width: 16512
HBT_W = 16512
ATT_W = NQ + 128         # att buffer width (main + extra tile)
RW = 16                  # pre-gate ring depth (steps)

_BUILT = {}
LAST_RESULT = None


def _build():
    if "nc" in _BUILT:
        return _BUILT["nc"]
    import contextlib
    from concourse import bacc, mybir
    from concourse.tile import TileContext

    F32 = mybir.dt.float32
    BF = mybir.dt.bfloat16
    AF = mybir.ActivationFunctionType
    ALU = mybir.AluOpType

    nc = bacc.Bacc()

    def din(name, shape, dt):
        return nc.declare_dram_parameter(name, list(shape), dt, isOutput=False)

    x_in = din("xT", [256, SH], BF)
    wih_f_in = din("wih_f", [256, 512], BF)
    wih_b_in = din("wih_b", [256, 512], BF)
    whh_f_in = din("whh_f", [128, 512], BF)
    whh_b_in = din("whh_b", [128, 512], BF)
    bc_in = din("bc", [128, 8], F32)
    h0f_in = din("h0f", [128, 128], BF)
    c0f_in = din("c0f", [128, 128], BF)
    h0b_in = din("h0b", [128, 128], BF)
    c0b_in = din("c0b", [128, 128], BF)
    wom_in = din("wom", [256, 256], BF)
    uo_in = din("uo", [256, 1], BF)
    iota_in = din("iota", [128, 256], F32)
    identb_in = din("identb", [128, 128], BF)
    identf_in = din("identf", [128, 128], F32)
    seg_in = din("seg", [128, 129], F32)
    ctx_out = nc.declare_dram_parameter("ctx", [256, 257], F32, isOutput=True)
    att_dram = nc.dram_tensor("att_stage", [1, ATT_W], F32)

    with TileContext(nc) as tc, contextlib.ExitStack() as ctx:
        pp = ctx.enter_context(tc.tile_pool(name="persist", bufs=1))

        xT0 = pp.tile([128, SH], BF, tag="xT0", name="xT0")
        xT1 = pp.tile([128, SH], BF, tag="xT1", name="xT1")
        hfT = pp.tile([128, NQ], BF, tag="hfT", name="hfT")
        hbT = pp.tile([128, HBT_W], BF, tag="hbT", name="hbT")
        hf_head = pp.tile([128, 64], BF, tag="hfh", name="hfh")
        hb_head = pp.tile([128, 64], BF, tag="hbh", name="hbh")
        wih = [[pp.tile([128, 512], BF, tag=f"wih{d}{k}", name=f"wih{d}{k}") for k in range(2)]
               for d in range(2)]
        whh = [pp.tile([128, 512], BF, tag=f"whh{d}", name=f"whh{d}") for d in range(2)]
        bc = pp.tile([128, 8], F32, tag="bc", name="bc")
        gring = pp.tile([128, 8 * 128 * RW], BF, tag="gring", name="gring")
        h0 = [pp.tile([128, 128], BF, tag=f"h0{d}", name=f"h0{d}") for d in range(2)]
        c0 = [pp.tile([128, 128], BF, tag=f"c0{d}", name=f"c0{d}") for d in range(2)]
        wom = [pp.tile([128, 256], BF, tag=f"wom{k}", name=f"wom{k}") for k in range(2)]
        uo = [pp.tile([128, 1], BF, tag=f"uo{k}", name=f"uo{k}") for k in range(2)]
        iota_t = pp.tile([128, 256], F32, tag="iota", name="iota")
        identb = pp.tile([128, 128], BF, tag="identb", name="identb")
        identf = pp.tile([128, 128], F32, tag="identf", name="identf")
        seg_t = pp.tile([128, 129], F32, tag="seg", name="seg")
        CFB = pp.tile([128, 256], BF, tag="CFB", name="CFB")
        e_cm = pp.tile([128, 129], F32, tag="ecm", name="ecm")
        hfx = pp.tile([128, 128], BF, tag="hfx", name="hfx")
        hbx = pp.tile([128, 128], BF, tag="hbx", name="hbx")
        ctx_sb = [pp.tile([128, 257], F32, tag=f"ctxsb{k}", name=f"ctxsb{k}") for k in range(2)]

        # ---- input DMAs ----
        nc.sync.dma_start(xT0[:], x_in[0:128, :])
        nc.sync.dma_start(xT1[:], x_in[128:256, :])
        for d, t_ in ((0, wih_f_in), (1, wih_b_in)):
            nc.sync.dma_start(wih[d][0][:], t_[0:128, :])
            nc.sync.dma_start(wih[d][1][:], t_[128:256, :])
        nc.sync.dma_start(whh[0][:], whh_f_in[:])
        nc.sync.dma_start(whh[1][:], whh_b_in[:])
        nc.sync.dma_start(bc[:], bc_in[:])
        nc.sync.dma_start(h0[0][:], h0f_in[:])
        nc.sync.dma_start(c0[0][:], c0f_in[:])
        nc.sync.dma_start(h0[1][:], h0b_in[:])
        nc.sync.dma_start(c0[1][:], c0b_in[:])
        nc.sync.dma_start(wom[0][:], wom_in[0:128, :])
        nc.sync.dma_start(wom[1][:], wom_in[128:256, :])
        nc.sync.dma_start(uo[0][:], uo_in[0:128, :])
        nc.sync.dma_start(uo[1][:], uo_in[128:256, :])
        nc.sync.dma_start(iota_t[:], iota_in[:])
        nc.sync.dma_start(identb[:], identb_in[:])
        nc.sync.dma_start(identf[:], identf_in[:])
        nc.sync.dma_start(seg_t[:], seg_in[:])

        # init cell state from seeds: CFB = [c0f | c0b]
        nc.vector.tensor_copy(CFB[:, 0:128], c0[0][:])
        nc.vector.tensor_copy(CFB[:, 128:256], c0[1][:])

        xT = [xT0, xT1]

        def grv():
            # slot-major ring: col = w*1024 + c*128 + l
            return gring[:].rearrange("p (w c l) -> p w c l", w=RW, c=8)

        with tc.tile_pool(name="psG", bufs=2, space="PSUM") as psg, \
             tc.tile_pool(name="psB", bufs=4, space="PSUM") as psb, \
             tc.tile_pool(name="sig", bufs=3) as sigp, \
             tc.tile_pool(name="tg", bufs=3) as tgp, \
             tc.tile_pool(name="tcn", bufs=3) as tcp, \
             tc.tile_pool(name="tmp1", bufs=3) as t1p, \
             tc.tile_pool(name="tmp2", bufs=3) as t2p, \
             tc.tile_pool(name="hsc", bufs=4) as hscp:

            # ---- pre-gate batch units: G_pre = x @ w_ih.T + b, evacuated to
            # the bf16 ring `gring` 16 steps ahead of consumption ----
            def emit_unit(sb, h2, c):
                d, j = divmod(c, 4)
                ps = [psb.tile([128, 512], F32, tag="psb", name="psb")
                      for _ in range(2)]
                for kh in range(2):
                    for b_ in range(2):
                        s0 = sb * 16 + h2 * 8 + b_ * 4
                        if d == 0:
                            rhs = xT[kh][:, 64 + s0:64 + s0 + 128 * 128].rearrange(
                                "p (l q) -> p l q", q=128)[:, :, 0:4]
                        else:
                            base = 190 - s0
                            rhs = xT[kh][:, base:base + 129 * 128].rearrange(
                                "p (l q) -> p l q", q=129)[:, :, 0:4]
                        nc.tensor.matmul(ps[b_][:],
                                         wih[d][kh][:, 128 * j:128 * j + 128],
                                         rhs, start=(kh == 0), stop=(kh == 1))
                for b_ in range(2):
                    w0 = h2 * 8 + b_ * 4
                    dst = grv()[:, w0:w0 + 4, c:c + 1, :]
                    src = ps[b_][:].rearrange("p (l a q) -> p q a l", a=1, q=4)
                    if b_ == 0:
                        nc.vector.tensor_scalar(dst, src, bc[:, c:c + 1], None,
                                                ALU.add)
                    else:
                        nc.scalar.activation(dst, src, AF.Identity,
                                             bias=bc[:, c:c + 1])

            def emit_unit12(c):
                # step 192, bwd chunks only
                d, j = divmod(c, 4)
                ps = psb.tile([128, 512], F32, tag="psb", name="psb")
                for kh in range(2):
                    rhs = xT[kh][:, 1:1 + 129 * 127 + 1:129]
                    nc.tensor.matmul(ps[:, 0:128],
                                     wih[d][kh][:, 128 * j:128 * j + 128],
                                     rhs, start=(kh == 0), stop=(kh == 1))
                dst = grv()[:, 3:4, c:c + 1, :]
                src = ps[:, 0:128].rearrange("p (a b l) -> p a b l", a=1, b=1)
                nc.vector.tensor_scalar(dst, src, bc[:, c:c + 1], None, ALU.add)

            units = []
            for sb in range(12):
                for h2 in range(2):
                    for c in range(8):
                        units.append((emit_unit, sb, h2, c))
            for c in range(4, 8):
                units.append((emit_unit12, c))

            def inject(s):
                # load G_pre for step s into a fresh PSUM gate tile
                g = psg.tile([128, 1024], F32, tag="G", name="G")
                wf = s % RW
                blk = (wf // 4) * 4
                wb = blk + 3 - (s % 4)
                nc.tensor.matmul(g[:, 0:512], identb[:],
                                 gring[:, wf * 1024:wf * 1024 + 512],
                                 start=True, stop=False)
                nc.tensor.matmul(g[:, 512:1024], identb[:],
                                 gring[:, wb * 1024 + 512:wb * 1024 + 1024],
                                 start=True, stop=False)
                return g

            for u in units[:9]:
                u[0](*u[1:])
            g_cur = inject(0)

            hs_prev = None
            for s in range(NSB):
                g = g_cur
                # w_hh matmuls, f-gate first so sigmoid_f can start early
                for j in (1, 0, 3, 2):
                    for d in range(2):
                        if d == 0 and s >= NSF:
                            continue
                        hs = h0[d][:] if s == 0 else hs_prev[:, 128 * d:128 * d + 128]
                        nc.tensor.matmul(
                            g[:, 512 * d + 128 * j:512 * d + 128 * j + 128],
                            whh[d][:, 128 * j:128 * j + 128], hs,
                            start=False, stop=True)
                if s + 1 < NSB:
                    g_cur = inject(s + 1)
                # gates: split activations in chain order (i,f), g, o
                sig = sigp.tile([128, 768], BF, tag="sig", name="sig")
                gv = g[:].rearrange("p (a q) -> p a q", q=512)
                sv = sig[:].rearrange("p (a q) -> p a q", q=384)
                nc.scalar.activation(sv[:, :, 0:256], gv[:, :, 0:256],
                                     AF.Sigmoid)
                tg = tgp.tile([128, 256], BF, tag="tg", name="tg")
                nc.scalar.activation(tg[:].rearrange("p (a q) -> p a q", q=128),
                                     gv[:, :, 384:512], AF.Tanh)
                nc.scalar.activation(sv[:, :, 256:384], gv[:, :, 256:384],
                                     AF.Sigmoid)
                # c update
                sigr = sig[:].rearrange("p (a q) -> p a q", q=384)
                t1 = t1p.tile([128, 256], BF, tag="t1", name="t1")
                t2 = t2p.tile([128, 256], BF, tag="t2", name="t2")
                cr = CFB[:].rearrange("p (a q) -> p a q", q=128)
                nc.vector.tensor_tensor(t1[:].rearrange("p (a q) -> p a q", q=128),
                                        sigr[:, :, 128:256], cr, ALU.mult)
                nc.vector.tensor_tensor(t2[:].rearrange("p (a q) -> p a q", q=128),
                                        sigr[:, :, 0:128],
                                        tg[:].rearrange("p (a q) -> p a q", q=128),
                                        ALU.mult)
                nc.vector.tensor_tensor(CFB[:], t1[:], t2[:], ALU.add)
                tcn = tcp.tile([128, 256], BF, tag="tcn", name="tcn")
                nc.scalar.activation(tcn[:], CFB[:], AF.Tanh)
                # h = sigma_o * tanh(c) -> contiguous scratch (fast DVE write)
                hs_cur = hscp.tile([128, 256], BF, tag="hsc", name="hsc")
                for d in range(2):
                    if d == 0 and s >= NSF:
                        continue
                    nc.vector.tensor_tensor(hs_cur[:, 128 * d:128 * d + 128],
                                            sig[:, 384 * d + 256:384 * d + 384],
                                            tcn[:, 128 * d:128 * d + 128],
                                            ALU.mult)
                # off-critical-path strided copies into token-major h stores
                if s >= B:
                    if s < NSF:
                        nc.gpsimd.tensor_copy(
                            hfT[:, s - 64:s - 64 + 127 * 128 + 1:128],
                            hs_cur[:, 0:128])
                    a = 192 - s
                    nc.gpsimd.tensor_copy(
                        hbT[:, a:a + 129 * 127 + 1:129],
                        hs_cur[:, 128:256])
                if s < B:
                    nc.gpsimd.tensor_copy(hf_head[:, s:s + 1],
                                          hs_cur[:, 0:1])
                    nc.gpsimd.tensor_copy(hb_head[:, 63 - s:64 - s],
                                          hs_cur[:, 254:255])
                hs_prev = hs_cur
                if 9 + s < len(units):
                    u = units[9 + s]
                    u[0](*u[1:])

        # ---------------- attention phase ----------------
        # assemble extra window tiles
        nc.vector.tensor_copy(hfx[:, 0:64], hf_head[:])
        nc.vector.tensor_copy(hfx[:, 64:128], hfT[:, 16256:16320])
        nc.vector.tensor_copy(hbx[:, 0:64], hbT[:, 63:127])
        nc.vector.tensor_copy(hbx[:, 64:128], hb_head[:])

        with tc.tile_pool(name="psU", bufs=2, space="PSUM") as psu, \
             tc.tile_pool(name="uT", bufs=2) as utp, \
             tc.tile_pool(name="psA", bufs=2, space="PSUM") as psa:
            for gidx in range(33):
                if gidx < 32:
                    n = 512
                    hfr = hfT[:, 512 * gidx:512 * gidx + 512]
                    hbr = hbT[:, 512 * gidx + 127:512 * gidx + 127 + 512]
                    aout = att_dram[0:1, 512 * gidx:512 * gidx + 512]
                else:
                    n = 128
                    hfr = hfx[:]
                    hbr = hbx[:]
                    aout = att_dram[0:1, NQ:NQ + 128]
                pa = psa.tile([1, 512], F32, tag="psA", name="psA")
                for c2 in range(2):
                    pu = psu.tile([128, 512], F32, tag="psU", name="psU")
                    nc.tensor.matmul(pu[:, 0:n], wom[0][:, 128 * c2:128 * c2 + 128],
                                     hfr, start=True, stop=False)
                    nc.tensor.matmul(pu[:, 0:n], wom[1][:, 128 * c2:128 * c2 + 128],
                                     hbr, start=False, stop=True)
                    ut = utp.tile([128, 512], BF, tag="uT", name="uT")
                    nc.scalar.activation(ut[:, 0:n], pu[:, 0:n], AF.Tanh)
                    nc.tensor.matmul(pa[0:1, 0:n], uo[c2][:], ut[:, 0:n],
                                     start=(c2 == 0), stop=(c2 == 1))
                asb = utp.tile([1, 512], F32, tag="asb", name="asb")
                nc.vector.tensor_copy(asb[0:1, 0:n], pa[0:1, 0:n])
                nc.sync.dma_start(aout, asb[0:1, 0:n])

        # att -> column-major e
        with tc.tile_pool(name="psT", bufs=2, space="PSUM") as pst, \
             tc.tile_pool(name="anm", bufs=1) as anmp:
            att_nm = anmp.tile([128, 128], F32, tag="anm", name="anm")
            nc.sync.dma_start(
                att_nm[:],
                att_dram[0:1, 0:NQ].rearrange("a (n p) -> (a n) p", p=128))
            ps_a = pst.tile([128, 128], F32, tag="psT", name="psT")
            nc.tensor.transpose(ps_a[:], att_nm[:], identf[:])
            nc.scalar.activation(e_cm[:, 0:128], ps_a[:], AF.Exp)
            att_x = anmp.tile([128, 1], F32, tag="attx", name="attx")
            nc.sync.dma_start(
                att_x[:],
                att_dram[0:1, NQ:NQ + 128].rearrange("a (n p) -> (a n) p", p=1))
            nc.scalar.activation(e_cm[:, 128:129], att_x[:], AF.Exp)

        # ragged context accumulation
        with tc.tile_pool(name="psT2", bufs=2, space="PSUM") as pst2, \
             tc.tile_pool(name="yp", bufs=2) as yp, \
             tc.tile_pool(name="iw", bufs=2) as iwp, \
             tc.tile_pool(name="psC", bufs=1, space="PSUM") as psc:
            ctxp = [psc.tile([128, 257], F32, tag=f"ctxp{k}", name=f"ctxp{k}") for k in range(2)]
            for nti in range(NTILE + 1):
                if nti < NTILE:
                    hfr = hfT[:, 128 * nti:128 * nti + 128]
                    hbr = hbT[:, 128 * nti + 127:128 * nti + 255]
                else:
                    hfr = hfx[:]
                    hbr = hbx[:]
                ps_t = pst2.tile([128, 256], BF, tag="psT2", name="psT2")
                nc.tensor.transpose(ps_t[:, 0:128], hfr, identb[:])
                nc.tensor.transpose(ps_t[:, 128:256], hbr, identb[:])
                y = yp.tile([128, 257], BF, tag="y", name="y")
                nc.vector.tensor_copy(y[:, 0:256], ps_t[:])
                nc.vector.memset(y[:, 256:257], 1.0)
                iw = iwp.tile([128, 256], BF, tag="iw", name="iw")
                nc.vector.tensor_scalar(iw[:], iota_t[:],
                                        seg_t[:, nti:nti + 1],
                                        e_cm[:, nti:nti + 1],
                                        ALU.is_equal, ALU.mult)
                for k in range(2):
                    nc.tensor.matmul(ctxp[k][:], iw[:, 128 * k:128 * k + 128],
                                     y[:], start=(nti == 0), stop=(nti == NTILE))
            for k in range(2):
                nc.vector.tensor_copy(ctx_sb[k][:], ctxp[k][:])
        for k in range(2):
            nc.sync.dma_start(ctx_out[128 * k:128 * k + 128, :], ctx_sb[k][:])

    nc.finalize()
    _BUILT["nc"] = nc
    return nc


def _host_prep(inputs):
    x = np.asarray(inputs["sentence"], np.float32)
    doc_mask = np.asarray(inputs["doc_mask"]).astype(np.int64)
    h0g = np.asarray(inputs["h0"], np.float32)
    c0g = np.asarray(inputs["c0"], np.float32)

    perm = np.r_[0:128, 128:256, 384:512, 256:384]  # i,f,o,g order

    def wprep(w):  # [4H, X] -> lhsT [X, 4H] with gate perm, bf16
        return np.ascontiguousarray(w.astype(np.float32).T[:, perm]).astype(BF16)

    wih = {d: wprep(np.asarray(inputs[f"w_ih_{s}"], np.float32))
           for d, s in ((0, "f"), (1, "b"))}
    whh = {d: wprep(np.asarray(inputs[f"w_hh_{s}"], np.float32))
           for d, s in ((0, "f"), (1, "b"))}
    bias = {d: (np.asarray(inputs[f"b_ih_{s}"], np.float32)
                + np.asarray(inputs[f"b_hh_{s}"], np.float32))[perm]
            for d, s in ((0, "f"), (1, "b"))}
    bc = np.zeros((128, 8), np.float32)
    for d in range(2):
        for j in range(4):
            bc[:, d * 4 + j] = bias[d][128 * j:128 * j + 128]

    wom = np.asarray(inputs["w_omega"], np.float32).astype(BF16)
    uo = np.asarray(inputs["u_omega"], np.float32).astype(BF16)
    iota = np.tile(np.arange(256, dtype=np.float32), (128, 1))
    identb = np.eye(128, dtype=np.float32).astype(BF16)
    identf = np.eye(128, dtype=np.float32)

    seg_global = np.searchsorted(doc_mask, np.arange(T), side="right")

    in_maps = []
    s_los = []
    xpad = np.zeros((T + 512, D), np.float32)
    xpad[64:64 + T] = x  # global row r ↔ token r - 64
    for c in range(NCORE):
        tc0 = c * PC
        xs = xpad[tc0:tc0 + SH]  # token tc0-64+i at row i
        xT = np.ascontiguousarray(xs.T).astype(BF16)

        # seeds
        h0f = np.zeros((128, 128), np.float32)
        c0f = np.zeros((128, 128), np.float32)
        h0b = np.zeros((128, 128), np.float32)
        c0b = np.zeros((128, 128), np.float32)
        if c == 0:
            h0f[:, 0] = h0g[0]
            c0f[:, 0] = c0g[0]
        if c == NCORE - 1:
            h0b[:, 126] = h0g[1]
            c0b[:, 126] = c0g[1]

        # segment ids, col-major [128, 129]
        segm = np.full((128, 129), -1.0, np.float32)
        toks_main = tc0 + 64 + np.arange(NQ)
        valid = toks_main < T
        if c == NCORE - 1:
            valid &= (np.arange(NQ) < 16256)  # tail handled by W_tail
        toks_extra = np.full(128, -1, np.int64)
        if c == 0:
            toks_extra[0:64] = np.arange(64)          # W_head: tokens [0,64)
        if c == NCORE - 1:
            toks_extra[64:128] = T - 64 + np.arange(64)  # W_tail
        all_toks = np.concatenate([toks_main[valid],
                                   toks_extra[toks_extra >= 0]])
        s_lo = int(seg_global[all_toks].min()) if all_toks.size else 0
        s_hi = int(seg_global[all_toks].max()) if all_toks.size else 0
        assert s_hi - s_lo < SWIN, f"segment window too wide: {s_hi - s_lo}"
        s_los.append(s_lo)
        sm = np.where(valid, seg_global[np.minimum(toks_main, T - 1)] - s_lo,
                      -1.0).astype(np.float32)
        segm[:, 0:128] = sm.reshape(128, 128).T  # segm[p, n] = seg(q=128n+p)
        se = np.full(128, -1.0, np.float32)
        mask_x = toks_extra >= 0
        se[mask_x] = seg_global[toks_extra[mask_x]] - s_lo
        segm[:, 128] = se

        in_maps.append({
            "xT": xT,
            "wih_f": wih[0], "wih_b": wih[1],
            "whh_f": whh[0], "whh_b": whh[1],
            "bc": bc,
            "h0f": h0f.astype(BF16), "c0f": c0f.astype(BF16),
            "h0b": h0b.astype(BF16), "c0b": c0b.astype(BF16),
            "wom": wom, "uo": uo, "iota": iota,
            "identb": identb, "identf": identf,
            "seg": segm,
        })
    return in_maps, s_los


def kernel(**inputs):
    global LAST_RESULT
    from concourse.bass_utils import run_bass_kernel_spmd

    nc = _build()
    in_maps, s_los = _host_prep(inputs)
    res = run_bass_kernel_spmd(nc, in_maps, core_ids=list(range(NCORE)))
    LAST_RESULT = res

    G = np.zeros((S + SWIN, 257), np.float64)
    for c in range(NCORE):
        ctx = np.asarray(res.results[c]["ctx"], np.float32)
        G[s_los[c]:s_los[c] + SWIN] += ctx
    G = G[:S]
    z = G[:, 256]
    ctx = G[:, :256] / np.where(z == 0, 1.0, z)[:, None]
    w_tag = np.asarray(inputs["w_tag"], np.float32)
    b_tag = np.asarray(inputs["b_tag"], np.float32)
    out = ctx.astype(np.float32) @ w_tag.T + b_tag
    return out.astype(np.float32)



# revision 26
# speedup vs baseline: 1.5742x; 1.0051x over previous
"""Bass/Trainium2 kernel for nn_BiLSTM_Tok_83837761618147.

Strategy (8 NeuronCores, SPMD, full inputs in / full output out):
  - Token dim sharded 8 ways (16384 tokens/core, with halos).
  - BiLSTM parallelized via chunked recurrence with burn-in: each core runs
    128 lanes x (128+64) steps forward and 128 lanes x (129+64) steps
    backward (state forgets exponentially; 64 warmup steps reach fp32
    accuracy; the true h0/c0-seeded lanes cover the sequence ends exactly).
  - Gate pre-activations computed by PE matmuls directly into PSUM
    (bias via a K=4 indicator matmul); w_hh @ h accumulated on top.
  - Attention (tanh/logits/exp) + ragged segment softmax-sum done on
    device via an e-weighted one-hot (token x segment-window) matmul.
  - Host combines per-core partial [segment, 257] sums, normalizes, and
    applies the tiny tag projection.
"""

import numpy as np
import ml_dtypes

BF16 = ml_dtypes.bfloat16

T = 131072
D = 256
H = 128
HID = 256
TAGS = 10
S = 1024
NCORE = 8
PC = T // NCORE          # 16384 tokens per core
B = 64                   # burn-in steps
LF = 128                 # forward lane length (tokens per lane)
LB = 129                 # backward lane length
NL = 128                 # lanes per direction
NSF = B + LF             # 192 forward steps
NSB = B + LB             # 193 backward steps
SH = 16704               # x shard rows [tc0-64, tc0-64+SH)
SWIN = 256               # segment window width per core
NQ = PC                  # main attention window positions
NTILE = NQ // 128        # 128 main token tiles
HBW = LB * NL - LB + LB + B  # hbT width: 16512
HBT_W = 16512
ATT_W = NQ + 128         # att buffer width (main + extra tile)
RW = 16                  # pre-gate ring depth (steps)

_BUILT = {}
LAST_RESULT = None


def _build():
    if "nc" in _BUILT:
        return _BUILT["nc"]
    import contextlib
    from concourse import bacc, mybir
    from concourse.tile import TileContext

    F32 = mybir.dt.float32
    BF = mybir.dt.bfloat16
    AF = mybir.ActivationFunctionType
    ALU = mybir.AluOpType

    nc = bacc.Bacc()

    def din(name, shape, dt):
        return nc.declare_dram_parameter(name, list(shape), dt, isOutput=False)

    x_in = din("xT", [256, SH], BF)
    wih_f_in = din("wih_f", [256, 512], BF)
    wih_b_in = din("wih_b", [256, 512], BF)
    whh_f_in = din("whh_f", [128, 512], BF)
    whh_b_in = din("whh_b", [128, 512], BF)
    bc_in = din("bc", [128, 8], F32)
    h0f_in = din("h0f", [128, 128], BF)
    c0f_in = din("c0f", [128, 128], BF)
    h0b_in = din("h0b", [128, 128], BF)
    c0b_in = din("c0b", [128, 128], BF)
    wom_in = din("wom", [256, 256], BF)
    uo_in = din("uo", [256, 1], BF)
    iota_in = din("iota", [128, 256], F32)
    identb_in = din("identb", [128, 128], BF)
    identf_in = din("identf", [128, 128], F32)
    seg_in = din("seg", [128, 129], F32)
    ctx_out = nc.declare_dram_parameter("ctx", [256, 257], F32, isOutput=True)
    att_dram = nc.dram_tensor("att_stage", [1, ATT_W], F32)

    with TileContext(nc) as tc, contextlib.ExitStack() as ctx:
        pp = ctx.enter_context(tc.tile_pool(name="persist", bufs=1))

        xT0 = pp.tile([128, SH], BF, tag="xT0", name="xT0")
        xT1 = pp.tile([128, SH], BF, tag="xT1", name="xT1")
        hfT = pp.tile([128, NQ], BF, tag="hfT", name="hfT")
        hbT = pp.tile([128, HBT_W], BF, tag="hbT", name="hbT")
        hf_head = pp.tile([128, 64], BF, tag="hfh", name="hfh")
        hb_head = pp.tile([128, 64], BF, tag="hbh", name="hbh")
        wih = [[pp.tile([128, 512], BF, tag=f"wih{d}{k}", name=f"wih{d}{k}") for k in range(2)]
               for d in range(2)]
        whh = [pp.tile([128, 512], BF, tag=f"whh{d}", name=f"whh{d}") for d in range(2)]
        bc = pp.tile([128, 8], F32, tag="bc", name="bc")
        gring = pp.tile([128, 8 * 128 * RW], BF, tag="gring", name="gring")
        h0 = [pp.tile([128, 128], BF, tag=f"h0{d}", name=f"h0{d}") for d in range(2)]
        c0 = [pp.tile([128, 128], BF, tag=f"c0{d}", name=f"c0{d}") for d in range(2)]
        wom = [pp.tile([128, 256], BF, tag=f"wom{k}", name=f"wom{k}") for k in range(2)]
        uo = [pp.tile([128, 1], BF, tag=f"uo{k}", name=f"uo{k}") for k in range(2)]
        iota_t = pp.tile([128, 256], F32, tag="iota", name="iota")
        identb = pp.tile([128, 128], BF, tag="identb", name="identb")
        identf = pp.tile([128, 128], F32, tag="identf", name="identf")
        seg_t = pp.tile([128, 129], F32, tag="seg", name="seg")
        CFB = pp.tile([128, 256], BF, tag="CFB", name="CFB")
        e_cm = pp.tile([128, 129], F32, tag="ecm", name="ecm")
        hfx = pp.tile([128, 128], BF, tag="hfx", name="hfx")
        hbx = pp.tile([128, 128], BF, tag="hbx", name="hbx")
        ctx_sb = [pp.tile([128, 257], F32, tag=f"ctxsb{k}", name=f"ctxsb{k}") for k in range(2)]

        # ---- input DMAs ----
        nc.sync.dma_start(xT0[:], x_in[0:128, :])
        nc.sync.dma_start(xT1[:], x_in[128:256, :])
        for d, t_ in ((0, wih_f_in), (1, wih_b_in)):
            nc.sync.dma_start(wih[d][0][:], t_[0:128, :])
            nc.sync.dma_start(wih[d][1][:], t_[128:256, :])
        nc.sync.dma_start(whh[0][:], whh_f_in[:])
        nc.sync.dma_start(whh[1][:], whh_b_in[:])
        nc.sync.dma_start(bc[:], bc_in[:])
        nc.sync.dma_start(h0[0][:], h0f_in[:])
        nc.sync.dma_start(c0[0][:], c0f_in[:])
        nc.sync.dma_start(h0[1][:], h0b_in[:])
        nc.sync.dma_start(c0[1][:], c0b_in[:])
        nc.sync.dma_start(wom[0][:], wom_in[0:128, :])
        nc.sync.dma_start(wom[1][:], wom_in[128:256, :])
        nc.sync.dma_start(uo[0][:], uo_in[0:128, :])
        nc.sync.dma_start(uo[1][:], uo_in[128:256, :])
        nc.sync.dma_start(iota_t[:], iota_in[:])
        nc.sync.dma_start(identb[:], identb_in[:])
        nc.sync.dma_start(identf[:], identf_in[:])
        nc.sync.dma_start(seg_t[:], seg_in[:])

        # init cell state from seeds: CFB = [c0f | c0b]
        nc.vector.tensor_copy(CFB[:, 0:128], c0[0][:])
        nc.vector.tensor_copy(CFB[:, 128:256], c0[1][:])

        xT = [xT0, xT1]

        def grv():
            # slot-major ring: col = w*1024 + c*128 + l
            return gring[:].rearrange("p (w c l) -> p w c l", w=RW, c=8)

        with tc.tile_pool(name="psG", bufs=2, space="PSUM") as psg, \
             tc.tile_pool(name="psB", bufs=4, space="PSUM") as psb, \
             tc.tile_pool(name="sig", bufs=3) as sigp, \
             tc.tile_pool(name="tg", bufs=3) as tgp, \
             tc.tile_pool(name="tcn", bufs=3) as tcp, \
             tc.tile_pool(name="tmp1", bufs=3) as t1p, \
             tc.tile_pool(name="tmp2", bufs=3) as t2p, \
             tc.tile_pool(name="hsc", bufs=4) as hscp:

            # ---- pre-gate batch units: G_pre = x @ w_ih.T + b, evacuated to
            # the bf16 ring `gring` 16 steps ahead of consumption ----
            def emit_unit(sb, h2, c):
                d, j = divmod(c, 4)
                ps = [psb.tile([128, 512], F32, tag="psb", name="psb")
                      for _ in range(2)]
                for kh in range(2):
                    for b_ in range(2):
                        s0 = sb * 16 + h2 * 8 + b_ * 4
                        if d == 0:
                            rhs = xT[kh][:, 64 + s0:64 + s0 + 128 * 128].rearrange(
                                "p (l q) -> p l q", q=128)[:, :, 0:4]
                        else:
                            base = 190 - s0
                            rhs = xT[kh][:, base:base + 129 * 128].rearrange(
                                "p (l q) -> p l q", q=129)[:, :, 0:4]
                        nc.tensor.matmul(ps[b_][:],
                                         wih[d][kh][:, 128 * j:128 * j + 128],
                                         rhs, start=(kh == 0), stop=(kh == 1))
                for b_ in range(2):
                    w0 = h2 * 8 + b_ * 4
                    dst = grv()[:, w0:w0 + 4, c:c + 1, :]
                    src = ps[b_][:].rearrange("p (l a q) -> p q a l", a=1, q=4)
                    if b_ == 0:
                        nc.vector.tensor_scalar(dst, src, bc[:, c:c + 1], None,
                                                ALU.add)
                    else:
                        nc.scalar.activation(dst, src, AF.Identity,
                                             bias=bc[:, c:c + 1])

            def emit_unit12(c):
                # step 192, bwd chunks only
                d, j = divmod(c, 4)
                ps = psb.tile([128, 512], F32, tag="psb", name="psb")
                for kh in range(2):
                    rhs = xT[kh][:, 1:1 + 129 * 127 + 1:129]
                    nc.tensor.matmul(ps[:, 0:128],
                                     wih[d][kh][:, 128 * j:128 * j + 128],
                                     rhs, start=(kh == 0), stop=(kh == 1))
                dst = grv()[:, 3:4, c:c + 1, :]
                src = ps[:, 0:128].rearrange("p (a b l) -> p a b l", a=1, b=1)
                nc.vector.tensor_scalar(dst, src, bc[:, c:c + 1], None, ALU.add)

            units = []
            for sb in range(12):
                for h2 in range(2):
                    for c in range(8):
                        units.append((emit_unit, sb, h2, c))
            for c in range(4, 8):
                units.append((emit_unit12, c))

            def inject(s):
                # load G_pre for step s into a fresh PSUM gate tile.
                # G layout: bank0 = [i0 f0 i1 f1], bank1 = [o0 g0 o1 g1]
                g = psg.tile([128, 1024], F32, tag="G", name="G")
                wf = s % RW
                blk = (wf // 4) * 4
                wb = blk + 3 - (s % 4)
                nc.tensor.matmul(g[:, 0:256], identb[:],
                                 gring[:, wf * 1024:wf * 1024 + 256],
                                 start=True, stop=False)
                nc.tensor.matmul(g[:, 256:512], identb[:],
                                 gring[:, wb * 1024 + 512:wb * 1024 + 768],
                                 start=False, stop=False)
                nc.tensor.matmul(g[:, 512:768], identb[:],
                                 gring[:, wf * 1024 + 256:wf * 1024 + 512],
                                 start=True, stop=False)
                nc.tensor.matmul(g[:, 768:1024], identb[:],
                                 gring[:, wb * 1024 + 768:wb * 1024 + 1024],
                                 start=False, stop=False)
                return g

            for u in units[:9]:
                u[0](*u[1:])
            g_cur = inject(0)

            # G column offset for gate j (i,f,o,g) of dir d
            def gcol(d, j):
                return (256 * d + 128 * j if j < 2
                        else 512 + 256 * d + 128 * (j - 2))

            hs_prev = None
            for s in range(NSB):
                g = g_cur
                # w_hh matmuls: bank0 gates (f, i) first so sig_if starts early
                for j in (1, 0, 3, 2):
                    for d in range(2):
                        if d == 0 and s >= NSF:
                            continue
                        hs = h0[d][:] if s == 0 else hs_prev[:, 128 * d:128 * d + 128]
                        co = gcol(d, j)
                        nc.tensor.matmul(
                            g[:, co:co + 128],
                            whh[d][:, 128 * j:128 * j + 128], hs,
                            start=False, stop=True)
                if s + 1 < NSB:
                    g_cur = inject(s + 1)
                # gates: sig_if = one contiguous op over bank0
                sig = sigp.tile([128, 768], BF, tag="sig", name="sig")
                nc.scalar.activation(sig[:, 0:512], g[:, 0:512], AF.Sigmoid)
                gq = g[:, 512:1024].rearrange("p (a q) -> p a q", q=256)
                tg = tgp.tile([128, 256], BF, tag="tg", name="tg")
                nc.scalar.activation(tg[:].rearrange("p (a q) -> p a q", q=128),
                                     gq[:, :, 128:256], AF.Tanh)
                nc.scalar.activation(
                    sig[:, 512:768].rearrange("p (a q) -> p a q", q=128),
                    gq[:, :, 0:128], AF.Sigmoid)
                # c update
                sigr = sig[:, 0:512].rearrange("p (a q) -> p a q", q=256)
                t1 = t1p.tile([128, 256], BF, tag="t1", name="t1")
                t2 = t2p.tile([128, 256], BF, tag="t2", name="t2")
                cr = CFB[:].rearrange("p (a q) -> p a q", q=128)
                nc.vector.tensor_tensor(t1[:].rearrange("p (a q) -> p a q", q=128),
                                        sigr[:, :, 128:256], cr, ALU.mult)
                nc.vector.tensor_tensor(t2[:].rearrange("p (a q) -> p a q", q=128),
                                        sigr[:, :, 0:128],
                                        tg[:].rearrange("p (a q) -> p a q", q=128),
                                        ALU.mult)
                nc.vector.tensor_tensor(CFB[:], t1[:], t2[:], ALU.add)
                tcn = tcp.tile([128, 256], BF, tag="tcn", name="tcn")
                nc.scalar.activation(tcn[:], CFB[:], AF.Tanh)
                # h = sigma_o * tanh(c) -> contiguous scratch (fast DVE write)
                hs_cur = hscp.tile([128, 256], BF, tag="hsc", name="hsc")
                for d in range(2):
                    if d == 0 and s >= NSF:
                        continue
                    nc.vector.tensor_tensor(hs_cur[:, 128 * d:128 * d + 128],
                                            sig[:, 512 + 128 * d:640 + 128 * d],
                                            tcn[:, 128 * d:128 * d + 128],
                                            ALU.mult)
                # off-critical-path strided copies into token-major h stores
                if s >= B:
                    if s < NSF:
                        nc.gpsimd.tensor_copy(
                            hfT[:, s - 64:s - 64 + 127 * 128 + 1:128],
                            hs_cur[:, 0:128])
                    a = 192 - s
                    nc.gpsimd.tensor_copy(
                        hbT[:, a:a + 129 * 127 + 1:129],
                        hs_cur[:, 128:256])
                if s < B:
                    nc.gpsimd.tensor_copy(hf_head[:, s:s + 1],
                                          hs_cur[:, 0:1])
                    nc.gpsimd.tensor_copy(hb_head[:, 63 - s:64 - s],
                                          hs_cur[:, 254:255])
                hs_prev = hs_cur
                if 9 + s < len(units):
                    u = units[9 + s]
                    u[0](*u[1:])

        # ---------------- attention phase ----------------
        # assemble extra window tiles
        nc.vector.tensor_copy(hfx[:, 0:64], hf_head[:])
        nc.vector.tensor_copy(hfx[:, 64:128], hfT[:, 16256:16320])
        nc.vector.tensor_copy(hbx[:, 0:64], hbT[:, 63:127])
        nc.vector.tensor_copy(hbx[:, 64:128], hb_head[:])

        with tc.tile_pool(name="psU", bufs=2, space="PSUM") as psu, \
             tc.tile_pool(name="uT", bufs=2) as utp, \
             tc.tile_pool(name="psA", bufs=2, space="PSUM") as psa:
            for gidx in range(33):
                if gidx < 32:
                    n = 512
                    hfr = hfT[:, 512 * gidx:512 * gidx + 512]
                    hbr = hbT[:, 512 * gidx + 127:512 * gidx + 127 + 512]
                    aout = att_dram[0:1, 512 * gidx:512 * gidx + 512]
                else:
                    n = 128
                    hfr = hfx[:]
                    hbr = hbx[:]
                    aout = att_dram[0:1, NQ:NQ + 128]
                pa = psa.tile([1, 512], F32, tag="psA", name="psA")
                for c2 in range(2):
                    pu = psu.tile([128, 512], F32, tag="psU", name="psU")
                    nc.tensor.matmul(pu[:, 0:n], wom[0][:, 128 * c2:128 * c2 + 128],
                                     hfr, start=True, stop=False)
                    nc.tensor.matmul(pu[:, 0:n], wom[1][:, 128 * c2:128 * c2 + 128],
                                     hbr, start=False, stop=True)
                    ut = utp.tile([128, 512], BF, tag="uT", name="uT")
                    nc.scalar.activation(ut[:, 0:n], pu[:, 0:n], AF.Tanh)
                    nc.tensor.matmul(pa[0:1, 0:n], uo[c2][:], ut[:, 0:n],
                                     start=(c2 == 0), stop=(c2 == 1))
                asb = utp.tile([1, 512], F32, tag="asb", name="asb")
                nc.vector.tensor_copy(asb[0:1, 0:n], pa[0:1, 0:n])
                nc.sync.dma_start(aout, asb[0:1, 0:n])

        # att -> column-major e
        with tc.tile_pool(name="psT", bufs=2, space="PSUM") as pst, \
             tc.tile_pool(name="anm", bufs=1) as anmp:
            att_nm = anmp.tile([128, 128], F32, tag="anm", name="anm")
            nc.sync.dma_start(
                att_nm[:],
                att_dram[0:1, 0:NQ].rearrange("a (n p) -> (a n) p", p=128))
            ps_a = pst.tile([128, 128], F32, tag="psT", name="psT")
            nc.tensor.transpose(ps_a[:], att_nm[:], identf[:])
            nc.scalar.activation(e_cm[:, 0:128], ps_a[:], AF.Exp)
            att_x = anmp.tile([128, 1], F32, tag="attx", name="attx")
            nc.sync.dma_start(
                att_x[:],
                att_dram[0:1, NQ:NQ + 128].rearrange("a (n p) -> (a n) p", p=1))
            nc.scalar.activation(e_cm[:, 128:129], att_x[:], AF.Exp)

        # ragged context accumulation
        with tc.tile_pool(name="psT2", bufs=2, space="PSUM") as pst2, \
             tc.tile_pool(name="yp", bufs=2) as yp, \
             tc.tile_pool(name="iw", bufs=2) as iwp, \
             tc.tile_pool(name="psC", bufs=1, space="PSUM") as psc:
            ctxp = [psc.tile([128, 257], F32, tag=f"ctxp{k}", name=f"ctxp{k}") for k in range(2)]
            for nti in range(NTILE + 1):
                if nti < NTILE:
                    hfr = hfT[:, 128 * nti:128 * nti + 128]
                    hbr = hbT[:, 128 * nti + 127:128 * nti + 255]
                else:
                    hfr = hfx[:]
                    hbr = hbx[:]
                ps_t = pst2.tile([128, 256], BF, tag="psT2", name="psT2")
                nc.tensor.transpose(ps_t[:, 0:128], hfr, identb[:])
                nc.tensor.transpose(ps_t[:, 128:256], hbr, identb[:])
                y = yp.tile([128, 257], BF, tag="y", name="y")
                nc.vector.tensor_copy(y[:, 0:256], ps_t[:])
                nc.vector.memset(y[:, 256:257], 1.0)
                iw = iwp.tile([128, 256], BF, tag="iw", name="iw")
                nc.vector.tensor_scalar(iw[:], iota_t[:],
                                        seg_t[:, nti:nti + 1],
                                        e_cm[:, nti:nti + 1],
                                        ALU.is_equal, ALU.mult)
                for k in range(2):
                    nc.tensor.matmul(ctxp[k][:], iw[:, 128 * k:128 * k + 128],
                                     y[:], start=(nti == 0), stop=(nti == NTILE))
            for k in range(2):
                nc.vector.tensor_copy(ctx_sb[k][:], ctxp[k][:])
        for k in range(2):
            nc.sync.dma_start(ctx_out[128 * k:128 * k + 128, :], ctx_sb[k][:])

    nc.finalize()
    _BUILT["nc"] = nc
    return nc


def _host_prep(inputs):
    x = np.asarray(inputs["sentence"], np.float32)
    doc_mask = np.asarray(inputs["doc_mask"]).astype(np.int64)
    h0g = np.asarray(inputs["h0"], np.float32)
    c0g = np.asarray(inputs["c0"], np.float32)

    perm = np.r_[0:128, 128:256, 384:512, 256:384]  # i,f,o,g order

    def wprep(w):  # [4H, X] -> lhsT [X, 4H] with gate perm, bf16
        return np.ascontiguousarray(w.astype(np.float32).T[:, perm]).astype(BF16)

    wih = {d: wprep(np.asarray(inputs[f"w_ih_{s}"], np.float32))
           for d, s in ((0, "f"), (1, "b"))}
    whh = {d: wprep(np.asarray(inputs[f"w_hh_{s}"], np.float32))
           for d, s in ((0, "f"), (1, "b"))}
    bias = {d: (np.asarray(inputs[f"b_ih_{s}"], np.float32)
                + np.asarray(inputs[f"b_hh_{s}"], np.float32))[perm]
            for d, s in ((0, "f"), (1, "b"))}
    bc = np.zeros((128, 8), np.float32)
    for d in range(2):
        for j in range(4):
            bc[:, d * 4 + j] = bias[d][128 * j:128 * j + 128]

    wom = np.asarray(inputs["w_omega"], np.float32).astype(BF16)
    uo = np.asarray(inputs["u_omega"], np.float32).astype(BF16)
    iota = np.tile(np.arange(256, dtype=np.float32), (128, 1))
    identb = np.eye(128, dtype=np.float32).astype(BF16)
    identf = np.eye(128, dtype=np.float32)

    seg_global = np.searchsorted(doc_mask, np.arange(T), side="right")

    in_maps = []
    s_los = []
    xpad = np.zeros((T + 512, D), np.float32)
    xpad[64:64 + T] = x  # global row r ↔ token r - 64
    for c in range(NCORE):
        tc0 = c * PC
        xs = xpad[tc0:tc0 + SH]  # token tc0-64+i at row i
        xT = np.ascontiguousarray(xs.T).astype(BF16)

        # seeds
        h0f = np.zeros((128, 128), np.float32)
        c0f = np.zeros((128, 128), np.float32)
        h0b = np.zeros((128, 128), np.float32)
        c0b = np.zeros((128, 128), np.float32)
        if c == 0:
            h0f[:, 0] = h0g[0]
            c0f[:, 0] = c0g[0]
        if c == NCORE - 1:
            h0b[:, 126] = h0g[1]
            c0b[:, 126] = c0g[1]

        # segment ids, col-major [128, 129]
        segm = np.full((128, 129), -1.0, np.float32)
        toks_main = tc0 + 64 + np.arange(NQ)
        valid = toks_main < T
        if c == NCORE - 1:
            valid &= (np.arange(NQ) < 16256)  # tail handled by W_tail
        toks_extra = np.full(128, -1, np.int64)
        if c == 0:
            toks_extra[0:64] = np.arange(64)          # W_head: tokens [0,64)
        if c == NCORE - 1:
            toks_extra[64:128] = T - 64 + np.arange(64)  # W_tail
        all_toks = np.concatenate([toks_main[valid],
                                   toks_extra[toks_extra >= 0]])
        s_lo = int(seg_global[all_toks].min()) if all_toks.size else 0
        s_hi = int(seg_global[all_toks].max()) if all_toks.size else 0
        assert s_hi - s_lo < SWIN, f"segment window too wide: {s_hi - s_lo}"
        s_los.append(s_lo)
        sm = np.where(valid, seg_global[np.minimum(toks_main, T - 1)] - s_lo,
                      -1.0).astype(np.float32)
        segm[:, 0:128] = sm.reshape(128, 128).T  # segm[p, n] = seg(q=128n+p)
        se = np.full(128, -1.0, np.float32)
        mask_x = toks_extra >= 0
        se[mask_x] = seg_global[toks_extra[mask_x]] - s_lo
        segm[:, 128] = se

        in_maps.append({
            "xT": xT,
            "wih_f": wih[0], "wih_b": wih[1],
            "whh_f": whh[0], "whh_b": whh[1],
            "bc": bc,
            "h0f": h0f.astype(BF16), "c0f": c0f.astype(BF16),
            "h0b": h0b.astype(BF16), "c0b": c0b.astype(BF16),
            "wom": wom, "uo": uo, "iota": iota,
            "identb": identb, "identf": identf,
            "seg": segm,
        })
    return in_maps, s_los


def kernel(**inputs):
    global LAST_RESULT
    from concourse.bass_utils import run_bass_kernel_spmd

    nc = _build()
    in_maps, s_los = _host_prep(inputs)
    res = run_bass_kernel_spmd(nc, in_maps, core_ids=list(range(NCORE)))
    LAST_RESULT = res

    G = np.zeros((S + SWIN, 257), np.float64)
    for c in range(NCORE):
        ctx = np.asarray(res.results[c]["ctx"], np.float32)
        G[s_los[c]:s_los[c] + SWIN] += ctx
    G = G[:S]
    z = G[:, 256]
    ctx = G[:, :256] / np.where(z == 0, 1.0, z)[:, None]
    w_tag = np.asarray(inputs["w_tag"], np.float32)
    b_tag = np.asarray(inputs["b_tag"], np.float32)
    out = ctx.astype(np.float32) @ w_tag.T + b_tag
    return out.astype(np.float32)



# revision 38
# speedup vs baseline: 1.8176x; 1.1547x over previous
"""Bass/Trainium2 kernel for nn_BiLSTM_Tok_83837761618147.

Strategy (8 NeuronCores, SPMD, full inputs in / full output out):
  - Token dim sharded 8 ways (16384 tokens/core, with halos).
  - BiLSTM parallelized via chunked recurrence with burn-in: each core runs
    128 lanes x (128+64) steps forward and 128 lanes x (129+64) steps
    backward (state forgets exponentially; 64 warmup steps reach fp32
    accuracy; the true h0/c0-seeded lanes cover the sequence ends exactly).
  - Gate pre-activations computed by PE matmuls directly into PSUM
    (bias via a K=4 indicator matmul); w_hh @ h accumulated on top.
  - Attention (tanh/logits/exp) + ragged segment softmax-sum done on
    device via an e-weighted one-hot (token x segment-window) matmul.
  - Host combines per-core partial [segment, 257] sums, normalizes, and
    applies the tiny tag projection.
"""

import numpy as np
import ml_dtypes

BF16 = ml_dtypes.bfloat16

T = 131072
D = 256
H = 128
HID = 256
TAGS = 10
S = 1024
NCORE = 8
PC = T // NCORE          # 16384 tokens per core
B = 32                   # burn-in steps (first/last B tokens computed on host)
LF = 128                 # forward lane length (tokens per lane)
LB = 129                 # backward lane length
NL = 128                 # lanes per direction
NSF = B + LF             # 192 forward steps
NSB = B + LB             # 193 backward steps
SH = 16704               # x shard rows [tc0-64, tc0-64+SH)
SWIN = 256               # segment window width per core
NQ = PC                  # main attention window positions
NTILE = NQ // 128        # 128 main token tiles
HBW = LB * NL - LB + LB + B  # hbT width: 16512
HBT_W = 16512
ATT_W = NQ + 128         # att buffer width (main + extra tile)
RW = 16                  # pre-gate ring depth (steps)

_BUILT = {}
LAST_RESULT = None


def _build():
    if "nc" in _BUILT:
        return _BUILT["nc"]
    import contextlib
    from concourse import bacc, mybir
    from concourse.tile import TileContext

    F32 = mybir.dt.float32
    BF = mybir.dt.bfloat16
    AF = mybir.ActivationFunctionType
    ALU = mybir.AluOpType

    nc = bacc.Bacc()

    def din(name, shape, dt):
        return nc.declare_dram_parameter(name, list(shape), dt, isOutput=False)

    x_in = din("xT", [256, SH], BF)
    wih_f_in = din("wih_f", [256, 512], BF)
    wih_b_in = din("wih_b", [256, 512], BF)
    whh_f_in = din("whh_f", [128, 512], BF)
    whh_b_in = din("whh_b", [128, 512], BF)
    bc_in = din("bc", [128, 8], F32)
    h0f_in = din("h0f", [128, 128], BF)
    c0f_in = din("c0f", [128, 128], BF)
    h0b_in = din("h0b", [128, 128], BF)
    c0b_in = din("c0b", [128, 128], BF)
    hfh_in = din("hfh", [128, 64 - B], BF)
    hbh_in = din("hbh", [128, 64 - B], BF)
    wom_in = din("wom", [256, 256], BF)
    uo_in = din("uo", [256, 1], BF)
    iota_in = din("iota", [128, 256], F32)
    identb_in = din("identb", [128, 128], BF)
    identf_in = din("identf", [128, 128], F32)
    seg_in = din("seg", [128, 129], F32)
    ctx_out = nc.declare_dram_parameter("ctx", [256, 257], F32, isOutput=True)
    att_dram = nc.dram_tensor("att_stage", [1, ATT_W], F32)

    with TileContext(nc) as tc, contextlib.ExitStack() as ctx:
        pp = ctx.enter_context(tc.tile_pool(name="persist", bufs=1))

        xT0 = pp.tile([128, SH], BF, tag="xT0", name="xT0")
        xT1 = pp.tile([128, SH], BF, tag="xT1", name="xT1")
        hfT = pp.tile([128, NQ], BF, tag="hfT", name="hfT")
        hbT = pp.tile([128, HBT_W], BF, tag="hbT", name="hbT")
        hf_head = pp.tile([128, 64], BF, tag="hfh", name="hfh")
        hb_head = pp.tile([128, 64], BF, tag="hbh", name="hbh")
        wih = [[pp.tile([128, 512], BF, tag=f"wih{d}{k}", name=f"wih{d}{k}") for k in range(2)]
               for d in range(2)]
        whh = [pp.tile([128, 512], BF, tag=f"whh{d}", name=f"whh{d}") for d in range(2)]
        bc = pp.tile([128, 8], F32, tag="bc", name="bc")
        gring = pp.tile([128, 8 * 128 * RW], BF, tag="gring", name="gring")
        h0 = [pp.tile([128, 128], BF, tag=f"h0{d}", name=f"h0{d}") for d in range(2)]
        c0 = [pp.tile([128, 128], BF, tag=f"c0{d}", name=f"c0{d}") for d in range(2)]
        wom = [pp.tile([128, 256], BF, tag=f"wom{k}", name=f"wom{k}") for k in range(2)]
        uo = [pp.tile([128, 1], BF, tag=f"uo{k}", name=f"uo{k}") for k in range(2)]
        iota_t = pp.tile([128, 256], F32, tag="iota", name="iota")
        identb = pp.tile([128, 128], BF, tag="identb", name="identb")
        identf = pp.tile([128, 128], F32, tag="identf", name="identf")
        seg_t = pp.tile([128, 129], F32, tag="seg", name="seg")
        CFB = pp.tile([128, 256], BF, tag="CFB", name="CFB")
        e_cm = pp.tile([128, 129], F32, tag="ecm", name="ecm")
        hfx = pp.tile([128, 128], BF, tag="hfx", name="hfx")
        hbx = pp.tile([128, 128], BF, tag="hbx", name="hbx")
        ctx_sb = [pp.tile([128, 257], F32, tag=f"ctxsb{k}", name=f"ctxsb{k}") for k in range(2)]

        # ---- input DMAs ----
        nc.sync.dma_start(xT0[:], x_in[0:128, :])
        nc.sync.dma_start(xT1[:], x_in[128:256, :])
        for d, t_ in ((0, wih_f_in), (1, wih_b_in)):
            nc.sync.dma_start(wih[d][0][:], t_[0:128, :])
            nc.sync.dma_start(wih[d][1][:], t_[128:256, :])
        nc.sync.dma_start(whh[0][:], whh_f_in[:])
        nc.sync.dma_start(whh[1][:], whh_b_in[:])
        nc.sync.dma_start(bc[:], bc_in[:])
        nc.sync.dma_start(h0[0][:], h0f_in[:])
        nc.sync.dma_start(c0[0][:], c0f_in[:])
        nc.sync.dma_start(h0[1][:], h0b_in[:])
        nc.sync.dma_start(c0[1][:], c0b_in[:])
        nc.sync.dma_start(wom[0][:], wom_in[0:128, :])
        nc.sync.dma_start(wom[1][:], wom_in[128:256, :])
        nc.sync.dma_start(uo[0][:], uo_in[0:128, :])
        nc.sync.dma_start(uo[1][:], uo_in[128:256, :])
        nc.sync.dma_start(iota_t[:], iota_in[:])
        nc.sync.dma_start(identb[:], identb_in[:])
        nc.sync.dma_start(identf[:], identf_in[:])
        nc.sync.dma_start(seg_t[:], seg_in[:])
        # host-computed exact h for the first/last (64-B) tokens (cores 0 / 7)
        nc.sync.dma_start(hf_head[:, 0:64 - B], hfh_in[:])
        nc.sync.dma_start(hb_head[:, B:64], hbh_in[:])

        # init cell state from seeds: CFB = [c0f | c0b]
        nc.vector.tensor_copy(CFB[:, 0:128], c0[0][:])
        nc.vector.tensor_copy(CFB[:, 128:256], c0[1][:])

        xT = [xT0, xT1]

        def grv():
            # slot-major ring: col = w*1024 + c*128 + l
            return gring[:].rearrange("p (w c l) -> p w c l", w=RW, c=8)

        with tc.tile_pool(name="psG", bufs=2, space="PSUM") as psg, \
             tc.tile_pool(name="psB", bufs=4, space="PSUM") as psb, \
             tc.tile_pool(name="sig", bufs=3) as sigp, \
             tc.tile_pool(name="tg", bufs=3) as tgp, \
             tc.tile_pool(name="tcn", bufs=3) as tcp, \
             tc.tile_pool(name="tmp1", bufs=3) as t1p, \
             tc.tile_pool(name="tmp2", bufs=3) as t2p, \
             tc.tile_pool(name="hsc", bufs=4) as hscp:

            # ---- pre-gate batch units: G_pre = x @ w_ih.T + b, evacuated to
            # the bf16 ring `gring` 16 steps ahead of consumption ----
            def emit_unit(sb, h2, c):
                d, j = divmod(c, 4)
                ps = [psb.tile([128, 512], F32, tag="psb", name="psb")
                      for _ in range(2)]
                for kh in range(2):
                    for b_ in range(2):
                        s0 = sb * 16 + h2 * 8 + b_ * 4
                        if d == 0:
                            fb = (128 - B) + s0
                            rhs = xT[kh][:, fb:fb + 128 * 128].rearrange(
                                "p (l q) -> p l q", q=128)[:, :, 0:4]
                        else:
                            base = (126 + B) - s0
                            rhs = xT[kh][:, base:base + 129 * 128].rearrange(
                                "p (l q) -> p l q", q=129)[:, :, 0:4]
                        nc.tensor.matmul(ps[b_][:],
                                         wih[d][kh][:, 128 * j:128 * j + 128],
                                         rhs, start=(kh == 0), stop=(kh == 1))
                for b_ in range(2):
                    w0 = h2 * 8 + b_ * 4
                    dst = grv()[:, w0:w0 + 4, c:c + 1, :]
                    src = ps[b_][:].rearrange("p (l a q) -> p q a l", a=1, q=4)
                    if b_ == 0:
                        nc.vector.tensor_scalar(dst, src, bc[:, c:c + 1], None,
                                                ALU.add)
                    else:
                        nc.scalar.activation(dst, src, AF.Identity,
                                             bias=bc[:, c:c + 1])

            def emit_unit12(c):
                # step 192, bwd chunks only
                d, j = divmod(c, 4)
                ps = psb.tile([128, 512], F32, tag="psb", name="psb")
                for kh in range(2):
                    rhs = xT[kh][:, 1:1 + 129 * 127 + 1:129]
                    nc.tensor.matmul(ps[:, 0:128],
                                     wih[d][kh][:, 128 * j:128 * j + 128],
                                     rhs, start=(kh == 0), stop=(kh == 1))
                dst = grv()[:, 3:4, c:c + 1, :]
                src = ps[:, 0:128].rearrange("p (a b l) -> p a b l", a=1, b=1)
                nc.vector.tensor_scalar(dst, src, bc[:, c:c + 1], None, ALU.add)

            units = []
            for sb in range((NSB - 1) // 16):
                for h2 in range(2):
                    for c in range(8):
                        units.append((emit_unit, sb, h2, c))
            for c in range(4, 8):
                units.append((emit_unit12, c))

            def inject(s):
                # load G_pre for step s into a fresh PSUM gate tile.
                # G layout: bank0 = [i0 f0 i1 f1], bank1 = [o0 g0 o1 g1]
                g = psg.tile([128, 1024], F32, tag="G", name="G")
                wf = s % RW
                blk = (wf // 4) * 4
                wb = blk + 3 - (s % 4)
                nc.tensor.matmul(g[:, 0:256], identb[:],
                                 gring[:, wf * 1024:wf * 1024 + 256],
                                 start=True, stop=False)
                nc.tensor.matmul(g[:, 256:512], identb[:],
                                 gring[:, wb * 1024 + 512:wb * 1024 + 768],
                                 start=False, stop=False)
                nc.tensor.matmul(g[:, 512:768], identb[:],
                                 gring[:, wf * 1024 + 256:wf * 1024 + 512],
                                 start=True, stop=False)
                nc.tensor.matmul(g[:, 768:1024], identb[:],
                                 gring[:, wb * 1024 + 768:wb * 1024 + 1024],
                                 start=False, stop=False)
                return g

            for u in units[:9]:
                u[0](*u[1:])
            g_cur = inject(0)

            # G column offset for gate j (i,f,o,g) of dir d
            def gcol(d, j):
                return (256 * d + 128 * j if j < 2
                        else 512 + 256 * d + 128 * (j - 2))

            hs_prev = None
            for s in range(NSB):
                g = g_cur
                # w_hh matmuls: bank0 gates (f, i) first so sig_if starts early
                for j in (1, 0, 3, 2):
                    for d in range(2):
                        if d == 0 and s >= NSF:
                            continue
                        hs = h0[d][:] if s == 0 else hs_prev[:, 128 * d:128 * d + 128]
                        co = gcol(d, j)
                        nc.tensor.matmul(
                            g[:, co:co + 128],
                            whh[d][:, 128 * j:128 * j + 128], hs,
                            start=False, stop=True)
                if s + 1 < NSB:
                    g_cur = inject(s + 1)
                # gates: sig_if = one contiguous op over bank0
                sig = sigp.tile([128, 768], BF, tag="sig", name="sig")
                nc.scalar.activation(sig[:, 0:512], g[:, 0:512], AF.Sigmoid)
                gq = g[:, 512:1024].rearrange("p (a q) -> p a q", q=256)
                tg = tgp.tile([128, 256], BF, tag="tg", name="tg")
                nc.scalar.activation(tg[:].rearrange("p (a q) -> p a q", q=128),
                                     gq[:, :, 128:256], AF.Tanh)
                nc.scalar.activation(
                    sig[:, 512:768].rearrange("p (a q) -> p a q", q=128),
                    gq[:, :, 0:128], AF.Sigmoid)
                # c update
                sigr = sig[:, 0:512].rearrange("p (a q) -> p a q", q=256)
                t1 = t1p.tile([128, 256], BF, tag="t1", name="t1")
                t2 = t2p.tile([128, 256], BF, tag="t2", name="t2")
                cr = CFB[:].rearrange("p (a q) -> p a q", q=128)
                nc.vector.tensor_tensor(t1[:].rearrange("p (a q) -> p a q", q=128),
                                        sigr[:, :, 128:256], cr, ALU.mult)
                nc.vector.tensor_tensor(t2[:].rearrange("p (a q) -> p a q", q=128),
                                        sigr[:, :, 0:128],
                                        tg[:].rearrange("p (a q) -> p a q", q=128),
                                        ALU.mult)
                nc.vector.tensor_tensor(CFB[:], t1[:], t2[:], ALU.add)
                tcn = tcp.tile([128, 256], BF, tag="tcn", name="tcn")
                nc.scalar.activation(tcn[:], CFB[:], AF.Tanh)
                # h = sigma_o * tanh(c) -> contiguous scratch (fast DVE write)
                hs_cur = hscp.tile([128, 256], BF, tag="hsc", name="hsc")
                for d in range(2):
                    if d == 0 and s >= NSF:
                        continue
                    nc.vector.tensor_tensor(hs_cur[:, 128 * d:128 * d + 128],
                                            sig[:, 512 + 128 * d:640 + 128 * d],
                                            tcn[:, 128 * d:128 * d + 128],
                                            ALU.mult)
                # off-critical-path strided copies into token-major h stores
                if s >= B:
                    if s < NSF:
                        p_ = s - B
                        nc.gpsimd.tensor_copy(
                            hfT[:, p_:p_ + 127 * 128 + 1:128],
                            hs_cur[:, 0:128])
                    a = 128 + B - s
                    nc.gpsimd.tensor_copy(
                        hbT[:, a:a + 129 * 127 + 1:129],
                        hs_cur[:, 128:256])
                if s < B:
                    nc.gpsimd.tensor_copy(hf_head[:, 64 - B + s:65 - B + s],
                                          hs_cur[:, 0:1])
                    nc.gpsimd.tensor_copy(hb_head[:, B - 1 - s:B - s],
                                          hs_cur[:, 254:255])
                hs_prev = hs_cur
                if 9 + s < len(units):
                    u = units[9 + s]
                    u[0](*u[1:])

        # ---------------- attention phase ----------------
        # assemble extra window tiles
        nc.vector.tensor_copy(hfx[:, 0:64], hf_head[:])
        nc.vector.tensor_copy(hfx[:, 64:128], hfT[:, 16256:16320])
        nc.vector.tensor_copy(hbx[:, 0:64], hbT[:, 63:127])
        nc.vector.tensor_copy(hbx[:, 64:128], hb_head[:])

        with tc.tile_pool(name="psU", bufs=2, space="PSUM") as psu, \
             tc.tile_pool(name="uT", bufs=2) as utp, \
             tc.tile_pool(name="psA", bufs=2, space="PSUM") as psa:
            for gidx in range(33):
                if gidx < 32:
                    n = 512
                    hfr = hfT[:, 512 * gidx:512 * gidx + 512]
                    hbr = hbT[:, 512 * gidx + 127:512 * gidx + 127 + 512]
                    aout = att_dram[0:1, 512 * gidx:512 * gidx + 512]
                else:
                    n = 128
                    hfr = hfx[:]
                    hbr = hbx[:]
                    aout = att_dram[0:1, NQ:NQ + 128]
                pa = psa.tile([1, 512], F32, tag="psA", name="psA")
                for c2 in range(2):
                    pu = psu.tile([128, 512], F32, tag="psU", name="psU")
                    nc.tensor.matmul(pu[:, 0:n], wom[0][:, 128 * c2:128 * c2 + 128],
                                     hfr, start=True, stop=False)
                    nc.tensor.matmul(pu[:, 0:n], wom[1][:, 128 * c2:128 * c2 + 128],
                                     hbr, start=False, stop=True)
                    ut = utp.tile([128, 512], BF, tag="uT", name="uT")
                    nc.scalar.activation(ut[:, 0:n], pu[:, 0:n], AF.Tanh)
                    nc.tensor.matmul(pa[0:1, 0:n], uo[c2][:], ut[:, 0:n],
                                     start=(c2 == 0), stop=(c2 == 1))
                asb = utp.tile([1, 512], F32, tag="asb", name="asb")
                nc.vector.tensor_copy(asb[0:1, 0:n], pa[0:1, 0:n])
                nc.sync.dma_start(aout, asb[0:1, 0:n])

        # att -> column-major e
        with tc.tile_pool(name="psT", bufs=2, space="PSUM") as pst, \
             tc.tile_pool(name="anm", bufs=1) as anmp:
            att_nm = anmp.tile([128, 128], F32, tag="anm", name="anm")
            nc.sync.dma_start(
                att_nm[:],
                att_dram[0:1, 0:NQ].rearrange("a (n p) -> (a n) p", p=128))
            ps_a = pst.tile([128, 128], F32, tag="psT", name="psT")
            nc.tensor.transpose(ps_a[:], att_nm[:], identf[:])
            nc.scalar.activation(e_cm[:, 0:128], ps_a[:], AF.Exp)
            att_x = anmp.tile([128, 1], F32, tag="attx", name="attx")
            nc.sync.dma_start(
                att_x[:],
                att_dram[0:1, NQ:NQ + 128].rearrange("a (n p) -> (a n) p", p=1))
            nc.scalar.activation(e_cm[:, 128:129], att_x[:], AF.Exp)

        # ragged context accumulation
        with tc.tile_pool(name="psT2", bufs=2, space="PSUM") as pst2, \
             tc.tile_pool(name="yp", bufs=2) as yp, \
             tc.tile_pool(name="iw", bufs=2) as iwp, \
             tc.tile_pool(name="psC", bufs=1, space="PSUM") as psc:
            ctxp = [psc.tile([128, 257], F32, tag=f"ctxp{k}", name=f"ctxp{k}") for k in range(2)]
            for nti in range(NTILE + 1):
                if nti < NTILE:
                    hfr = hfT[:, 128 * nti:128 * nti + 128]
                    hbr = hbT[:, 128 * nti + 127:128 * nti + 255]
                else:
                    hfr = hfx[:]
                    hbr = hbx[:]
                ps_t = pst2.tile([128, 256], BF, tag="psT2", name="psT2")
                nc.tensor.transpose(ps_t[:, 0:128], hfr, identb[:])
                nc.tensor.transpose(ps_t[:, 128:256], hbr, identb[:])
                y = yp.tile([128, 257], BF, tag="y", name="y")
                nc.vector.tensor_copy(y[:, 0:256], ps_t[:])
                nc.vector.memset(y[:, 256:257], 1.0)
                iw = iwp.tile([128, 256], BF, tag="iw", name="iw")
                nc.vector.tensor_scalar(iw[:], iota_t[:],
                                        seg_t[:, nti:nti + 1],
                                        e_cm[:, nti:nti + 1],
                                        ALU.is_equal, ALU.mult)
                for k in range(2):
                    nc.tensor.matmul(ctxp[k][:], iw[:, 128 * k:128 * k + 128],
                                     y[:], start=(nti == 0), stop=(nti == NTILE))
            for k in range(2):
                nc.vector.tensor_copy(ctx_sb[k][:], ctxp[k][:])
        for k in range(2):
            nc.sync.dma_start(ctx_out[128 * k:128 * k + 128, :], ctx_sb[k][:])

    nc.finalize()
    _BUILT["nc"] = nc
    return nc


def _host_prep(inputs):
    x = np.asarray(inputs["sentence"], np.float32)
    doc_mask = np.asarray(inputs["doc_mask"]).astype(np.int64)
    h0g = np.asarray(inputs["h0"], np.float32)
    c0g = np.asarray(inputs["c0"], np.float32)

    perm = np.r_[0:128, 128:256, 384:512, 256:384]  # i,f,o,g order

    def wprep(w):  # [4H, X] -> lhsT [X, 4H] with gate perm, bf16
        return np.ascontiguousarray(w.astype(np.float32).T[:, perm]).astype(BF16)

    wih = {d: wprep(np.asarray(inputs[f"w_ih_{s}"], np.float32))
           for d, s in ((0, "f"), (1, "b"))}
    whh = {d: wprep(np.asarray(inputs[f"w_hh_{s}"], np.float32))
           for d, s in ((0, "f"), (1, "b"))}
    bias = {d: (np.asarray(inputs[f"b_ih_{s}"], np.float32)
                + np.asarray(inputs[f"b_hh_{s}"], np.float32))[perm]
            for d, s in ((0, "f"), (1, "b"))}
    bc = np.zeros((128, 8), np.float32)
    for d in range(2):
        for j in range(4):
            bc[:, d * 4 + j] = bias[d][128 * j:128 * j + 128]

    wom = np.asarray(inputs["w_omega"], np.float32).astype(BF16)
    uo = np.asarray(inputs["u_omega"], np.float32).astype(BF16)
    iota = np.tile(np.arange(256, dtype=np.float32), (128, 1))
    identb = np.eye(128, dtype=np.float32).astype(BF16)
    identf = np.eye(128, dtype=np.float32)

    seg_global = np.searchsorted(doc_mask, np.arange(T), side="right")

    # exact h/c for the first/last (64-B) tokens, evolved on host
    def _sig(v):
        return 1.0 / (1.0 + np.exp(-v))

    def _lstm_steps(x_seq, w_ih, w_hh, b, h, c):
        hs = []
        for t in range(x_seq.shape[0]):
            gv = x_seq[t] @ w_ih.T + h @ w_hh.T + b
            ig, fg, gg, og = np.split(gv, 4)
            c = _sig(fg) * c + _sig(ig) * np.tanh(gg)
            h = _sig(og) * np.tanh(c)
            hs.append(h)
        return np.stack(hs), h, c

    NH = 64 - B
    wraw = {s: (np.asarray(inputs[f"w_ih_{s}"], np.float32),
                np.asarray(inputs[f"w_hh_{s}"], np.float32),
                np.asarray(inputs[f"b_ih_{s}"], np.float32)
                + np.asarray(inputs[f"b_hh_{s}"], np.float32))
            for s in ("f", "b")}
    hs_pre, hF, cF = _lstm_steps(x[0:NH], *wraw["f"], h0g[0], c0g[0])
    hs_suf, hBs, cBs = _lstm_steps(x[T - NH:][::-1], *wraw["b"], h0g[1], c0g[1])

    in_maps = []
    s_los = []
    xpad = np.zeros((T + 512, D), np.float32)
    xpad[64:64 + T] = x  # global row r ↔ token r - 64
    for c in range(NCORE):
        tc0 = c * PC
        xs = xpad[tc0:tc0 + SH]  # token tc0-64+i at row i
        xT = np.ascontiguousarray(xs.T).astype(BF16)

        # seeds (boundary lanes get the host-evolved exact state)
        h0f = np.zeros((128, 128), np.float32)
        c0f = np.zeros((128, 128), np.float32)
        h0b = np.zeros((128, 128), np.float32)
        c0b = np.zeros((128, 128), np.float32)
        hfh = np.zeros((128, NH), np.float32)
        hbh = np.zeros((128, NH), np.float32)
        if c == 0:
            h0f[:, 0] = hF
            c0f[:, 0] = cF
            hfh = hs_pre.T
        if c == NCORE - 1:
            h0b[:, 126] = hBs
            c0b[:, 126] = cBs
            hbh = hs_suf[::-1].T

        # segment ids, col-major [128, 129]
        segm = np.full((128, 129), -1.0, np.float32)
        toks_main = tc0 + 64 + np.arange(NQ)
        valid = toks_main < T
        if c == NCORE - 1:
            valid &= (np.arange(NQ) < 16256)  # tail handled by W_tail
        toks_extra = np.full(128, -1, np.int64)
        if c == 0:
            toks_extra[0:64] = np.arange(64)          # W_head: tokens [0,64)
        if c == NCORE - 1:
            toks_extra[64:128] = T - 64 + np.arange(64)  # W_tail
        all_toks = np.concatenate([toks_main[valid],
                                   toks_extra[toks_extra >= 0]])
        s_lo = int(seg_global[all_toks].min()) if all_toks.size else 0
        s_hi = int(seg_global[all_toks].max()) if all_toks.size else 0
        assert s_hi - s_lo < SWIN, f"segment window too wide: {s_hi - s_lo}"
        s_los.append(s_lo)
        sm = np.where(valid, seg_global[np.minimum(toks_main, T - 1)] - s_lo,
                      -1.0).astype(np.float32)
        segm[:, 0:128] = sm.reshape(128, 128).T  # segm[p, n] = seg(q=128n+p)
        se = np.full(128, -1.0, np.float32)
        mask_x = toks_extra >= 0
        se[mask_x] = seg_global[toks_extra[mask_x]] - s_lo
        segm[:, 128] = se

        in_maps.append({
            "xT": xT,
            "wih_f": wih[0], "wih_b": wih[1],
            "whh_f": whh[0], "whh_b": whh[1],
            "bc": bc,
            "h0f": h0f.astype(BF16), "c0f": c0f.astype(BF16),
            "h0b": h0b.astype(BF16), "c0b": c0b.astype(BF16),
            "hfh": np.ascontiguousarray(hfh).astype(BF16),
            "hbh": np.ascontiguousarray(hbh).astype(BF16),
            "wom": wom, "uo": uo, "iota": iota,
            "identb": identb, "identf": identf,
            "seg": segm,
        })
    return in_maps, s_los


def kernel(**inputs):
    global LAST_RESULT
    from concourse.bass_utils import run_bass_kernel_spmd

    nc = _build()
    in_maps, s_los = _host_prep(inputs)
    res = run_bass_kernel_spmd(nc, in_maps, core_ids=list(range(NCORE)))
    LAST_RESULT = res

    G = np.zeros((S + SWIN, 257), np.float64)
    for c in range(NCORE):
        ctx = np.asarray(res.results[c]["ctx"], np.float32)
        G[s_los[c]:s_los[c] + SWIN] += ctx
    G = G[:S]
    z = G[:, 256]
    ctx = G[:, :256] / np.where(z == 0, 1.0, z)[:, None]
    w_tag = np.asarray(inputs["w_tag"], np.float32)
    b_tag = np.asarray(inputs["b_tag"], np.float32)
    out = ctx.astype(np.float32) @ w_tag.T + b_tag
    return out.astype(np.float32)



# revision 44
# speedup vs baseline: 1.9270x; 1.0601x over previous
"""Bass/Trainium2 kernel for nn_BiLSTM_Tok_83837761618147.

Strategy (8 NeuronCores, SPMD, full inputs in / full output out):
  - Token dim sharded 8 ways (16384 tokens/core, with halos).
  - BiLSTM parallelized via chunked recurrence with burn-in: each core runs
    128 lanes x (128+64) steps forward and 128 lanes x (129+64) steps
    backward (state forgets exponentially; 64 warmup steps reach fp32
    accuracy; the true h0/c0-seeded lanes cover the sequence ends exactly).
  - Gate pre-activations computed by PE matmuls directly into PSUM
    (bias via a K=4 indicator matmul); w_hh @ h accumulated on top.
  - Attention (tanh/logits/exp) + ragged segment softmax-sum done on
    device via an e-weighted one-hot (token x segment-window) matmul.
  - Host combines per-core partial [segment, 257] sums, normalizes, and
    applies the tiny tag projection.
"""

import numpy as np
import ml_dtypes

BF16 = ml_dtypes.bfloat16

T = 131072
D = 256
H = 128
HID = 256
TAGS = 10
S = 1024
NCORE = 8
PC = T // NCORE          # 16384 tokens per core
B = 32                   # burn-in steps (first/last B tokens computed on host)
LF = 128                 # forward lane length (tokens per lane)
LB = 129                 # backward lane length
NL = 128                 # lanes per direction
NSF = B + LF             # 192 forward steps
NSB = B + LB             # 193 backward steps
SH = 16704               # x shard rows [tc0-64, tc0-64+SH)
SWIN = 256               # segment window width per core
NQ = PC                  # main attention window positions
NTILE = NQ // 128        # 128 main token tiles
HBW = LB * NL - LB + LB + B  # hbT width: 16512
HBT_W = 16512
ATT_W = NQ + 128         # att buffer width (main + extra tile)
RW = 16                  # pre-gate ring depth (steps)

_BUILT = {}
LAST_RESULT = None


def _build():
    if "nc" in _BUILT:
        return _BUILT["nc"]
    import contextlib
    from concourse import bacc, mybir
    from concourse.tile import TileContext

    F32 = mybir.dt.float32
    BF = mybir.dt.bfloat16
    AF = mybir.ActivationFunctionType
    ALU = mybir.AluOpType

    nc = bacc.Bacc()

    def din(name, shape, dt):
        return nc.declare_dram_parameter(name, list(shape), dt, isOutput=False)

    x_in = din("xT", [256, SH], BF)
    wih_f_in = din("wih_f", [256, 512], BF)
    wih_b_in = din("wih_b", [256, 512], BF)
    whh_f_in = din("whh_f", [128, 512], BF)
    whh_b_in = din("whh_b", [128, 512], BF)
    bc_in = din("bc", [128, 8], F32)
    h0f_in = din("h0f", [128, 128], BF)
    c0f_in = din("c0f", [128, 128], BF)
    h0b_in = din("h0b", [128, 128], BF)
    c0b_in = din("c0b", [128, 128], BF)
    hfh_in = din("hfh", [128, 64 - B], BF)
    hbh_in = din("hbh", [128, 64 - B], BF)
    wom_in = din("wom", [256, 256], BF)
    uo_in = din("uo", [256, 1], BF)
    iota_in = din("iota", [128, 256], BF)
    identb_in = din("identb", [128, 128], BF)
    seg_in = din("seg", [128, 129], F32)
    ctx_out = nc.declare_dram_parameter("ctx", [256, 257], F32, isOutput=True)

    with TileContext(nc) as tc, contextlib.ExitStack() as ctx:
        pp = ctx.enter_context(tc.tile_pool(name="persist", bufs=1))

        xT0 = pp.tile([128, SH], BF, tag="xT0", name="xT0")
        xT1 = pp.tile([128, SH], BF, tag="xT1", name="xT1")
        hfT = pp.tile([128, NQ], BF, tag="hfT", name="hfT")
        hbT = pp.tile([128, HBT_W], BF, tag="hbT", name="hbT")
        hf_head = pp.tile([128, 64], BF, tag="hfh", name="hfh")
        hb_head = pp.tile([128, 64], BF, tag="hbh", name="hbh")
        wih = [[pp.tile([128, 512], BF, tag=f"wih{d}{k}", name=f"wih{d}{k}") for k in range(2)]
               for d in range(2)]
        whh = [pp.tile([128, 512], BF, tag=f"whh{d}", name=f"whh{d}") for d in range(2)]
        bc = pp.tile([128, 8], F32, tag="bc", name="bc")
        gring = pp.tile([128, 8 * 128 * RW], BF, tag="gring", name="gring")
        h0 = [pp.tile([128, 128], BF, tag=f"h0{d}", name=f"h0{d}") for d in range(2)]
        c0 = [pp.tile([128, 128], BF, tag=f"c0{d}", name=f"c0{d}") for d in range(2)]
        wom = [pp.tile([128, 256], BF, tag=f"wom{k}", name=f"wom{k}") for k in range(2)]
        uo = [pp.tile([128, 1], BF, tag=f"uo{k}", name=f"uo{k}") for k in range(2)]
        iota_t = pp.tile([128, 256], BF, tag="iota", name="iota")
        identb = pp.tile([128, 128], BF, tag="identb", name="identb")
        seg_t = pp.tile([128, 129], F32, tag="seg", name="seg")
        CFB = pp.tile([128, 256], BF, tag="CFB", name="CFB")
        e_cm = pp.tile([128, 129], F32, tag="ecm", name="ecm")
        hfx = pp.tile([128, 128], BF, tag="hfx", name="hfx")
        hbx = pp.tile([128, 128], BF, tag="hbx", name="hbx")
        ctx_sb = [pp.tile([128, 257], F32, tag=f"ctxsb{k}", name=f"ctxsb{k}") for k in range(2)]

        # ---- input DMAs ----
        nc.sync.dma_start(xT0[:], x_in[0:128, :])
        nc.sync.dma_start(xT1[:], x_in[128:256, :])
        for d, t_ in ((0, wih_f_in), (1, wih_b_in)):
            nc.sync.dma_start(wih[d][0][:], t_[0:128, :])
            nc.sync.dma_start(wih[d][1][:], t_[128:256, :])
        nc.sync.dma_start(whh[0][:], whh_f_in[:])
        nc.sync.dma_start(whh[1][:], whh_b_in[:])
        nc.sync.dma_start(bc[:], bc_in[:])
        nc.sync.dma_start(h0[0][:], h0f_in[:])
        nc.sync.dma_start(c0[0][:], c0f_in[:])
        nc.sync.dma_start(h0[1][:], h0b_in[:])
        nc.sync.dma_start(c0[1][:], c0b_in[:])
        nc.sync.dma_start(wom[0][:], wom_in[0:128, :])
        nc.sync.dma_start(wom[1][:], wom_in[128:256, :])
        nc.sync.dma_start(uo[0][:], uo_in[0:128, :])
        nc.sync.dma_start(uo[1][:], uo_in[128:256, :])
        nc.sync.dma_start(iota_t[:], iota_in[:])
        nc.sync.dma_start(identb[:], identb_in[:])
        nc.sync.dma_start(seg_t[:], seg_in[:])
        # host-computed exact h for the first/last (64-B) tokens (cores 0 / 7)
        nc.sync.dma_start(hf_head[:, 0:64 - B], hfh_in[:])
        nc.sync.dma_start(hb_head[:, B:64], hbh_in[:])

        # init cell state from seeds: CFB = [c0f | c0b]
        nc.vector.tensor_copy(CFB[:, 0:128], c0[0][:])
        nc.vector.tensor_copy(CFB[:, 128:256], c0[1][:])

        xT = [xT0, xT1]

        def grv():
            # slot-major ring: col = w*1024 + c*128 + l
            return gring[:].rearrange("p (w c l) -> p w c l", w=RW, c=8)

        with tc.tile_pool(name="psG", bufs=2, space="PSUM") as psg, \
             tc.tile_pool(name="psB", bufs=4, space="PSUM") as psb, \
             tc.tile_pool(name="sig", bufs=3) as sigp, \
             tc.tile_pool(name="tg", bufs=3) as tgp, \
             tc.tile_pool(name="tcn", bufs=3) as tcp, \
             tc.tile_pool(name="tmp1", bufs=3) as t1p, \
             tc.tile_pool(name="tmp2", bufs=3) as t2p, \
             tc.tile_pool(name="hsc", bufs=4) as hscp:

            # ---- pre-gate batch units: G_pre = x @ w_ih.T + b, evacuated to
            # the bf16 ring `gring` 16 steps ahead of consumption ----
            def emit_unit(sb, h2, c):
                d, j = divmod(c, 4)
                ps = [psb.tile([128, 512], F32, tag="psb", name="psb")
                      for _ in range(2)]
                for kh in range(2):
                    for b_ in range(2):
                        s0 = sb * 16 + h2 * 8 + b_ * 4
                        if d == 0:
                            fb = (128 - B) + s0
                            rhs = xT[kh][:, fb:fb + 128 * 128].rearrange(
                                "p (l q) -> p l q", q=128)[:, :, 0:4]
                        else:
                            base = (126 + B) - s0
                            rhs = xT[kh][:, base:base + 129 * 128].rearrange(
                                "p (l q) -> p l q", q=129)[:, :, 0:4]
                        nc.tensor.matmul(ps[b_][:],
                                         wih[d][kh][:, 128 * j:128 * j + 128],
                                         rhs, start=(kh == 0), stop=(kh == 1))
                for b_ in range(2):
                    w0 = h2 * 8 + b_ * 4
                    dst = grv()[:, w0:w0 + 4, c:c + 1, :]
                    src = ps[b_][:].rearrange("p (l a q) -> p q a l", a=1, q=4)
                    if b_ == 0:
                        nc.vector.tensor_scalar(dst, src, bc[:, c:c + 1], None,
                                                ALU.add)
                    else:
                        nc.scalar.activation(dst, src, AF.Identity,
                                             bias=bc[:, c:c + 1])

            def emit_unit12(c):
                # step 192, bwd chunks only
                d, j = divmod(c, 4)
                ps = psb.tile([128, 512], F32, tag="psb", name="psb")
                for kh in range(2):
                    rhs = xT[kh][:, 1:1 + 129 * 127 + 1:129]
                    nc.tensor.matmul(ps[:, 0:128],
                                     wih[d][kh][:, 128 * j:128 * j + 128],
                                     rhs, start=(kh == 0), stop=(kh == 1))
                dst = grv()[:, 3:4, c:c + 1, :]
                src = ps[:, 0:128].rearrange("p (a b l) -> p a b l", a=1, b=1)
                nc.vector.tensor_scalar(dst, src, bc[:, c:c + 1], None, ALU.add)

            units = []
            for sb in range((NSB - 1) // 16):
                for h2 in range(2):
                    for c in range(8):
                        units.append((emit_unit, sb, h2, c))
            for c in range(4, 8):
                units.append((emit_unit12, c))

            def inject(s):
                # load G_pre for step s into a fresh PSUM gate tile.
                # G layout: bank0 = [i0 f0 i1 f1], bank1 = [o0 g0 o1 g1]
                g = psg.tile([128, 1024], F32, tag="G", name="G")
                wf = s % RW
                blk = (wf // 4) * 4
                wb = blk + 3 - (s % 4)
                nc.tensor.matmul(g[:, 0:256], identb[:],
                                 gring[:, wf * 1024:wf * 1024 + 256],
                                 start=True, stop=False)
                nc.tensor.matmul(g[:, 256:512], identb[:],
                                 gring[:, wb * 1024 + 512:wb * 1024 + 768],
                                 start=False, stop=False)
                nc.tensor.matmul(g[:, 512:768], identb[:],
                                 gring[:, wf * 1024 + 256:wf * 1024 + 512],
                                 start=True, stop=False)
                nc.tensor.matmul(g[:, 768:1024], identb[:],
                                 gring[:, wb * 1024 + 768:wb * 1024 + 1024],
                                 start=False, stop=False)
                return g

            for u in units[:9]:
                u[0](*u[1:])
            g_cur = inject(0)

            # G column offset for gate j (i,f,o,g) of dir d
            def gcol(d, j):
                return (256 * d + 128 * j if j < 2
                        else 512 + 256 * d + 128 * (j - 2))

            hs_prev = None
            for s in range(NSB):
                g = g_cur
                # w_hh matmuls: bank0 gates (f, i) first so sig_if starts early
                for j in (1, 0, 3, 2):
                    for d in range(2):
                        if d == 0 and s >= NSF:
                            continue
                        hs = h0[d][:] if s == 0 else hs_prev[:, 128 * d:128 * d + 128]
                        co = gcol(d, j)
                        nc.tensor.matmul(
                            g[:, co:co + 128],
                            whh[d][:, 128 * j:128 * j + 128], hs,
                            start=False, stop=True)
                if s + 1 < NSB:
                    g_cur = inject(s + 1)
                # gates: sig_if = one contiguous op over bank0
                sig = sigp.tile([128, 768], BF, tag="sig", name="sig")
                nc.scalar.activation(sig[:, 0:512], g[:, 0:512], AF.Sigmoid)
                gq = g[:, 512:1024].rearrange("p (a q) -> p a q", q=256)
                tg = tgp.tile([128, 256], BF, tag="tg", name="tg")
                nc.scalar.activation(tg[:].rearrange("p (a q) -> p a q", q=128),
                                     gq[:, :, 128:256], AF.Tanh)
                nc.scalar.activation(
                    sig[:, 512:768].rearrange("p (a q) -> p a q", q=128),
                    gq[:, :, 0:128], AF.Sigmoid)
                # c update
                sigr = sig[:, 0:512].rearrange("p (a q) -> p a q", q=256)
                t1 = t1p.tile([128, 256], BF, tag="t1", name="t1")
                t2 = t2p.tile([128, 256], BF, tag="t2", name="t2")
                cr = CFB[:].rearrange("p (a q) -> p a q", q=128)
                nc.vector.tensor_tensor(t1[:].rearrange("p (a q) -> p a q", q=128),
                                        sigr[:, :, 128:256], cr, ALU.mult)
                nc.vector.tensor_tensor(t2[:].rearrange("p (a q) -> p a q", q=128),
                                        sigr[:, :, 0:128],
                                        tg[:].rearrange("p (a q) -> p a q", q=128),
                                        ALU.mult)
                nc.vector.tensor_tensor(CFB[:], t1[:], t2[:], ALU.add)
                tcn = tcp.tile([128, 256], BF, tag="tcn", name="tcn")
                nc.scalar.activation(tcn[:], CFB[:], AF.Tanh)
                # h = sigma_o * tanh(c) -> contiguous scratch (fast DVE write)
                hs_cur = hscp.tile([128, 256], BF, tag="hsc", name="hsc")
                for d in range(2):
                    if d == 0 and s >= NSF:
                        continue
                    nc.vector.tensor_tensor(hs_cur[:, 128 * d:128 * d + 128],
                                            sig[:, 512 + 128 * d:640 + 128 * d],
                                            tcn[:, 128 * d:128 * d + 128],
                                            ALU.mult)
                # off-critical-path strided copies into token-major h stores
                if s >= B:
                    if s < NSF:
                        p_ = s - B
                        nc.gpsimd.tensor_copy(
                            hfT[:, p_:p_ + 127 * 128 + 1:128],
                            hs_cur[:, 0:128])
                    a = 128 + B - s
                    nc.gpsimd.tensor_copy(
                        hbT[:, a:a + 129 * 127 + 1:129],
                        hs_cur[:, 128:256])
                if s < B:
                    nc.gpsimd.tensor_copy(hf_head[:, 64 - B + s:65 - B + s],
                                          hs_cur[:, 0:1])
                    nc.gpsimd.tensor_copy(hb_head[:, B - 1 - s:B - s],
                                          hs_cur[:, 254:255])
                hs_prev = hs_cur
                if 9 + s < len(units):
                    u = units[9 + s]
                    u[0](*u[1:])

        # ---------------- fused attention + ragged phase ----------------
        # assemble extra window tiles
        nc.vector.tensor_copy(hfx[:, 0:64], hf_head[:])
        nc.vector.tensor_copy(hfx[:, 64:128], hfT[:, 16256:16320])
        nc.vector.tensor_copy(hbx[:, 0:64], hbT[:, 63:127])
        nc.vector.tensor_copy(hbx[:, 64:128], hb_head[:])

        with tc.tile_pool(name="psU", bufs=2, space="PSUM") as psu, \
             tc.tile_pool(name="uT", bufs=3) as utp, \
             tc.tile_pool(name="psE", bufs=2, space="PSUM") as pse, \
             tc.tile_pool(name="psT2", bufs=2, space="PSUM") as pst2, \
             tc.tile_pool(name="yp", bufs=3) as yp, \
             tc.tile_pool(name="iw", bufs=3) as iwp, \
             tc.tile_pool(name="psC", bufs=1, space="PSUM") as psc:
            ctxp = [psc.tile([128, 257], F32, tag=f"ctxp{k}", name=f"ctxp{k}")
                    for k in range(2)]
            for gidx in range(33):
                if gidx < 32:
                    n = 512
                    hfr = hfT[:, 512 * gidx:512 * gidx + 512]
                    hbr = hbT[:, 512 * gidx + 127:512 * gidx + 127 + 512]
                    ebase = 4 * gidx
                else:
                    n = 128
                    hfr = hfx[:]
                    hbr = hbx[:]
                    ebase = 128
                ntl = n // 128
                # u = tanh(x @ w_omega), feature-major
                ut = []
                for c2 in range(2):
                    pu = psu.tile([128, 512], F32, tag="psU", name="psU")
                    nc.tensor.matmul(pu[:, 0:n], wom[0][:, 128 * c2:128 * c2 + 128],
                                     hfr, start=True, stop=False)
                    nc.tensor.matmul(pu[:, 0:n], wom[1][:, 128 * c2:128 * c2 + 128],
                                     hbr, start=False, stop=True)
                    u_ = utp.tile([128, 512], BF, tag="uT", name="uT")
                    nc.scalar.activation(u_[:, 0:n], pu[:, 0:n], AF.Tanh)
                    ut.append(u_)
                # att logits token-on-partition: [128, ntl] column per tile
                pe_ = pse.tile([128, 4], F32, tag="psE", name="psE")
                for t_ in range(ntl):
                    for c2 in range(2):
                        nc.tensor.matmul(pe_[:, t_:t_ + 1],
                                         ut[c2][:, 128 * t_:128 * t_ + 128],
                                         uo[c2][:],
                                         start=(t_ == 0 and c2 == 0),
                                         stop=(t_ == ntl - 1 and c2 == 1))
                nc.scalar.activation(e_cm[:, ebase:ebase + ntl], pe_[:, 0:ntl],
                                     AF.Exp)
                # ragged context tiles of this group
                for t_ in range(ntl):
                    nti = ebase + t_
                    if nti < NTILE:
                        hfr_t = hfT[:, 128 * nti:128 * nti + 128]
                        hbr_t = hbT[:, 128 * nti + 127:128 * nti + 255]
                    else:
                        hfr_t = hfx[:]
                        hbr_t = hbx[:]
                    ps_t = pst2.tile([128, 256], BF, tag="psT2", name="psT2")
                    nc.tensor.transpose(ps_t[:, 0:128], hfr_t, identb[:])
                    nc.tensor.transpose(ps_t[:, 128:256], hbr_t, identb[:])
                    y = yp.tile([128, 257], BF, tag="y", name="y")
                    nc.scalar.activation(y[:, 0:256], ps_t[:], AF.Copy,
                                         scale=e_cm[:, nti:nti + 1])
                    nc.vector.tensor_copy(y[:, 256:257], e_cm[:, nti:nti + 1])
                    iw = iwp.tile([128, 256], BF, tag="iw", name="iw")
                    nc.vector.tensor_scalar(iw[:], iota_t[:],
                                            seg_t[:, nti:nti + 1], None,
                                            ALU.is_equal)
                    for k in range(2):
                        nc.tensor.matmul(ctxp[k][:], iw[:, 128 * k:128 * k + 128],
                                         y[:], start=(nti == 0),
                                         stop=(nti == NTILE))
            for k in range(2):
                nc.vector.tensor_copy(ctx_sb[k][:], ctxp[k][:])
        for k in range(2):
            nc.sync.dma_start(ctx_out[128 * k:128 * k + 128, :], ctx_sb[k][:])

    nc.finalize()
    _BUILT["nc"] = nc
    return nc


def _host_prep(inputs):
    x = np.asarray(inputs["sentence"], np.float32)
    doc_mask = np.asarray(inputs["doc_mask"]).astype(np.int64)
    h0g = np.asarray(inputs["h0"], np.float32)
    c0g = np.asarray(inputs["c0"], np.float32)

    perm = np.r_[0:128, 128:256, 384:512, 256:384]  # i,f,o,g order

    def wprep(w):  # [4H, X] -> lhsT [X, 4H] with gate perm, bf16
        return np.ascontiguousarray(w.astype(np.float32).T[:, perm]).astype(BF16)

    wih = {d: wprep(np.asarray(inputs[f"w_ih_{s}"], np.float32))
           for d, s in ((0, "f"), (1, "b"))}
    whh = {d: wprep(np.asarray(inputs[f"w_hh_{s}"], np.float32))
           for d, s in ((0, "f"), (1, "b"))}
    bias = {d: (np.asarray(inputs[f"b_ih_{s}"], np.float32)
                + np.asarray(inputs[f"b_hh_{s}"], np.float32))[perm]
            for d, s in ((0, "f"), (1, "b"))}
    bc = np.zeros((128, 8), np.float32)
    for d in range(2):
        for j in range(4):
            bc[:, d * 4 + j] = bias[d][128 * j:128 * j + 128]

    wom = np.asarray(inputs["w_omega"], np.float32).astype(BF16)
    uo = np.asarray(inputs["u_omega"], np.float32).astype(BF16)
    iota = np.tile(np.arange(256, dtype=np.float32), (128, 1)).astype(BF16)
    identb = np.eye(128, dtype=np.float32).astype(BF16)

    seg_global = np.searchsorted(doc_mask, np.arange(T), side="right")

    # exact h/c for the first/last (64-B) tokens, evolved on host
    def _sig(v):
        return 1.0 / (1.0 + np.exp(-v))

    def _lstm_steps(x_seq, w_ih, w_hh, b, h, c):
        hs = []
        for t in range(x_seq.shape[0]):
            gv = x_seq[t] @ w_ih.T + h @ w_hh.T + b
            ig, fg, gg, og = np.split(gv, 4)
            c = _sig(fg) * c + _sig(ig) * np.tanh(gg)
            h = _sig(og) * np.tanh(c)
            hs.append(h)
        return np.stack(hs), h, c

    NH = 64 - B
    wraw = {s: (np.asarray(inputs[f"w_ih_{s}"], np.float32),
                np.asarray(inputs[f"w_hh_{s}"], np.float32),
                np.asarray(inputs[f"b_ih_{s}"], np.float32)
                + np.asarray(inputs[f"b_hh_{s}"], np.float32))
            for s in ("f", "b")}
    hs_pre, hF, cF = _lstm_steps(x[0:NH], *wraw["f"], h0g[0], c0g[0])
    hs_suf, hBs, cBs = _lstm_steps(x[T - NH:][::-1], *wraw["b"], h0g[1], c0g[1])

    in_maps = []
    s_los = []
    xpad = np.zeros((T + 512, D), np.float32)
    xpad[64:64 + T] = x  # global row r ↔ token r - 64
    for c in range(NCORE):
        tc0 = c * PC
        xs = xpad[tc0:tc0 + SH]  # token tc0-64+i at row i
        xT = np.ascontiguousarray(xs.T).astype(BF16)

        # seeds (boundary lanes get the host-evolved exact state)
        h0f = np.zeros((128, 128), np.float32)
        c0f = np.zeros((128, 128), np.float32)
        h0b = np.zeros((128, 128), np.float32)
        c0b = np.zeros((128, 128), np.float32)
        hfh = np.zeros((128, NH), np.float32)
        hbh = np.zeros((128, NH), np.float32)
        if c == 0:
            h0f[:, 0] = hF
            c0f[:, 0] = cF
            hfh = hs_pre.T
        if c == NCORE - 1:
            h0b[:, 126] = hBs
            c0b[:, 126] = cBs
            hbh = hs_suf[::-1].T

        # segment ids, col-major [128, 129]
        segm = np.full((128, 129), -1.0, np.float32)
        toks_main = tc0 + 64 + np.arange(NQ)
        valid = toks_main < T
        if c == NCORE - 1:
            valid &= (np.arange(NQ) < 16256)  # tail handled by W_tail
        toks_extra = np.full(128, -1, np.int64)
        if c == 0:
            toks_extra[0:64] = np.arange(64)          # W_head: tokens [0,64)
        if c == NCORE - 1:
            toks_extra[64:128] = T - 64 + np.arange(64)  # W_tail
        all_toks = np.concatenate([toks_main[valid],
                                   toks_extra[toks_extra >= 0]])
        s_lo = int(seg_global[all_toks].min()) if all_toks.size else 0
        s_hi = int(seg_global[all_toks].max()) if all_toks.size else 0
        assert s_hi - s_lo < SWIN, f"segment window too wide: {s_hi - s_lo}"
        s_los.append(s_lo)
        sm = np.where(valid, seg_global[np.minimum(toks_main, T - 1)] - s_lo,
                      -1.0).astype(np.float32)
        segm[:, 0:128] = sm.reshape(128, 128).T  # segm[p, n] = seg(q=128n+p)
        se = np.full(128, -1.0, np.float32)
        mask_x = toks_extra >= 0
        se[mask_x] = seg_global[toks_extra[mask_x]] - s_lo
        segm[:, 128] = se

        in_maps.append({
            "xT": xT,
            "wih_f": wih[0], "wih_b": wih[1],
            "whh_f": whh[0], "whh_b": whh[1],
            "bc": bc,
            "h0f": h0f.astype(BF16), "c0f": c0f.astype(BF16),
            "h0b": h0b.astype(BF16), "c0b": c0b.astype(BF16),
            "hfh": np.ascontiguousarray(hfh).astype(BF16),
            "hbh": np.ascontiguousarray(hbh).astype(BF16),
            "wom": wom, "uo": uo, "iota": iota,
            "identb": identb,
            "seg": segm,
        })
    return in_maps, s_los


def kernel(**inputs):
    global LAST_RESULT
    from concourse.bass_utils import run_bass_kernel_spmd

    nc = _build()
    in_maps, s_los = _host_prep(inputs)
    res = run_bass_kernel_spmd(nc, in_maps, core_ids=list(range(NCORE)))
    LAST_RESULT = res

    G = np.zeros((S + SWIN, 257), np.float64)
    for c in range(NCORE):
        ctx = np.asarray(res.results[c]["ctx"], np.float32)
        G[s_los[c]:s_los[c] + SWIN] += ctx
    G = G[:S]
    z = G[:, 256]
    ctx = G[:, :256] / np.where(z == 0, 1.0, z)[:, None]
    w_tag = np.asarray(inputs["w_tag"], np.float32)
    b_tag = np.asarray(inputs["b_tag"], np.float32)
    out = ctx.astype(np.float32) @ w_tag.T + b_tag
    return out.astype(np.float32)



# revision 49
# speedup vs baseline: 2.1022x; 1.0909x over previous
"""Bass/Trainium2 kernel for nn_BiLSTM_Tok_83837761618147.

Strategy (8 NeuronCores, SPMD, full inputs in / full output out):
  - Token dim sharded 8 ways (16384 tokens/core, with halos).
  - BiLSTM parallelized via chunked recurrence with burn-in: each core runs
    128 lanes x (128+64) steps forward and 128 lanes x (129+64) steps
    backward (state forgets exponentially; 64 warmup steps reach fp32
    accuracy; the true h0/c0-seeded lanes cover the sequence ends exactly).
  - Gate pre-activations computed by PE matmuls directly into PSUM
    (bias via a K=4 indicator matmul); w_hh @ h accumulated on top.
  - Attention (tanh/logits/exp) + ragged segment softmax-sum done on
    device via an e-weighted one-hot (token x segment-window) matmul.
  - Host combines per-core partial [segment, 257] sums, normalizes, and
    applies the tiny tag projection.
"""

import numpy as np
import ml_dtypes

BF16 = ml_dtypes.bfloat16

T = 131072
D = 256
H = 128
HID = 256
TAGS = 10
S = 1024
NCORE = 8
PC = T // NCORE          # 16384 tokens per core
B = 16                   # burn-in steps (first/last 64-B tokens computed on host)
LF = 128                 # forward lane length (tokens per lane)
LB = 129                 # backward lane length
NL = 128                 # lanes per direction
NSF = B + LF             # 192 forward steps
NSB = B + LB             # 193 backward steps
SH = 16704               # x shard rows [tc0-64, tc0-64+SH)
SWIN = 256               # segment window width per core
NQ = PC                  # main attention window positions
NTILE = NQ // 128        # 128 main token tiles
HBW = LB * NL - LB + LB + B  # hbT width: 16512
HBT_W = 16512
ATT_W = NQ + 128         # att buffer width (main + extra tile)
RW = 16                  # pre-gate ring depth (steps)

_BUILT = {}
LAST_RESULT = None


def _build():
    if "nc" in _BUILT:
        return _BUILT["nc"]
    import contextlib
    from concourse import bacc, mybir
    from concourse.tile import TileContext

    F32 = mybir.dt.float32
    BF = mybir.dt.bfloat16
    AF = mybir.ActivationFunctionType
    ALU = mybir.AluOpType

    nc = bacc.Bacc()

    def din(name, shape, dt):
        return nc.declare_dram_parameter(name, list(shape), dt, isOutput=False)

    x_in = din("xT", [256, SH], BF)
    wih_f_in = din("wih_f", [256, 512], BF)
    wih_b_in = din("wih_b", [256, 512], BF)
    whh_f_in = din("whh_f", [128, 512], BF)
    whh_b_in = din("whh_b", [128, 512], BF)
    bc_in = din("bc", [128, 8], F32)
    h0f_in = din("h0f", [128, 128], BF)
    c0f_in = din("c0f", [128, 128], BF)
    h0b_in = din("h0b", [128, 128], BF)
    c0b_in = din("c0b", [128, 128], BF)
    hfh_in = din("hfh", [128, 64 - B], BF)
    hbh_in = din("hbh", [128, 64 - B], BF)
    wom_in = din("wom", [256, 256], BF)
    uo_in = din("uo", [256, 1], BF)
    iota_in = din("iota", [128, 256], BF)
    identb_in = din("identb", [128, 128], BF)
    seg_in = din("seg", [128, 129], F32)
    ctx_out = nc.declare_dram_parameter("ctx", [256, 257], F32, isOutput=True)

    with TileContext(nc) as tc, contextlib.ExitStack() as ctx:
        pp = ctx.enter_context(tc.tile_pool(name="persist", bufs=1))

        xT0 = pp.tile([128, SH], BF, tag="xT0", name="xT0")
        xT1 = pp.tile([128, SH], BF, tag="xT1", name="xT1")
        hfT = pp.tile([128, NQ], BF, tag="hfT", name="hfT")
        hbT = pp.tile([128, HBT_W], BF, tag="hbT", name="hbT")
        hf_head = pp.tile([128, 64], BF, tag="hfh", name="hfh")
        hb_head = pp.tile([128, 64], BF, tag="hbh", name="hbh")
        wih = [[pp.tile([128, 512], BF, tag=f"wih{d}{k}", name=f"wih{d}{k}") for k in range(2)]
               for d in range(2)]
        whh = [pp.tile([128, 512], BF, tag=f"whh{d}", name=f"whh{d}") for d in range(2)]
        bc = pp.tile([128, 8], F32, tag="bc", name="bc")
        gring = pp.tile([128, 8 * 128 * RW], BF, tag="gring", name="gring")
        h0 = [pp.tile([128, 128], BF, tag=f"h0{d}", name=f"h0{d}") for d in range(2)]
        c0 = [pp.tile([128, 128], BF, tag=f"c0{d}", name=f"c0{d}") for d in range(2)]
        wom = [pp.tile([128, 256], BF, tag=f"wom{k}", name=f"wom{k}") for k in range(2)]
        uo = [pp.tile([128, 1], BF, tag=f"uo{k}", name=f"uo{k}") for k in range(2)]
        iota_t = pp.tile([128, 256], BF, tag="iota", name="iota")
        identb = pp.tile([128, 128], BF, tag="identb", name="identb")
        seg_t = pp.tile([128, 129], F32, tag="seg", name="seg")
        CFB = pp.tile([128, 256], BF, tag="CFB", name="CFB")
        e_cm = pp.tile([128, 129], F32, tag="ecm", name="ecm")
        hfx = pp.tile([128, 128], BF, tag="hfx", name="hfx")
        hbx = pp.tile([128, 128], BF, tag="hbx", name="hbx")
        ctx_sb = [pp.tile([128, 257], F32, tag=f"ctxsb{k}", name=f"ctxsb{k}") for k in range(2)]

        # ---- input DMAs ----
        nc.sync.dma_start(xT0[:], x_in[0:128, :])
        nc.sync.dma_start(xT1[:], x_in[128:256, :])
        for d, t_ in ((0, wih_f_in), (1, wih_b_in)):
            nc.sync.dma_start(wih[d][0][:], t_[0:128, :])
            nc.sync.dma_start(wih[d][1][:], t_[128:256, :])
        nc.sync.dma_start(whh[0][:], whh_f_in[:])
        nc.sync.dma_start(whh[1][:], whh_b_in[:])
        nc.sync.dma_start(bc[:], bc_in[:])
        nc.sync.dma_start(h0[0][:], h0f_in[:])
        nc.sync.dma_start(c0[0][:], c0f_in[:])
        nc.sync.dma_start(h0[1][:], h0b_in[:])
        nc.sync.dma_start(c0[1][:], c0b_in[:])
        nc.sync.dma_start(wom[0][:], wom_in[0:128, :])
        nc.sync.dma_start(wom[1][:], wom_in[128:256, :])
        nc.sync.dma_start(uo[0][:], uo_in[0:128, :])
        nc.sync.dma_start(uo[1][:], uo_in[128:256, :])
        nc.sync.dma_start(iota_t[:], iota_in[:])
        nc.sync.dma_start(identb[:], identb_in[:])
        nc.sync.dma_start(seg_t[:], seg_in[:])
        # host-computed exact h for the first/last (64-B) tokens (cores 0 / 7)
        nc.sync.dma_start(hf_head[:, 0:64 - B], hfh_in[:])
        nc.sync.dma_start(hb_head[:, B:64], hbh_in[:])

        # init cell state from seeds: CFB = [c0f | c0b]
        nc.vector.tensor_copy(CFB[:, 0:128], c0[0][:])
        nc.vector.tensor_copy(CFB[:, 128:256], c0[1][:])

        xT = [xT0, xT1]

        def grv():
            # slot-major ring: col = w*1024 + c*128 + l
            return gring[:].rearrange("p (w c l) -> p w c l", w=RW, c=8)

        with tc.tile_pool(name="psG", bufs=2, space="PSUM") as psg, \
             tc.tile_pool(name="psB", bufs=4, space="PSUM") as psb, \
             tc.tile_pool(name="sig", bufs=3) as sigp, \
             tc.tile_pool(name="tg", bufs=3) as tgp, \
             tc.tile_pool(name="tcn", bufs=3) as tcp, \
             tc.tile_pool(name="tmp1", bufs=3) as t1p, \
             tc.tile_pool(name="tmp2", bufs=3) as t2p, \
             tc.tile_pool(name="hsc", bufs=4) as hscp:

            # ---- pre-gate batch units: G_pre = x @ w_ih.T + b, evacuated to
            # the bf16 ring `gring` 16 steps ahead of consumption ----
            def emit_unit(sb, h2, c):
                d, j = divmod(c, 4)
                ps = [psb.tile([128, 512], F32, tag="psb", name="psb")
                      for _ in range(2)]
                for kh in range(2):
                    for b_ in range(2):
                        s0 = sb * 16 + h2 * 8 + b_ * 4
                        if d == 0:
                            fb = (128 - B) + s0
                            rhs = xT[kh][:, fb:fb + 128 * 128].rearrange(
                                "p (l q) -> p l q", q=128)[:, :, 0:4]
                        else:
                            base = (126 + B) - s0
                            rhs = xT[kh][:, base:base + 129 * 128].rearrange(
                                "p (l q) -> p l q", q=129)[:, :, 0:4]
                        nc.tensor.matmul(ps[b_][:],
                                         wih[d][kh][:, 128 * j:128 * j + 128],
                                         rhs, start=(kh == 0), stop=(kh == 1))
                for b_ in range(2):
                    w0 = h2 * 8 + b_ * 4
                    dst = grv()[:, w0:w0 + 4, c:c + 1, :]
                    src = ps[b_][:].rearrange("p (l a q) -> p q a l", a=1, q=4)
                    if b_ == 0:
                        nc.vector.tensor_scalar(dst, src, bc[:, c:c + 1], None,
                                                ALU.add)
                    else:
                        nc.scalar.activation(dst, src, AF.Identity,
                                             bias=bc[:, c:c + 1])

            def emit_unit12(c):
                # step 192, bwd chunks only
                d, j = divmod(c, 4)
                ps = psb.tile([128, 512], F32, tag="psb", name="psb")
                for kh in range(2):
                    rhs = xT[kh][:, 1:1 + 129 * 127 + 1:129]
                    nc.tensor.matmul(ps[:, 0:128],
                                     wih[d][kh][:, 128 * j:128 * j + 128],
                                     rhs, start=(kh == 0), stop=(kh == 1))
                dst = grv()[:, 3:4, c:c + 1, :]
                src = ps[:, 0:128].rearrange("p (a b l) -> p a b l", a=1, b=1)
                nc.vector.tensor_scalar(dst, src, bc[:, c:c + 1], None, ALU.add)

            units = []
            for sb in range((NSB - 1) // 16):
                for h2 in range(2):
                    for c in range(8):
                        units.append((emit_unit, sb, h2, c))
            for c in range(4, 8):
                units.append((emit_unit12, c))

            def inject(s):
                # load G_pre for step s into a fresh PSUM gate tile.
                # G layout: bank0 = [i0 f0 i1 f1], bank1 = [o0 g0 o1 g1]
                g = psg.tile([128, 1024], F32, tag="G", name="G")
                wf = s % RW
                blk = (wf // 4) * 4
                wb = blk + 3 - (s % 4)
                nc.tensor.matmul(g[:, 0:256], identb[:],
                                 gring[:, wf * 1024:wf * 1024 + 256],
                                 start=True, stop=False)
                nc.tensor.matmul(g[:, 256:512], identb[:],
                                 gring[:, wb * 1024 + 512:wb * 1024 + 768],
                                 start=False, stop=False)
                nc.tensor.matmul(g[:, 512:768], identb[:],
                                 gring[:, wf * 1024 + 256:wf * 1024 + 512],
                                 start=True, stop=False)
                nc.tensor.matmul(g[:, 768:1024], identb[:],
                                 gring[:, wb * 1024 + 768:wb * 1024 + 1024],
                                 start=False, stop=False)
                return g

            for u in units[:9]:
                u[0](*u[1:])
            g_cur = inject(0)

            # G column offset for gate j (i,f,o,g) of dir d
            def gcol(d, j):
                return (256 * d + 128 * j if j < 2
                        else 512 + 256 * d + 128 * (j - 2))

            hs_prev = None
            for s in range(NSB):
                g = g_cur
                # w_hh matmuls: bank0 gates (f, i) first so sig_if starts early
                for j in (1, 0, 3, 2):
                    for d in range(2):
                        if d == 0 and s >= NSF:
                            continue
                        hs = h0[d][:] if s == 0 else hs_prev[:, 128 * d:128 * d + 128]
                        co = gcol(d, j)
                        nc.tensor.matmul(
                            g[:, co:co + 128],
                            whh[d][:, 128 * j:128 * j + 128], hs,
                            start=False, stop=True)
                if s + 1 < NSB:
                    g_cur = inject(s + 1)
                # gates: sig_if = one contiguous op over bank0
                sig = sigp.tile([128, 768], BF, tag="sig", name="sig")
                nc.scalar.activation(sig[:, 0:512], g[:, 0:512], AF.Sigmoid)
                gq = g[:, 512:1024].rearrange("p (a q) -> p a q", q=256)
                tg = tgp.tile([128, 256], BF, tag="tg", name="tg")
                nc.scalar.activation(tg[:].rearrange("p (a q) -> p a q", q=128),
                                     gq[:, :, 128:256], AF.Tanh)
                nc.scalar.activation(
                    sig[:, 512:768].rearrange("p (a q) -> p a q", q=128),
                    gq[:, :, 0:128], AF.Sigmoid)
                # c update
                sigr = sig[:, 0:512].rearrange("p (a q) -> p a q", q=256)
                t1 = t1p.tile([128, 256], BF, tag="t1", name="t1")
                t2 = t2p.tile([128, 256], BF, tag="t2", name="t2")
                cr = CFB[:].rearrange("p (a q) -> p a q", q=128)
                nc.vector.tensor_tensor(t1[:].rearrange("p (a q) -> p a q", q=128),
                                        sigr[:, :, 128:256], cr, ALU.mult)
                nc.vector.tensor_tensor(t2[:].rearrange("p (a q) -> p a q", q=128),
                                        sigr[:, :, 0:128],
                                        tg[:].rearrange("p (a q) -> p a q", q=128),
                                        ALU.mult)
                nc.vector.tensor_tensor(CFB[:], t1[:], t2[:], ALU.add)
                tcn = tcp.tile([128, 256], BF, tag="tcn", name="tcn")
                nc.scalar.activation(tcn[:], CFB[:], AF.Tanh)
                # h = sigma_o * tanh(c) -> contiguous scratch (fast DVE write)
                hs_cur = hscp.tile([128, 256], BF, tag="hsc", name="hsc")
                for d in range(2):
                    if d == 0 and s >= NSF:
                        continue
                    nc.vector.tensor_tensor(hs_cur[:, 128 * d:128 * d + 128],
                                            sig[:, 512 + 128 * d:640 + 128 * d],
                                            tcn[:, 128 * d:128 * d + 128],
                                            ALU.mult)
                # off-critical-path strided copies into token-major h stores
                if s >= B:
                    if s < NSF:
                        p_ = s - B
                        nc.gpsimd.tensor_copy(
                            hfT[:, p_:p_ + 127 * 128 + 1:128],
                            hs_cur[:, 0:128])
                    a = 128 + B - s
                    nc.gpsimd.tensor_copy(
                        hbT[:, a:a + 129 * 127 + 1:129],
                        hs_cur[:, 128:256])
                if s < B:
                    nc.gpsimd.tensor_copy(hf_head[:, 64 - B + s:65 - B + s],
                                          hs_cur[:, 0:1])
                    nc.gpsimd.tensor_copy(hb_head[:, B - 1 - s:B - s],
                                          hs_cur[:, 254:255])
                hs_prev = hs_cur
                if 9 + s < len(units):
                    u = units[9 + s]
                    u[0](*u[1:])

        # ---------------- fused attention + ragged phase ----------------
        # assemble extra window tiles
        nc.vector.tensor_copy(hfx[:, 0:64], hf_head[:])
        nc.vector.tensor_copy(hfx[:, 64:128], hfT[:, 16256:16320])
        nc.vector.tensor_copy(hbx[:, 0:64], hbT[:, 63:127])
        nc.vector.tensor_copy(hbx[:, 64:128], hb_head[:])

        with tc.tile_pool(name="psU", bufs=2, space="PSUM") as psu, \
             tc.tile_pool(name="uT", bufs=3) as utp, \
             tc.tile_pool(name="psE", bufs=2, space="PSUM") as pse, \
             tc.tile_pool(name="psT2", bufs=2, space="PSUM") as pst2, \
             tc.tile_pool(name="yp", bufs=3) as yp, \
             tc.tile_pool(name="iw", bufs=3) as iwp, \
             tc.tile_pool(name="psC", bufs=1, space="PSUM") as psc:
            ctxp = [psc.tile([128, 257], F32, tag=f"ctxp{k}", name=f"ctxp{k}")
                    for k in range(2)]
            for gidx in range(33):
                if gidx < 32:
                    n = 512
                    hfr = hfT[:, 512 * gidx:512 * gidx + 512]
                    hbr = hbT[:, 512 * gidx + 127:512 * gidx + 127 + 512]
                    ebase = 4 * gidx
                else:
                    n = 128
                    hfr = hfx[:]
                    hbr = hbx[:]
                    ebase = 128
                ntl = n // 128
                # u = tanh(x @ w_omega), feature-major
                ut = []
                for c2 in range(2):
                    pu = psu.tile([128, 512], F32, tag="psU", name="psU")
                    nc.tensor.matmul(pu[:, 0:n], wom[0][:, 128 * c2:128 * c2 + 128],
                                     hfr, start=True, stop=False)
                    nc.tensor.matmul(pu[:, 0:n], wom[1][:, 128 * c2:128 * c2 + 128],
                                     hbr, start=False, stop=True)
                    u_ = utp.tile([128, 512], BF, tag="uT", name="uT")
                    nc.scalar.activation(u_[:, 0:n], pu[:, 0:n], AF.Tanh)
                    ut.append(u_)
                # att logits token-on-partition: [128, ntl] column per tile
                pe_ = pse.tile([128, 4], F32, tag="psE", name="psE")
                for t_ in range(ntl):
                    for c2 in range(2):
                        nc.tensor.matmul(pe_[:, t_:t_ + 1],
                                         ut[c2][:, 128 * t_:128 * t_ + 128],
                                         uo[c2][:],
                                         start=(t_ == 0 and c2 == 0),
                                         stop=(t_ == ntl - 1 and c2 == 1))
                nc.scalar.activation(e_cm[:, ebase:ebase + ntl], pe_[:, 0:ntl],
                                     AF.Exp)
                # ragged context tiles of this group
                for t_ in range(ntl):
                    nti = ebase + t_
                    if nti < NTILE:
                        hfr_t = hfT[:, 128 * nti:128 * nti + 128]
                        hbr_t = hbT[:, 128 * nti + 127:128 * nti + 255]
                    else:
                        hfr_t = hfx[:]
                        hbr_t = hbx[:]
                    ps_t = pst2.tile([128, 256], BF, tag="psT2", name="psT2")
                    nc.tensor.transpose(ps_t[:, 0:128], hfr_t, identb[:])
                    nc.tensor.transpose(ps_t[:, 128:256], hbr_t, identb[:])
                    y = yp.tile([128, 257], BF, tag="y", name="y")
                    nc.scalar.activation(y[:, 0:256], ps_t[:], AF.Copy,
                                         scale=e_cm[:, nti:nti + 1])
                    nc.vector.tensor_copy(y[:, 256:257], e_cm[:, nti:nti + 1])
                    iw = iwp.tile([128, 256], BF, tag="iw", name="iw")
                    nc.vector.tensor_scalar(iw[:], iota_t[:],
                                            seg_t[:, nti:nti + 1], None,
                                            ALU.is_equal)
                    for k in range(2):
                        nc.tensor.matmul(ctxp[k][:], iw[:, 128 * k:128 * k + 128],
                                         y[:], start=(nti == 0),
                                         stop=(nti == NTILE))
            for k in range(2):
                nc.vector.tensor_copy(ctx_sb[k][:], ctxp[k][:])
        for k in range(2):
            nc.sync.dma_start(ctx_out[128 * k:128 * k + 128, :], ctx_sb[k][:])

    nc.finalize()
    _BUILT["nc"] = nc
    return nc


def _host_prep(inputs):
    x = np.asarray(inputs["sentence"], np.float32)
    doc_mask = np.asarray(inputs["doc_mask"]).astype(np.int64)
    h0g = np.asarray(inputs["h0"], np.float32)
    c0g = np.asarray(inputs["c0"], np.float32)

    perm = np.r_[0:128, 128:256, 384:512, 256:384]  # i,f,o,g order

    def wprep(w):  # [4H, X] -> lhsT [X, 4H] with gate perm, bf16
        return np.ascontiguousarray(w.astype(np.float32).T[:, perm]).astype(BF16)

    wih = {d: wprep(np.asarray(inputs[f"w_ih_{s}"], np.float32))
           for d, s in ((0, "f"), (1, "b"))}
    whh = {d: wprep(np.asarray(inputs[f"w_hh_{s}"], np.float32))
           for d, s in ((0, "f"), (1, "b"))}
    bias = {d: (np.asarray(inputs[f"b_ih_{s}"], np.float32)
                + np.asarray(inputs[f"b_hh_{s}"], np.float32))[perm]
            for d, s in ((0, "f"), (1, "b"))}
    bc = np.zeros((128, 8), np.float32)
    for d in range(2):
        for j in range(4):
            bc[:, d * 4 + j] = bias[d][128 * j:128 * j + 128]

    wom = np.asarray(inputs["w_omega"], np.float32).astype(BF16)
    uo = np.asarray(inputs["u_omega"], np.float32).astype(BF16)
    iota = np.tile(np.arange(256, dtype=np.float32), (128, 1)).astype(BF16)
    identb = np.eye(128, dtype=np.float32).astype(BF16)

    seg_global = np.searchsorted(doc_mask, np.arange(T), side="right")

    # exact h/c for the first/last (64-B) tokens, evolved on host
    def _sig(v):
        return 1.0 / (1.0 + np.exp(-v))

    def _lstm_steps(x_seq, w_ih, w_hh, b, h, c):
        hs = []
        for t in range(x_seq.shape[0]):
            gv = x_seq[t] @ w_ih.T + h @ w_hh.T + b
            ig, fg, gg, og = np.split(gv, 4)
            c = _sig(fg) * c + _sig(ig) * np.tanh(gg)
            h = _sig(og) * np.tanh(c)
            hs.append(h)
        return np.stack(hs), h, c

    NH = 64 - B
    wraw = {s: (np.asarray(inputs[f"w_ih_{s}"], np.float32),
                np.asarray(inputs[f"w_hh_{s}"], np.float32),
                np.asarray(inputs[f"b_ih_{s}"], np.float32)
                + np.asarray(inputs[f"b_hh_{s}"], np.float32))
            for s in ("f", "b")}
    hs_pre, hF, cF = _lstm_steps(x[0:NH], *wraw["f"], h0g[0], c0g[0])
    hs_suf, hBs, cBs = _lstm_steps(x[T - NH:][::-1], *wraw["b"], h0g[1], c0g[1])

    in_maps = []
    s_los = []
    xpad = np.zeros((T + 512, D), np.float32)
    xpad[64:64 + T] = x  # global row r ↔ token r - 64
    for c in range(NCORE):
        tc0 = c * PC
        xs = xpad[tc0:tc0 + SH]  # token tc0-64+i at row i
        xT = np.ascontiguousarray(xs.T).astype(BF16)

        # seeds (boundary lanes get the host-evolved exact state)
        h0f = np.zeros((128, 128), np.float32)
        c0f = np.zeros((128, 128), np.float32)
        h0b = np.zeros((128, 128), np.float32)
        c0b = np.zeros((128, 128), np.float32)
        hfh = np.zeros((128, NH), np.float32)
        hbh = np.zeros((128, NH), np.float32)
        if c == 0:
            h0f[:, 0] = hF
            c0f[:, 0] = cF
            hfh = hs_pre.T
        if c == NCORE - 1:
            h0b[:, 126] = hBs
            c0b[:, 126] = cBs
            hbh = hs_suf[::-1].T

        # segment ids, col-major [128, 129]
        segm = np.full((128, 129), -1.0, np.float32)
        toks_main = tc0 + 64 + np.arange(NQ)
        valid = toks_main < T
        if c == NCORE - 1:
            valid &= (np.arange(NQ) < 16256)  # tail handled by W_tail
        toks_extra = np.full(128, -1, np.int64)
        if c == 0:
            toks_extra[0:64] = np.arange(64)          # W_head: tokens [0,64)
        if c == NCORE - 1:
            toks_extra[64:128] = T - 64 + np.arange(64)  # W_tail
        all_toks = np.concatenate([toks_main[valid],
                                   toks_extra[toks_extra >= 0]])
        s_lo = int(seg_global[all_toks].min()) if all_toks.size else 0
        s_hi = int(seg_global[all_toks].max()) if all_toks.size else 0
        assert s_hi - s_lo < SWIN, f"segment window too wide: {s_hi - s_lo}"
        s_los.append(s_lo)
        sm = np.where(valid, seg_global[np.minimum(toks_main, T - 1)] - s_lo,
                      -1.0).astype(np.float32)
        segm[:, 0:128] = sm.reshape(128, 128).T  # segm[p, n] = seg(q=128n+p)
        se = np.full(128, -1.0, np.float32)
        mask_x = toks_extra >= 0
        se[mask_x] = seg_global[toks_extra[mask_x]] - s_lo
        segm[:, 128] = se

        in_maps.append({
            "xT": xT,
            "wih_f": wih[0], "wih_b": wih[1],
            "whh_f": whh[0], "whh_b": whh[1],
            "bc": bc,
            "h0f": h0f.astype(BF16), "c0f": c0f.astype(BF16),
            "h0b": h0b.astype(BF16), "c0b": c0b.astype(BF16),
            "hfh": np.ascontiguousarray(hfh).astype(BF16),
            "hbh": np.ascontiguousarray(hbh).astype(BF16),
            "wom": wom, "uo": uo, "iota": iota,
            "identb": identb,
            "seg": segm,
        })
    return in_maps, s_los


def kernel(**inputs):
    global LAST_RESULT
    from concourse.bass_utils import run_bass_kernel_spmd

    nc = _build()
    in_maps, s_los = _host_prep(inputs)
    res = run_bass_kernel_spmd(nc, in_maps, core_ids=list(range(NCORE)))
    LAST_RESULT = res

    G = np.zeros((S + SWIN, 257), np.float64)
    for c in range(NCORE):
        ctx = np.asarray(res.results[c]["ctx"], np.float32)
        G[s_los[c]:s_los[c] + SWIN] += ctx
    G = G[:S]
    z = G[:, 256]
    ctx = G[:, :256] / np.where(z == 0, 1.0, z)[:, None]
    w_tag = np.asarray(inputs["w_tag"], np.float32)
    b_tag = np.asarray(inputs["b_tag"], np.float32)
    out = ctx.astype(np.float32) @ w_tag.T + b_tag
    return out.astype(np.float32)

